# revision 20
# baseline (speedup 1.0000x reference)
"""BondFastAttention Trainium2 kernel (self-contained), v2.

Shapes (hardcoded from the problem spec):
  edge_attr [65536, 512] fp32, B=64 graphs x L=1024 bonds, HID=512, 8 heads x D=64.
  8 NeuronCores, data-parallel over graphs: G=8 graphs per core.

Device layout: transposed domain - features on partitions, tokens on free dim
for Q/K/kvout; tokens on partitions for the Wo/LayerNorm stage.

Key structure vs v1:
  - The V projection, Wr matmul, gk scaling and +q add are all folded into a
    single per-graph combined weight W''' = Wv^T (gk . Wr^T) + Wq^T, built on
    the PE (16 small matmuls + identity-add), so one X-stream produces
    relu-input directly.
  - Projection PSUM is consumed in place (ACT exp, Pool multiply) - no
    psum->sbuf copies for q/k/v.
  - Optional fp8 path: Q/K projections and softmax seg-sums run as fp8
    DoubleRow matmuls (K=256 per pass).
  - Output is written bf16 and upcast to f32 on the host.
"""
import numpy as np

HID = 512
HEADS = 8
D = 64
B = 64
L = 1024
SCALE = D ** -0.5
EPS = 1e-5
NCORES = 8
G = B // NCORES          # graphs per core
NCH = HID // 128         # 4 feature chunks (2 heads each)
NT = L // 128            # 8 token chunks
SW = 16.0                # fp8 weight prescale

USE_FP8 = False


def _build(apply_bo: bool, apply_affine: bool, use_fp8: bool):
    import concourse.bass as bass
    from concourse import bacc
    import concourse.mybir as mybir
    from concourse.tile import TileContext

    F32 = mybir.dt.float32
    F32R = mybir.dt.float32r
    BF16 = mybir.dt.bfloat16
    FP8 = mybir.dt.float8e4
    AT = mybir.ActivationFunctionType
    OP = mybir.AluOpType
    PM = mybir.MatmulPerfMode

    nc = bacc.Bacc()

    import concourse.bacc as _bacc_mod
    _orig_gat = _bacc_mod.get_activation_tables

    def _gat(arch):
        # Keep dict order but strip our funcs from every other set, so the
        # table-load pass assigns all of them to natural_log_exp_and_others
        # -> a single physical table load.
        t = _orig_gat(arch)
        ours = {AT.Exp, AT.Ln, AT.Copy, AT.Relu, AT.Identity}
        out = {}
        for k, funcs in t.items():
            if k == "natural_log_exp_and_others":
                out[k] = funcs
            else:
                out[k] = {f for f in funcs if f not in ours}
        return out

    # ---------------- dram tensors ----------------
    xtb = nc.dram_tensor("xtb", [HID, G * L], BF16, kind="ExternalInput")
    segs8d = nc.dram_tensor("segs8", [128, 2 * 32], FP8, kind="ExternalInput")
    if use_fp8:
        xt8 = nc.dram_tensor("xt8", [HID, G * L], FP8, kind="ExternalInput")
        wq8d = nc.dram_tensor("wq8", [128, NCH * 2 * 256], FP8, kind="ExternalInput")
        wk8d = nc.dram_tensor("wk8", [128, NCH * 2 * 256], FP8, kind="ExternalInput")
    else:
        wktd = nc.dram_tensor("wkt", [HID, HID], BF16, kind="ExternalInput")
    wqtd = nc.dram_tensor("wqt", [HID, HID], BF16, kind="ExternalInput")
    wvrd = nc.dram_tensor("wvr", [HID, HID], BF16, kind="ExternalInput")
    wotd = nc.dram_tensor("wot", [HID, HID], BF16, kind="ExternalInput")
    wrbdd = nc.dram_tensor("wrbd", [128, 128], BF16, kind="ExternalInput")
    identd = nc.dram_tensor("ident", [128, 128], BF16, kind="ExternalInput")
    segsd = nc.dram_tensor("segs", [128, 8 * NCH], BF16, kind="ExternalInput")
    selsd = nc.dram_tensor("sels", [8, HID], F32, kind="ExternalInput")
    wacold = nc.dram_tensor("wacol", [128, NCH], F32, kind="ExternalInput")
    wbcold = nc.dram_tensor("wbcol", [128, NCH], F32, kind="ExternalInput")
    if apply_bo:
        bod = nc.dram_tensor("bo", [1, HID], F32, kind="ExternalInput")
        onesd = nc.dram_tensor("ones1", [1, 128], F32, kind="ExternalInput")
    if apply_affine:
        lngd = nc.dram_tensor("ln_g", [128, HID], BF16, kind="ExternalInput")
        lnbd = nc.dram_tensor("ln_b", [128, HID], BF16, kind="ExternalInput")
    outd = nc.dram_tensor("out", [G * L, HID], BF16, kind="ExternalOutput")

    SWV = SW if use_fp8 else 1.0

    with TileContext(nc) as tc:
        with tc.tile_pool(name="consts", bufs=1) as cp, \
             tc.tile_pool(name="big", bufs=1) as bp, \
             tc.tile_pool(name="small", bufs=2) as sp, \
             tc.tile_pool(name="psum", bufs=1, space="PSUM") as ppool:

            # ---- constants to SBUF ----
            wqt_sb = [cp.tile([128, HID], BF16, name=f"wqt{i}") for i in range(NCH)]
            for i in range(NCH):
                nc.sync.dma_start(out=wqt_sb[i], in_=wqtd.ap()[128 * i:128 * (i + 1), :])
            segs8_sb = cp.tile([128, 2 * 32], FP8)
            nc.sync.dma_start(out=segs8_sb, in_=segs8d.ap())
            if use_fp8:
                wq8_sb = cp.tile([128, NCH * 2 * 256], FP8)
                nc.sync.dma_start(out=wq8_sb, in_=wq8d.ap())
                wk8_sb = cp.tile([128, NCH * 2 * 256], FP8)
                nc.sync.dma_start(out=wk8_sb, in_=wk8d.ap())
            else:
                wkt_sb = [cp.tile([128, HID], BF16, name=f"wkt{i}") for i in range(NCH)]
                for i in range(NCH):
                    nc.sync.dma_start(out=wkt_sb[i], in_=wktd.ap()[128 * i:128 * (i + 1), :])
            wvr_sb = [cp.tile([128, HID], BF16, name=f"wvr{j}") for j in range(NCH)]
            for j in range(NCH):
                nc.sync.dma_start(out=wvr_sb[j], in_=wvrd.ap()[128 * j:128 * (j + 1), :])
            wot_sb = [cp.tile([128, HID], BF16, name=f"wot{j}") for j in range(NCH)]
            for j in range(NCH):
                nc.sync.dma_start(out=wot_sb[j], in_=wotd.ap()[128 * j:128 * (j + 1), :])
            wrbd_sb = cp.tile([128, 128], BF16)
            nc.sync.dma_start(out=wrbd_sb, in_=wrbdd.ap())
            ident_sb = cp.tile([128, 128], BF16)
            nc.sync.dma_start(out=ident_sb, in_=identd.ap())
            segs_sb = cp.tile([128, 8 * NCH], BF16)
            nc.sync.dma_start(out=segs_sb, in_=segsd.ap())
            sels_sb = cp.tile([8, HID], F32R)
            nc.sync.dma_start(out=sels_sb, in_=selsd.ap().bitcast(F32R))
            wacol_sb = cp.tile([128, NCH], F32)
            nc.sync.dma_start(out=wacol_sb, in_=wacold.ap())
            wbcol_sb = cp.tile([128, NCH], F32)
            nc.sync.dma_start(out=wbcol_sb, in_=wbcold.ap())
            if apply_bo:
                ones1_sb = cp.tile([1, 128], F32R)
                nc.sync.dma_start(out=ones1_sb, in_=onesd.ap().bitcast(F32R))
                bo_sb = cp.tile([1, HID], F32R)
                nc.sync.dma_start(out=bo_sb, in_=bod.ap().bitcast(F32R))
            if apply_affine:
                lng_sb = cp.tile([128, HID], BF16)
                nc.sync.dma_start(out=lng_sb, in_=lngd.ap())
                lnb_sb = cp.tile([128, HID], BF16)
                nc.sync.dma_start(out=lnb_sb, in_=lnbd.ap())

            EDT = FP8 if use_fp8 else BF16

            # -------- per-graph state (software-pipelined emission) --------
            st = {}

            def phase_load(g):
                s = {}
                s["xtb"] = bp.tile([128, NCH * L], BF16, name=f"xtb{g}", tag="xtb",
                                   bufs=3)
                xtb_src = bass.AP(
                    tensor=xtb.ap().tensor, offset=g * L,
                    ap=[[G * L, 128], [128 * G * L, NCH], [1, L]])
                nc.sync.dma_start(
                    out=s["xtb"].rearrange("p (i l) -> p i l", i=NCH), in_=xtb_src)
                if use_fp8:
                    s["xt8"] = bp.tile([128, NCH * L], FP8, name=f"xt8{g}",
                                       tag="xt8", bufs=2)
                    xt8_src = bass.AP(
                        tensor=xt8.ap().tensor, offset=g * L,
                        ap=[[G * L, 128], [128 * G * L, NCH], [1, L]])
                    nc.sync.dma_start(
                        out=s["xt8"].rearrange("p (i l) -> p i l", i=NCH),
                        in_=xt8_src)
                st[g] = s

            def proj_half(g, w8_sb, w_sb, j, n0, pp):
                if use_fp8:
                    xt8_3d = st[g]["xt8"].rearrange("p (i l) -> p i l", i=NCH)
                    for p in range(2):
                        lhs = w8_sb[:, (2 * j + p) * 256:(2 * j + p + 1) * 256] \
                            .rearrange("p (two f) -> p two f", two=2)
                        rhs = xt8_3d[:, 2 * p:2 * p + 2, n0:n0 + 512]
                        nc.tensor.matmul(pp, lhs, rhs, start=(p == 0),
                                         stop=(p == 1), perf_mode=PM.DoubleRow)
                else:
                    for i in range(NCH):
                        nc.tensor.matmul(
                            pp, w_sb[i][:, 128 * j:128 * (j + 1)],
                            st[g]["xtb"][:, i * L + n0: i * L + n0 + 512],
                            start=(i == 0), stop=(i == NCH - 1))

            def proj_stage(g, tag, w8_sb, w_sb, scale_ap):
                """projection + exp + m for one of Q/K; then seg-sums + recip."""
                e_all = bp.tile([128, NCH * L], EDT, name=f"e{tag}{g}", tag="e",
                                bufs=3)
                m_tiles = []
                for j in range(NCH):
                    mt = sp.tile([128, L], BF16, name=f"m{tag}{g}{j}", tag="scr",
                                 bufs=4)
                    m_tiles.append(mt)
                for j in range(NCH):
                    for n0 in (0, 512):
                        pp = ppool.tile([128, 512], F32, name=f"pp{tag}{g}{j}{n0}",
                                        tag="pp", bufs=5)
                        proj_half(g, w8_sb, w_sb, j, n0, pp)
                        nc.scalar.activation(
                            out=e_all[:, j * L + n0: j * L + n0 + 512], in_=pp,
                            func=AT.Exp, scale=scale_ap[:, j:j + 1])
                        nc.vector.tensor_mul(
                            out=m_tiles[j][:, n0:n0 + 512],
                            in0=e_all[:, j * L + n0: j * L + n0 + 512], in1=pp)
                sos = []
                for hi, n0 in enumerate((0, 512)):
                    so = ppool.tile([16, 512], F32, name=f"so{tag}{g}{n0}",
                                    tag="rs", bufs=3, padded_shape=[128, 512])
                    if use_fp8:
                        e3d = e_all.rearrange("p (i l) -> p i l", i=NCH)
                        for p in range(2):
                            lhs = segs8_sb[:, 32 * p:32 * (p + 1)] \
                                .rearrange("p (two c) -> p two c", two=2)
                            nc.tensor.matmul(
                                so, lhs, e3d[:, 2 * p:2 * p + 2, n0:n0 + 512],
                                start=(p == 0), stop=(p == 1),
                                perf_mode=PM.DoubleRow)
                    else:
                        for j in range(NCH):
                            nc.tensor.matmul(
                                so[0:8, :], segs_sb[:, 8 * j:8 * (j + 1)],
                                e_all[:, j * L + n0: j * L + n0 + 512],
                                start=(j == 0), stop=(j == NCH - 1))
                    sos.append(so)
                rt = sp.tile([8, 1024], F32, name=f"rt{tag}{g}", tag="rt", bufs=3)
                nc.vector.reciprocal_approx_fast(out=rt[:, 0:512], in_=sos[0][0:8, :])
                nc.vector.reciprocal_approx_fast(out=rt[:, 512:1024], in_=sos[1][0:8, :])
                rtr = sp.tile([8, 1024], F32R, name=f"rtr{tag}{g}", tag="rtr", bufs=3)
                nc.vector.tensor_copy(out=rtr, in_=rt)
                st[g][f"m{tag}"] = m_tiles
                st[g][f"rt{tag}"] = rtr

            def rbc_stage(g, tag):
                """rbc expand + stt accumulate; returns summed [128, NCH] tile."""
                rtr = st[g][f"rt{tag}"]
                m_tiles = st[g][f"m{tag}"]
                parts = sp.tile([128, 2 * NCH], F32, name=f"pts{tag}{g}",
                                tag=f"pts_{tag}")
                for j in range(NCH):
                    for hi, n0 in enumerate((0, 512)):
                        rbc = ppool.tile([128, 512], F32, name=f"rbc{tag}{g}{j}{n0}",
                                         tag="rs", bufs=3)
                        nc.tensor.matmul(rbc, sels_sb[:, 128 * j:128 * (j + 1)],
                                         rtr[:, n0:n0 + 512])
                        nc.vector.scalar_tensor_tensor(
                            out=m_tiles[j][:, n0:n0 + 512],
                            in0=m_tiles[j][:, n0:n0 + 512],
                            scalar=1.0 / SWV, in1=rbc,
                            op0=OP.mult, op1=OP.mult,
                            accum_out=parts[:, hi * NCH + j:hi * NCH + j + 1])
                tot = sp.tile([128, NCH], F32, name=f"tot{tag}{g}", tag=f"tot{tag}")
                nc.gpsimd.tensor_add(out=tot, in0=parts[:, 0:NCH],
                                     in1=parts[:, NCH:2 * NCH])
                return tot

            def phase_A(g):
                proj_stage(g, "a", wq8_sb if use_fp8 else None,
                           None if use_fp8 else wqt_sb, wacol_sb)

            def phase_rbcA(g):
                gq = rbc_stage(g, "a")
                gqwb = sp.tile([128, NCH], F32, name=f"gqwb{g}", tag="gqwb")
                nc.gpsimd.tensor_mul(out=gqwb, in0=gq, in1=wbcol_sb)
                st[g]["gq"] = gq
                st[g]["gqwb"] = gqwb

            def phase_K(g):
                proj_stage(g, "b", wk8_sb if use_fp8 else None,
                           None if use_fp8 else wkt_sb, st[g]["gqwb"])

            def phase_rbcB(g):
                acc = rbc_stage(g, "b")
                gk = sp.tile([128, NCH], F32, name=f"gk{g}", tag="gk")
                nc.gpsimd.tensor_mul(out=gk, in0=acc, in1=st[g]["gq"])
                gkwr = sp.tile([128, NCH * 128], BF16, name=f"gkwr{g}", tag="gkwr")
                for j in range(NCH):
                    nc.gpsimd.tensor_scalar_mul(
                        out=gkwr[:, 128 * j:128 * (j + 1)], in0=wrbd_sb,
                        scalar1=gk[:, j:j + 1])
                st[g]["gkwr"] = gkwr

            def phase_prep(g):
                gkwr = st[g]["gkwr"]
                w3_sb = []
                for i in range(NCH):
                    ppw = ppool.tile([128, 512], F32, name=f"ppw{g}{i}", tag="pp",
                                     bufs=5)
                    for j in range(NCH):
                        nc.tensor.matmul(
                            ppw[:, 128 * j:128 * (j + 1)],
                            wvr_sb[j][:, 128 * i:128 * (i + 1)],
                            gkwr[:, 128 * j:128 * (j + 1)],
                            start=True, stop=False)
                        nc.tensor.matmul(
                            ppw[:, 128 * j:128 * (j + 1)], ident_sb,
                            wqt_sb[i][:, 128 * j:128 * (j + 1)],
                            start=False, stop=True)
                    w3 = sp.tile([128, 512], BF16, name=f"w3{g}{i}", tag="w3",
                                 bufs=8)
                    nc.scalar.copy(out=w3, in_=ppw)
                    w3_sb.append(w3)
                st[g]["w3"] = w3_sb

            def phase_stream(g):
                w3_sb = st[g]["w3"]
                xtb_all = st[g]["xtb"]
                att_all = bp.tile([128, NCH * L], BF16, name=f"att{g}", tag="att",
                                  bufs=2)
                for j in range(NCH):
                    for n0 in (0, 512):
                        ppv = ppool.tile([128, 512], F32, name=f"ppv{g}{j}{n0}",
                                         tag="pp", bufs=5)
                        for i in range(NCH):
                            nc.tensor.matmul(
                                ppv, w3_sb[i][:, 128 * j:128 * (j + 1)],
                                xtb_all[:, i * L + n0: i * L + n0 + 512],
                                start=(i == 0), stop=(i == NCH - 1))
                        nc.scalar.activation(
                            out=att_all[:, j * L + n0: j * L + n0 + 512], in_=ppv,
                            func=AT.Relu)
                st[g]["att"] = att_all

            def phase_Wo(g):
                att_all = st[g]["att"]
                mv_all = sp.tile([128, 2 * NT], F32, name=f"mv{g}", tag="mv")
                rstd_all = sp.tile([128, NT], F32, name=f"rstd{g}", tag="rstd")
                vf = sp.tile([128, NT], F32, name=f"vf{g}", tag="vf")
                lnv = sp.tile([128, NT], F32, name=f"lnv{g}", tag="lnv")
                obs = []
                for t in range(NT):
                    o_ps = ppool.tile([128, HID], F32, name=f"ops{g}{t}", tag="pp",
                                      bufs=5)
                    last = NCH - 1
                    for j in range(NCH):
                        nc.tensor.matmul(
                            o_ps, att_all[:, j * L + 128 * t: j * L + 128 * (t + 1)],
                            wot_sb[j], start=(j == 0),
                            stop=(j == last and not apply_bo))
                    if apply_bo:
                        nc.tensor.matmul(o_ps, ones1_sb, bo_sb, start=False,
                                         stop=True)
                    ob = sp.tile([128, HID], BF16, name=f"ob{g}{t}", tag="ob",
                                 bufs=NT + 2)
                    nc.scalar.copy(out=ob, in_=o_ps)
                    stats = sp.tile([128, 6], F32, name=f"sst{g}{t}", tag="sst")
                    nc.vector.bn_stats(out=stats, in_=ob)
                    nc.vector.bn_aggr(out=mv_all[:, 2 * t:2 * t + 2], in_=stats)
                    obs.append(ob)
                nc.gpsimd.tensor_scalar_add(out=vf, in0=mv_all[:, 1:2 * NT:2],
                                            scalar1=EPS)
                nc.scalar.activation(out=lnv, in_=vf, func=AT.Ln)
                nc.scalar.activation(out=rstd_all, in_=lnv, func=AT.Exp, scale=-0.5)
                for t in range(NT):
                    och = sp.tile([128, HID], BF16, name=f"och{g}{t}", tag="och",
                                  bufs=4)
                    nc.gpsimd.tensor_scalar(
                        out=och, in0=obs[t], scalar1=mv_all[:, 2 * t:2 * t + 1],
                        scalar2=rstd_all[:, t:t + 1], op0=OP.subtract, op1=OP.mult)
                    if apply_affine:
                        nc.vector.tensor_mul(out=och, in0=och, in1=lng_sb)
                        nc.vector.tensor_add(out=och, in0=och, in1=lnb_sb)
                    nc.sync.dma_start(
                        out=outd.ap()[g * L + 128 * t: g * L + 128 * (t + 1), :],
                        in_=och)
                del st[g]

            # -------- modulo schedule: stream/Wo of g-1 fill g's stt chains ----
            phase_load(0)
            for g in range(G):
                if g + 1 < G:
                    phase_load(g + 1)
                phase_A(g)
                phase_rbcA(g)
                if g > 0:
                    phase_stream(g - 1)
                phase_K(g)
                phase_rbcB(g)
                if g > 0:
                    phase_Wo(g - 1)
                phase_prep(g)
            phase_stream(G - 1)
            phase_Wo(G - 1)

    _bacc_mod.get_activation_tables = _gat
    try:
        nc.compile()
    finally:
        _bacc_mod.get_activation_tables = _orig_gat
    return nc


_NC_CACHE = {}


def _get_nc(apply_bo, apply_affine):
    key = (apply_bo, apply_affine, USE_FP8)
    if key not in _NC_CACHE:
        _NC_CACHE[key] = _build(apply_bo, apply_affine, USE_FP8)
    return _NC_CACHE[key]


def _host_consts(Wq, Wk, Wv, Wr, w_alpha, w_beta, Wo, bo, ln_g, ln_b):
    import ml_dtypes
    bf = ml_dtypes.bfloat16
    f8 = ml_dtypes.float8_e4m3fn

    wqt = np.ascontiguousarray(Wq.T)                       # [h, e]
    wvr = np.ascontiguousarray(Wv)                         # [d, h]
    wot = np.ascontiguousarray(Wo.T)
    wrt = Wr.T.astype(np.float32)                          # WrT[d, e] = Wr[e, d]
    wrbd = np.zeros((128, 128), np.float32)
    wrbd[:64, :64] = wrt; wrbd[64:, 64:] = wrt
    ident = np.eye(128, dtype=np.float32)
    wa_vec = np.tile(w_alpha, HEADS) * SCALE               # [512]
    wb_vec = np.tile(w_beta, HEADS) * SCALE
    SWV = SW if USE_FP8 else 1.0
    wacol = (wa_vec / SWV).reshape(NCH, 128).T.copy()      # [128, NCH]
    wbcol = (wb_vec / SWV).reshape(NCH, 128).T.copy()

    segs = np.zeros((128, 8 * NCH), np.float32)
    sels = np.zeros((8, HID), np.float32)
    for j in range(NCH):
        for p in range(128):
            segs[p, 8 * j + 2 * j + p // 64] = 1.0
        for m in range(HID):
            if m // 128 == j:
                sels[2 * j + (m % 128) // 64, m] = 1.0
    segs8 = np.zeros((128, 2 * 32), np.float32)
    for p in range(2):
        for d in range(128):
            hA = 4 * p + d // 64          # head of (chunk 2p, partition d)
            hB = 4 * p + 2 + d // 64      # head of (chunk 2p+1, partition d)
            segs8[d, 32 * p + hA] = 1.0
            segs8[d, 32 * p + 16 + hB] = 1.0

    common = {"wqt": wqt.astype(bf), "wvr": wvr.astype(bf),
              "wot": wot.astype(bf), "wrbd": wrbd.astype(bf),
              "ident": ident.astype(bf), "segs": segs.astype(bf),
              "segs8": segs8.astype(f8),
              "sels": sels, "wacol": wacol.astype(np.float32),
              "wbcol": wbcol.astype(np.float32)}

    if USE_FP8:
        def pack_dr(WT):   # WT [h, e] -> [128, NCH*2*256] DoubleRow stationary
            out = np.zeros((128, NCH * 2 * 256), np.float32)
            for j in range(NCH):
                for p in range(2):
                    blkA = WT[256 * p:256 * p + 128, 128 * j:128 * (j + 1)]
                    blkB = WT[256 * p + 128:256 * (p + 1), 128 * j:128 * (j + 1)]
                    c0 = (2 * j + p) * 256
                    out[:, c0:c0 + 128] = SW * blkA
                    out[:, c0 + 128:c0 + 256] = SW * blkB
            return out
        common["wq8"] = pack_dr(Wq.T).astype(f8)
        common["wk8"] = pack_dr(Wk.T).astype(f8)
    else:
        common["wkt"] = np.ascontiguousarray(Wk.T).astype(bf)

    apply_bo = not np.allclose(bo, 0.0)
    apply_affine = not (np.allclose(ln_g, 1.0) and np.allclose(ln_b, 0.0))
    if apply_bo:
        common["bo"] = bo.reshape(1, HID).astype(np.float32)
        common["ones1"] = np.ones((1, 128), np.float32)
    if apply_affine:
        common["ln_g"] = np.tile(ln_g, (128, 1)).astype(bf)
        common["ln_b"] = np.tile(ln_b, (128, 1)).astype(bf)
    return common, apply_bo, apply_affine


def kernel(edge_attr, batch_scopes, Wq, Wk, Wv, Wr, w_alpha, w_beta, Wo, bo,
           ln_g, ln_b):
    from concourse import bass_utils
    import ml_dtypes

    edge_attr = np.asarray(edge_attr, dtype=np.float32)
    scopes = np.asarray(batch_scopes)
    Wq = np.asarray(Wq, np.float32); Wk = np.asarray(Wk, np.float32)
    Wv = np.asarray(Wv, np.float32); Wr = np.asarray(Wr, np.float32)
    Wo = np.asarray(Wo, np.float32)
    w_alpha = np.asarray(w_alpha, np.float32); w_beta = np.asarray(w_beta, np.float32)
    bo = np.asarray(bo, np.float32)
    ln_g = np.asarray(ln_g, np.float32); ln_b = np.asarray(ln_b, np.float32)

    assert np.all(scopes[:, 1] == L), "equal-length contiguous scopes expected"
    starts = scopes[:, 0].astype(np.int64)

    common, apply_bo, apply_affine = _host_consts(
        Wq, Wk, Wv, Wr, w_alpha, w_beta, Wo, bo, ln_g, ln_b)
    nc = _get_nc(apply_bo, apply_affine)

    bf = ml_dtypes.bfloat16
    f8 = ml_dtypes.float8_e4m3fn
    in_maps = []
    for c in range(NCORES):
        rows = np.concatenate([
            np.arange(starts[c * G + g], starts[c * G + g] + L)
            for g in range(G)])
        xslab = edge_attr[rows]                       # [G*L, 512]
        xt = np.ascontiguousarray(xslab.T)
        m = {"xtb": xt.astype(bf), **common}
        if USE_FP8:
            m["xt8"] = xt.astype(f8)
        in_maps.append(m)

    res = bass_utils.run_bass_kernel_spmd(nc, in_maps, core_ids=list(range(NCORES)))
    out = np.concatenate([r["out"] for r in res.results], axis=0)
    return out.astype(np.float32)


# revision 22
# speedup vs baseline: 1.0434x; 1.0434x over previous
"""BondFastAttention Trainium2 kernel (self-contained), v2.

Shapes (hardcoded from the problem spec):
  edge_attr [65536, 512] fp32, B=64 graphs x L=1024 bonds, HID=512, 8 heads x D=64.
  8 NeuronCores, data-parallel over graphs: G=8 graphs per core.

Device layout: transposed domain - features on partitions, tokens on free dim
for Q/K/kvout; tokens on partitions for the Wo/LayerNorm stage.

Key structure vs v1:
  - The V projection, Wr matmul, gk scaling and +q add are all folded into a
    single per-graph combined weight W''' = Wv^T (gk . Wr^T) + Wq^T, built on
    the PE (16 small matmuls + identity-add), so one X-stream produces
    relu-input directly.
  - Projection PSUM is consumed in place (ACT exp, Pool multiply) - no
    psum->sbuf copies for q/k/v.
  - Optional fp8 path: Q/K projections and softmax seg-sums run as fp8
    DoubleRow matmuls (K=256 per pass).
  - Output is written bf16 and upcast to f32 on the host.
"""
import numpy as np

HID = 512
HEADS = 8
D = 64
B = 64
L = 1024
SCALE = D ** -0.5
EPS = 1e-5
NCORES = 8
G = B // NCORES          # graphs per core
NCH = HID // 128         # 4 feature chunks (2 heads each)
NT = L // 128            # 8 token chunks
SW = 16.0                # fp8 weight prescale

USE_FP8 = False


def _build(apply_bo: bool, apply_affine: bool, use_fp8: bool):
    import concourse.bass as bass
    from concourse import bacc
    import concourse.mybir as mybir
    from concourse.tile import TileContext

    F32 = mybir.dt.float32
    F32R = mybir.dt.float32r
    BF16 = mybir.dt.bfloat16
    FP8 = mybir.dt.float8e4
    AT = mybir.ActivationFunctionType
    OP = mybir.AluOpType
    PM = mybir.MatmulPerfMode

    nc = bacc.Bacc()

    import concourse.bacc as _bacc_mod
    _orig_gat = _bacc_mod.get_activation_tables

    def _gat(arch):
        # Keep dict order but strip our funcs from every other set, so the
        # table-load pass assigns all of them to natural_log_exp_and_others
        # -> a single physical table load.
        t = _orig_gat(arch)
        ours = {AT.Exp, AT.Ln, AT.Copy, AT.Relu, AT.Identity}
        out = {}
        for k, funcs in t.items():
            if k == "natural_log_exp_and_others":
                out[k] = funcs
            else:
                out[k] = {f for f in funcs if f not in ours}
        return out

    # ---------------- dram tensors ----------------
    xtb = nc.dram_tensor("xtb", [HID, G * L], BF16, kind="ExternalInput")
    segs8d = nc.dram_tensor("segs8", [128, 2 * 32], FP8, kind="ExternalInput")
    if use_fp8:
        xt8 = nc.dram_tensor("xt8", [HID, G * L], FP8, kind="ExternalInput")
        wq8d = nc.dram_tensor("wq8", [128, NCH * 2 * 256], FP8, kind="ExternalInput")
        wk8d = nc.dram_tensor("wk8", [128, NCH * 2 * 256], FP8, kind="ExternalInput")
    else:
        wktd = nc.dram_tensor("wkt", [HID, HID], BF16, kind="ExternalInput")
    wqtd = nc.dram_tensor("wqt", [HID, HID], BF16, kind="ExternalInput")
    wvrd = nc.dram_tensor("wvr", [HID, HID], BF16, kind="ExternalInput")
    wotd = nc.dram_tensor("wot", [HID, HID], BF16, kind="ExternalInput")
    wrbdd = nc.dram_tensor("wrbd", [128, 128], BF16, kind="ExternalInput")
    identd = nc.dram_tensor("ident", [128, 128], BF16, kind="ExternalInput")
    segsd = nc.dram_tensor("segs", [128, 8 * NCH], BF16, kind="ExternalInput")
    selsd = nc.dram_tensor("sels", [8, HID], F32, kind="ExternalInput")
    wacold = nc.dram_tensor("wacol", [128, NCH], F32, kind="ExternalInput")
    wbcold = nc.dram_tensor("wbcol", [128, NCH], F32, kind="ExternalInput")
    if apply_bo:
        bod = nc.dram_tensor("bo", [1, HID], F32, kind="ExternalInput")
        onesd = nc.dram_tensor("ones1", [1, 128], F32, kind="ExternalInput")
    if apply_affine:
        lngd = nc.dram_tensor("ln_g", [128, HID], BF16, kind="ExternalInput")
        lnbd = nc.dram_tensor("ln_b", [128, HID], BF16, kind="ExternalInput")
    outd = nc.dram_tensor("out", [G * L, HID], BF16, kind="ExternalOutput")

    SWV = SW if use_fp8 else 1.0

    with TileContext(nc) as tc:
        with tc.tile_pool(name="consts", bufs=1) as cp, \
             tc.tile_pool(name="big", bufs=1) as bp, \
             tc.tile_pool(name="small", bufs=2) as sp, \
             tc.tile_pool(name="psum", bufs=1, space="PSUM") as ppool:

            # ---- constants to SBUF ----
            # Q-path consts first (on sync, ahead of xtb(0)); the rest spread
            # across the scalar/vector/gpsimd DGE queues so the preamble
            # parallelizes instead of serializing on SP.
            wqt_sb = [cp.tile([128, HID], BF16, name=f"wqt{i}") for i in range(NCH)]
            for i in range(NCH):
                nc.sync.dma_start(out=wqt_sb[i], in_=wqtd.ap()[128 * i:128 * (i + 1), :])
            wacol_sb = cp.tile([128, NCH], F32)
            nc.sync.dma_start(out=wacol_sb, in_=wacold.ap())
            segs8_sb = cp.tile([128, 2 * 32], FP8)
            nc.scalar.dma_start(out=segs8_sb, in_=segs8d.ap())
            if use_fp8:
                wq8_sb = cp.tile([128, NCH * 2 * 256], FP8)
                nc.scalar.dma_start(out=wq8_sb, in_=wq8d.ap())
                wk8_sb = cp.tile([128, NCH * 2 * 256], FP8)
                nc.scalar.dma_start(out=wk8_sb, in_=wk8d.ap())
            else:
                wkt_sb = [cp.tile([128, HID], BF16, name=f"wkt{i}") for i in range(NCH)]
                for i in range(NCH):
                    nc.scalar.dma_start(out=wkt_sb[i], in_=wktd.ap()[128 * i:128 * (i + 1), :])
            wvr_sb = [cp.tile([128, HID], BF16, name=f"wvr{j}") for j in range(NCH)]
            for j in range(NCH):
                nc.gpsimd.dma_start(out=wvr_sb[j], in_=wvrd.ap()[128 * j:128 * (j + 1), :])
            wot_sb = [cp.tile([128, HID], BF16, name=f"wot{j}") for j in range(NCH)]
            for j in range(NCH):
                nc.gpsimd.dma_start(out=wot_sb[j], in_=wotd.ap()[128 * j:128 * (j + 1), :])
            wrbd_sb = cp.tile([128, 128], BF16)
            nc.scalar.dma_start(out=wrbd_sb, in_=wrbdd.ap())
            ident_sb = cp.tile([128, 128], BF16)
            nc.scalar.dma_start(out=ident_sb, in_=identd.ap())
            segs_sb = cp.tile([128, 8 * NCH], BF16)
            nc.scalar.dma_start(out=segs_sb, in_=segsd.ap())
            sels_sb = cp.tile([8, HID], F32R)
            nc.scalar.dma_start(out=sels_sb, in_=selsd.ap().bitcast(F32R))
            wbcol_sb = cp.tile([128, NCH], F32)
            nc.scalar.dma_start(out=wbcol_sb, in_=wbcold.ap())
            if apply_bo:
                ones1_sb = cp.tile([1, 128], F32R)
                nc.scalar.dma_start(out=ones1_sb, in_=onesd.ap().bitcast(F32R))
                bo_sb = cp.tile([1, HID], F32R)
                nc.scalar.dma_start(out=bo_sb, in_=bod.ap().bitcast(F32R))
            if apply_affine:
                lng_sb = cp.tile([128, HID], BF16)
                nc.gpsimd.dma_start(out=lng_sb, in_=lngd.ap())
                lnb_sb = cp.tile([128, HID], BF16)
                nc.gpsimd.dma_start(out=lnb_sb, in_=lnbd.ap())

            EDT = FP8 if use_fp8 else BF16

            # -------- per-graph state (software-pipelined emission) --------
            st = {}

            def phase_load(g):
                s = {}
                s["xtb"] = bp.tile([128, NCH * L], BF16, name=f"xtb{g}", tag="xtb",
                                   bufs=3)
                xtb_src = bass.AP(
                    tensor=xtb.ap().tensor, offset=g * L,
                    ap=[[G * L, 128], [128 * G * L, NCH], [1, L]])
                nc.sync.dma_start(
                    out=s["xtb"].rearrange("p (i l) -> p i l", i=NCH), in_=xtb_src)
                if use_fp8:
                    s["xt8"] = bp.tile([128, NCH * L], FP8, name=f"xt8{g}",
                                       tag="xt8", bufs=2)
                    xt8_src = bass.AP(
                        tensor=xt8.ap().tensor, offset=g * L,
                        ap=[[G * L, 128], [128 * G * L, NCH], [1, L]])
                    nc.sync.dma_start(
                        out=s["xt8"].rearrange("p (i l) -> p i l", i=NCH),
                        in_=xt8_src)
                st[g] = s

            def proj_half(g, w8_sb, w_sb, j, n0, pp):
                if use_fp8:
                    xt8_3d = st[g]["xt8"].rearrange("p (i l) -> p i l", i=NCH)
                    for p in range(2):
                        lhs = w8_sb[:, (2 * j + p) * 256:(2 * j + p + 1) * 256] \
                            .rearrange("p (two f) -> p two f", two=2)
                        rhs = xt8_3d[:, 2 * p:2 * p + 2, n0:n0 + 512]
                        nc.tensor.matmul(pp, lhs, rhs, start=(p == 0),
                                         stop=(p == 1), perf_mode=PM.DoubleRow)
                else:
                    for i in range(NCH):
                        nc.tensor.matmul(
                            pp, w_sb[i][:, 128 * j:128 * (j + 1)],
                            st[g]["xtb"][:, i * L + n0: i * L + n0 + 512],
                            start=(i == 0), stop=(i == NCH - 1))

            def proj_stage(g, tag, w8_sb, w_sb, scale_ap):
                """projection + exp + m for one of Q/K; then seg-sums + recip."""
                e_all = bp.tile([128, NCH * L], EDT, name=f"e{tag}{g}", tag="e",
                                bufs=3)
                m_tiles = []
                for j in range(NCH):
                    mt = sp.tile([128, L], BF16, name=f"m{tag}{g}{j}", tag="scr",
                                 bufs=4)
                    m_tiles.append(mt)
                for j in range(NCH):
                    for n0 in (0, 512):
                        pp = ppool.tile([128, 512], F32, name=f"pp{tag}{g}{j}{n0}",
                                        tag="pp", bufs=5)
                        proj_half(g, w8_sb, w_sb, j, n0, pp)
                        nc.scalar.activation(
                            out=e_all[:, j * L + n0: j * L + n0 + 512], in_=pp,
                            func=AT.Exp, scale=scale_ap[:, j:j + 1])
                        nc.vector.tensor_mul(
                            out=m_tiles[j][:, n0:n0 + 512],
                            in0=e_all[:, j * L + n0: j * L + n0 + 512], in1=pp)
                sos = []
                for hi, n0 in enumerate((0, 512)):
                    so = ppool.tile([16, 512], F32, name=f"so{tag}{g}{n0}",
                                    tag="rs", bufs=3, padded_shape=[128, 512])
                    if use_fp8:
                        e3d = e_all.rearrange("p (i l) -> p i l", i=NCH)
                        for p in range(2):
                            lhs = segs8_sb[:, 32 * p:32 * (p + 1)] \
                                .rearrange("p (two c) -> p two c", two=2)
                            nc.tensor.matmul(
                                so, lhs, e3d[:, 2 * p:2 * p + 2, n0:n0 + 512],
                                start=(p == 0), stop=(p == 1),
                                perf_mode=PM.DoubleRow)
                    else:
                        for j in range(NCH):
                            nc.tensor.matmul(
                                so[0:8, :], segs_sb[:, 8 * j:8 * (j + 1)],
                                e_all[:, j * L + n0: j * L + n0 + 512],
                                start=(j == 0), stop=(j == NCH - 1))
                    sos.append(so)
                rt = sp.tile([8, 1024], F32, name=f"rt{tag}{g}", tag="rt", bufs=3)
                nc.vector.reciprocal_approx_fast(out=rt[:, 0:512], in_=sos[0][0:8, :])
                nc.vector.reciprocal_approx_fast(out=rt[:, 512:1024], in_=sos[1][0:8, :])
                rtr = sp.tile([8, 1024], F32R, name=f"rtr{tag}{g}", tag="rtr", bufs=3)
                nc.vector.tensor_copy(out=rtr, in_=rt)
                st[g][f"m{tag}"] = m_tiles
                st[g][f"rt{tag}"] = rtr

            def rbc_stage(g, tag):
                """rbc expand + stt accumulate; returns summed [128, NCH] tile."""
                rtr = st[g][f"rt{tag}"]
                m_tiles = st[g][f"m{tag}"]
                parts = sp.tile([128, 2 * NCH], F32, name=f"pts{tag}{g}",
                                tag=f"pts_{tag}")
                for j in range(NCH):
                    for hi, n0 in enumerate((0, 512)):
                        rbc = ppool.tile([128, 512], F32, name=f"rbc{tag}{g}{j}{n0}",
                                         tag="rs", bufs=3)
                        nc.tensor.matmul(rbc, sels_sb[:, 128 * j:128 * (j + 1)],
                                         rtr[:, n0:n0 + 512])
                        nc.vector.scalar_tensor_tensor(
                            out=m_tiles[j][:, n0:n0 + 512],
                            in0=m_tiles[j][:, n0:n0 + 512],
                            scalar=1.0 / SWV, in1=rbc,
                            op0=OP.mult, op1=OP.mult,
                            accum_out=parts[:, hi * NCH + j:hi * NCH + j + 1])
                tot = sp.tile([128, NCH], F32, name=f"tot{tag}{g}", tag=f"tot{tag}")
                nc.gpsimd.tensor_add(out=tot, in0=parts[:, 0:NCH],
                                     in1=parts[:, NCH:2 * NCH])
                return tot

            def phase_A(g):
                proj_stage(g, "a", wq8_sb if use_fp8 else None,
                           None if use_fp8 else wqt_sb, wacol_sb)

            def phase_rbcA(g):
                gq = rbc_stage(g, "a")
                gqwb = sp.tile([128, NCH], F32, name=f"gqwb{g}", tag="gqwb")
                nc.gpsimd.tensor_mul(out=gqwb, in0=gq, in1=wbcol_sb)
                st[g]["gq"] = gq
                st[g]["gqwb"] = gqwb

            def phase_K(g):
                proj_stage(g, "b", wk8_sb if use_fp8 else None,
                           None if use_fp8 else wkt_sb, st[g]["gqwb"])

            def phase_rbcB(g):
                acc = rbc_stage(g, "b")
                gk = sp.tile([128, NCH], F32, name=f"gk{g}", tag="gk")
                nc.gpsimd.tensor_mul(out=gk, in0=acc, in1=st[g]["gq"])
                gkwr = sp.tile([128, NCH * 128], BF16, name=f"gkwr{g}", tag="gkwr")
                for j in range(NCH):
                    nc.gpsimd.tensor_scalar_mul(
                        out=gkwr[:, 128 * j:128 * (j + 1)], in0=wrbd_sb,
                        scalar1=gk[:, j:j + 1])
                st[g]["gkwr"] = gkwr

            def phase_prep(g):
                gkwr = st[g]["gkwr"]
                w3_sb = []
                for i in range(NCH):
                    ppw = ppool.tile([128, 512], F32, name=f"ppw{g}{i}", tag="pp",
                                     bufs=5)
                    for j in range(NCH):
                        nc.tensor.matmul(
                            ppw[:, 128 * j:128 * (j + 1)],
                            wvr_sb[j][:, 128 * i:128 * (i + 1)],
                            gkwr[:, 128 * j:128 * (j + 1)],
                            start=True, stop=False)
                        nc.tensor.matmul(
                            ppw[:, 128 * j:128 * (j + 1)], ident_sb,
                            wqt_sb[i][:, 128 * j:128 * (j + 1)],
                            start=False, stop=True)
                    w3 = sp.tile([128, 512], BF16, name=f"w3{g}{i}", tag="w3",
                                 bufs=8)
                    nc.scalar.copy(out=w3, in_=ppw)
                    w3_sb.append(w3)
                st[g]["w3"] = w3_sb

            def phase_stream(g):
                w3_sb = st[g]["w3"]
                xtb_all = st[g]["xtb"]
                att_all = bp.tile([128, NCH * L], BF16, name=f"att{g}", tag="att",
                                  bufs=2)
                for j in range(NCH):
                    for n0 in (0, 512):
                        ppv = ppool.tile([128, 512], F32, name=f"ppv{g}{j}{n0}",
                                         tag="pp", bufs=5)
                        for i in range(NCH):
                            nc.tensor.matmul(
                                ppv, w3_sb[i][:, 128 * j:128 * (j + 1)],
                                xtb_all[:, i * L + n0: i * L + n0 + 512],
                                start=(i == 0), stop=(i == NCH - 1))
                        nc.scalar.activation(
                            out=att_all[:, j * L + n0: j * L + n0 + 512], in_=ppv,
                            func=AT.Relu)
                st[g]["att"] = att_all

            def ln_apply(g, obs, mv_all, rstd_all, t):
                och = sp.tile([128, HID], BF16, name=f"och{g}{t}", tag="och",
                              bufs=4)
                nc.gpsimd.tensor_scalar(
                    out=och, in0=obs[t], scalar1=mv_all[:, 2 * t:2 * t + 1],
                    scalar2=rstd_all[:, t:t + 1], op0=OP.subtract, op1=OP.mult)
                if apply_affine:
                    nc.vector.tensor_mul(out=och, in0=och, in1=lng_sb)
                    nc.vector.tensor_add(out=och, in0=och, in1=lnb_sb)
                nc.sync.dma_start(
                    out=outd.ap()[g * L + 128 * t: g * L + 128 * (t + 1), :],
                    in_=och)

            def phase_Wo(g, tail=False):
                att_all = st[g]["att"]
                mv_all = sp.tile([128, 2 * NT], F32, name=f"mv{g}", tag="mv")
                rstd_all = sp.tile([128, NT], F32, name=f"rstd{g}", tag="rstd")
                vf = sp.tile([128, NT], F32, name=f"vf{g}", tag="vf")
                lnv = sp.tile([128, NT], F32, name=f"lnv{g}", tag="lnv")
                obs = []
                for t in range(NT):
                    o_ps = ppool.tile([128, HID], F32, name=f"ops{g}{t}", tag="pp",
                                      bufs=5)
                    last = NCH - 1
                    for j in range(NCH):
                        nc.tensor.matmul(
                            o_ps, att_all[:, j * L + 128 * t: j * L + 128 * (t + 1)],
                            wot_sb[j], start=(j == 0),
                            stop=(j == last and not apply_bo))
                    if apply_bo:
                        nc.tensor.matmul(o_ps, ones1_sb, bo_sb, start=False,
                                         stop=True)
                    ob = sp.tile([128, HID], BF16, name=f"ob{g}{t}", tag="ob",
                                 bufs=NT + 2)
                    nc.scalar.copy(out=ob, in_=o_ps)
                    stats = sp.tile([128, 6], F32, name=f"sst{g}{t}", tag="sst")
                    nc.vector.bn_stats(out=stats, in_=ob)
                    nc.vector.bn_aggr(out=mv_all[:, 2 * t:2 * t + 2], in_=stats)
                    obs.append(ob)
                    if tail:
                        nc.vector.tensor_scalar_add(
                            out=vf[:, t:t + 1], in0=mv_all[:, 2 * t + 1:2 * t + 2],
                            scalar1=EPS)
                        nc.scalar.activation(out=lnv[:, t:t + 1], in_=vf[:, t:t + 1],
                                             func=AT.Ln)
                        nc.scalar.activation(out=rstd_all[:, t:t + 1],
                                             in_=lnv[:, t:t + 1], func=AT.Exp,
                                             scale=-0.5)
                        ln_apply(g, obs, mv_all, rstd_all, t)
                if not tail:
                    nc.gpsimd.tensor_scalar_add(out=vf, in0=mv_all[:, 1:2 * NT:2],
                                                scalar1=EPS)
                    nc.scalar.activation(out=lnv, in_=vf, func=AT.Ln)
                    nc.scalar.activation(out=rstd_all, in_=lnv, func=AT.Exp, scale=-0.5)
                    for t in range(NT):
                        ln_apply(g, obs, mv_all, rstd_all, t)
                del st[g]

            # -------- modulo schedule: stream/Wo of g-1 fill g's stt chains ----
            phase_load(0)
            for g in range(G):
                if g + 1 < G:
                    phase_load(g + 1)
                phase_A(g)
                phase_rbcA(g)
                if g > 0:
                    phase_stream(g - 1)
                phase_K(g)
                phase_rbcB(g)
                if g > 0:
                    phase_Wo(g - 1)
                phase_prep(g)
            phase_stream(G - 1)
            phase_Wo(G - 1, tail=True)

    _bacc_mod.get_activation_tables = _gat
    try:
        nc.compile()
    finally:
        _bacc_mod.get_activation_tables = _orig_gat
    return nc


_NC_CACHE = {}


def _get_nc(apply_bo, apply_affine):
    key = (apply_bo, apply_affine, USE_FP8)
    if key not in _NC_CACHE:
        _NC_CACHE[key] = _build(apply_bo, apply_affine, USE_FP8)
    return _NC_CACHE[key]


def _host_consts(Wq, Wk, Wv, Wr, w_alpha, w_beta, Wo, bo, ln_g, ln_b):
    import ml_dtypes
    bf = ml_dtypes.bfloat16
    f8 = ml_dtypes.float8_e4m3fn

    wqt = np.ascontiguousarray(Wq.T)                       # [h, e]
    wvr = np.ascontiguousarray(Wv)                         # [d, h]
    wot = np.ascontiguousarray(Wo.T)
    wrt = Wr.T.astype(np.float32)                          # WrT[d, e] = Wr[e, d]
    wrbd = np.zeros((128, 128), np.float32)
    wrbd[:64, :64] = wrt; wrbd[64:, 64:] = wrt
    ident = np.eye(128, dtype=np.float32)
    wa_vec = np.tile(w_alpha, HEADS) * SCALE               # [512]
    wb_vec = np.tile(w_beta, HEADS) * SCALE
    SWV = SW if USE_FP8 else 1.0
    wacol = (wa_vec / SWV).reshape(NCH, 128).T.copy()      # [128, NCH]
    wbcol = (wb_vec / SWV).reshape(NCH, 128).T.copy()

    segs = np.zeros((128, 8 * NCH), np.float32)
    sels = np.zeros((8, HID), np.float32)
    for j in range(NCH):
        for p in range(128):
            segs[p, 8 * j + 2 * j + p // 64] = 1.0
        for m in range(HID):
            if m // 128 == j:
                sels[2 * j + (m % 128) // 64, m] = 1.0
    segs8 = np.zeros((128, 2 * 32), np.float32)
    for p in range(2):
        for d in range(128):
            hA = 4 * p + d // 64          # head of (chunk 2p, partition d)
            hB = 4 * p + 2 + d // 64      # head of (chunk 2p+1, partition d)
            segs8[d, 32 * p + hA] = 1.0
            segs8[d, 32 * p + 16 + hB] = 1.0

    common = {"wqt": wqt.astype(bf), "wvr": wvr.astype(bf),
              "wot": wot.astype(bf), "wrbd": wrbd.astype(bf),
              "ident": ident.astype(bf), "segs": segs.astype(bf),
              "segs8": segs8.astype(f8),
              "sels": sels, "wacol": wacol.astype(np.float32),
              "wbcol": wbcol.astype(np.float32)}

    if USE_FP8:
        def pack_dr(WT):   # WT [h, e] -> [128, NCH*2*256] DoubleRow stationary
            out = np.zeros((128, NCH * 2 * 256), np.float32)
            for j in range(NCH):
                for p in range(2):
                    blkA = WT[256 * p:256 * p + 128, 128 * j:128 * (j + 1)]
                    blkB = WT[256 * p + 128:256 * (p + 1), 128 * j:128 * (j + 1)]
                    c0 = (2 * j + p) * 256
                    out[:, c0:c0 + 128] = SW * blkA
                    out[:, c0 + 128:c0 + 256] = SW * blkB
            return out
        common["wq8"] = pack_dr(Wq.T).astype(f8)
        common["wk8"] = pack_dr(Wk.T).astype(f8)
    else:
        common["wkt"] = np.ascontiguousarray(Wk.T).astype(bf)

    apply_bo = not np.allclose(bo, 0.0)
    apply_affine = not (np.allclose(ln_g, 1.0) and np.allclose(ln_b, 0.0))
    if apply_bo:
        common["bo"] = bo.reshape(1, HID).astype(np.float32)
        common["ones1"] = np.ones((1, 128), np.float32)
    if apply_affine:
        common["ln_g"] = np.tile(ln_g, (128, 1)).astype(bf)
        common["ln_b"] = np.tile(ln_b, (128, 1)).astype(bf)
    return common, apply_bo, apply_affine


def kernel(edge_attr, batch_scopes, Wq, Wk, Wv, Wr, w_alpha, w_beta, Wo, bo,
           ln_g, ln_b):
    from concourse import bass_utils
    import ml_dtypes

    edge_attr = np.asarray(edge_attr, dtype=np.float32)
    scopes = np.asarray(batch_scopes)
    Wq = np.asarray(Wq, np.float32); Wk = np.asarray(Wk, np.float32)
    Wv = np.asarray(Wv, np.float32); Wr = np.asarray(Wr, np.float32)
    Wo = np.asarray(Wo, np.float32)
    w_alpha = np.asarray(w_alpha, np.float32); w_beta = np.asarray(w_beta, np.float32)
    bo = np.asarray(bo, np.float32)
    ln_g = np.asarray(ln_g, np.float32); ln_b = np.asarray(ln_b, np.float32)

    assert np.all(scopes[:, 1] == L), "equal-length contiguous scopes expected"
    starts = scopes[:, 0].astype(np.int64)

    common, apply_bo, apply_affine = _host_consts(
        Wq, Wk, Wv, Wr, w_alpha, w_beta, Wo, bo, ln_g, ln_b)
    nc = _get_nc(apply_bo, apply_affine)

    bf = ml_dtypes.bfloat16
    f8 = ml_dtypes.float8_e4m3fn
    in_maps = []
    for c in range(NCORES):
        rows = np.concatenate([
            np.arange(starts[c * G + g], starts[c * G + g] + L)
            for g in range(G)])
        xslab = edge_attr[rows]                       # [G*L, 512]
        xt = np.ascontiguousarray(xslab.T)
        m = {"xtb": xt.astype(bf), **common}
        if USE_FP8:
            m["xt8"] = xt.astype(f8)
        in_maps.append(m)

    res = bass_utils.run_bass_kernel_spmd(nc, in_maps, core_ids=list(range(NCORES)))
    out = np.concatenate([r["out"] for r in res.results], axis=0)
    return out.astype(np.float32)


# revision 24
# speedup vs baseline: 1.0860x; 1.0409x over previous
"""BondFastAttention Trainium2 kernel (self-contained), v2.

Shapes (hardcoded from the problem spec):
  edge_attr [65536, 512] fp32, B=64 graphs x L=1024 bonds, HID=512, 8 heads x D=64.
  8 NeuronCores, data-parallel over graphs: G=8 graphs per core.

Device layout: transposed domain - features on partitions, tokens on free dim
for Q/K/kvout; tokens on partitions for the Wo/LayerNorm stage.

Key structure vs v1:
  - The V projection, Wr matmul, gk scaling and +q add are all folded into a
    single per-graph combined weight W''' = Wv^T (gk . Wr^T) + Wq^T, built on
    the PE (16 small matmuls + identity-add), so one X-stream produces
    relu-input directly.
  - Projection PSUM is consumed in place (ACT exp, Pool multiply) - no
    psum->sbuf copies for q/k/v.
  - Optional fp8 path: Q/K projections and softmax seg-sums run as fp8
    DoubleRow matmuls (K=256 per pass).
  - Output is written bf16 and upcast to f32 on the host.
"""
import numpy as np

HID = 512
HEADS = 8
D = 64
B = 64
L = 1024
SCALE = D ** -0.5
EPS = 1e-5
NCORES = 8
G = B // NCORES          # graphs per core
NCH = HID // 128         # 4 feature chunks (2 heads each)
NT = L // 128            # 8 token chunks
SW = 16.0                # fp8 weight prescale

USE_FP8 = True


def _build(apply_bo: bool, apply_affine: bool, use_fp8: bool):
    import concourse.bass as bass
    from concourse import bacc
    import concourse.mybir as mybir
    from concourse.tile import TileContext

    F32 = mybir.dt.float32
    F32R = mybir.dt.float32r
    BF16 = mybir.dt.bfloat16
    FP8 = mybir.dt.float8e4
    AT = mybir.ActivationFunctionType
    OP = mybir.AluOpType
    PM = mybir.MatmulPerfMode

    nc = bacc.Bacc()

    import concourse.bacc as _bacc_mod
    _orig_gat = _bacc_mod.get_activation_tables

    def _gat(arch):
        # Keep dict order but strip our funcs from every other set, so the
        # table-load pass assigns all of them to natural_log_exp_and_others
        # -> a single physical table load.
        t = _orig_gat(arch)
        ours = {AT.Exp, AT.Ln, AT.Copy, AT.Relu, AT.Identity}
        out = {}
        for k, funcs in t.items():
            if k == "natural_log_exp_and_others":
                out[k] = funcs
            else:
                out[k] = {f for f in funcs if f not in ours}
        return out

    # ---------------- dram tensors ----------------
    xtb = nc.dram_tensor("xtb", [HID, G * L], BF16, kind="ExternalInput")
    segs8d = nc.dram_tensor("segs8", [128, 2 * 32], FP8, kind="ExternalInput")
    if use_fp8:
        xt8 = nc.dram_tensor("xt8", [HID, G * L], FP8, kind="ExternalInput")
        dxt8 = nc.dram_tensor("dxt8", [HID, G * L], FP8, kind="ExternalInput")
        wq8d = nc.dram_tensor("wq8", [128, NCH * 2 * 256], FP8, kind="ExternalInput")
        wk8d = nc.dram_tensor("wk8", [128, NCH * 2 * 256], FP8, kind="ExternalInput")
        dwq8d = nc.dram_tensor("dwq8", [128, NCH * 2 * 256], FP8, kind="ExternalInput")
        dwk8d = nc.dram_tensor("dwk8", [128, NCH * 2 * 256], FP8, kind="ExternalInput")
    else:
        wktd = nc.dram_tensor("wkt", [HID, HID], BF16, kind="ExternalInput")
    wqtd = nc.dram_tensor("wqt", [HID, HID], BF16, kind="ExternalInput")
    wvrd = nc.dram_tensor("wvr", [HID, HID], BF16, kind="ExternalInput")
    wotd = nc.dram_tensor("wot", [HID, HID], BF16, kind="ExternalInput")
    wrbdd = nc.dram_tensor("wrbd", [128, 128], BF16, kind="ExternalInput")
    identd = nc.dram_tensor("ident", [128, 128], BF16, kind="ExternalInput")
    segsd = nc.dram_tensor("segs", [128, 8 * NCH], BF16, kind="ExternalInput")
    selsd = nc.dram_tensor("sels", [8, HID], F32, kind="ExternalInput")
    wacold = nc.dram_tensor("wacol", [128, NCH], F32, kind="ExternalInput")
    wbcold = nc.dram_tensor("wbcol", [128, NCH], F32, kind="ExternalInput")
    if apply_bo:
        bod = nc.dram_tensor("bo", [1, HID], F32, kind="ExternalInput")
        onesd = nc.dram_tensor("ones1", [1, 128], F32, kind="ExternalInput")
    if apply_affine:
        lngd = nc.dram_tensor("ln_g", [128, HID], BF16, kind="ExternalInput")
        lnbd = nc.dram_tensor("ln_b", [128, HID], BF16, kind="ExternalInput")
    outd = nc.dram_tensor("out", [G * L, HID], BF16, kind="ExternalOutput")

    SWV = SW if use_fp8 else 1.0

    with TileContext(nc) as tc:
        with tc.tile_pool(name="consts", bufs=1) as cp, \
             tc.tile_pool(name="big", bufs=1) as bp, \
             tc.tile_pool(name="small", bufs=2) as sp, \
             tc.tile_pool(name="psum", bufs=1, space="PSUM") as ppool:

            # ---- constants to SBUF ----
            # Q-path consts first (on sync, ahead of xtb(0)); the rest spread
            # across the scalar/vector/gpsimd DGE queues so the preamble
            # parallelizes instead of serializing on SP.
            wqt_sb = [cp.tile([128, HID], BF16, name=f"wqt{i}") for i in range(NCH)]
            for i in range(NCH):
                nc.sync.dma_start(out=wqt_sb[i], in_=wqtd.ap()[128 * i:128 * (i + 1), :])
            wacol_sb = cp.tile([128, NCH], F32)
            nc.sync.dma_start(out=wacol_sb, in_=wacold.ap())
            segs8_sb = cp.tile([128, 2 * 32], FP8)
            nc.scalar.dma_start(out=segs8_sb, in_=segs8d.ap())
            if use_fp8:
                wq8_sb = cp.tile([128, NCH * 2 * 256], FP8)
                nc.scalar.dma_start(out=wq8_sb, in_=wq8d.ap())
                wk8_sb = cp.tile([128, NCH * 2 * 256], FP8)
                nc.scalar.dma_start(out=wk8_sb, in_=wk8d.ap())
                dwq8_sb = cp.tile([128, NCH * 2 * 256], FP8)
                nc.scalar.dma_start(out=dwq8_sb, in_=dwq8d.ap())
                dwk8_sb = cp.tile([128, NCH * 2 * 256], FP8)
                nc.scalar.dma_start(out=dwk8_sb, in_=dwk8d.ap())
            else:
                wkt_sb = [cp.tile([128, HID], BF16, name=f"wkt{i}") for i in range(NCH)]
                for i in range(NCH):
                    nc.scalar.dma_start(out=wkt_sb[i], in_=wktd.ap()[128 * i:128 * (i + 1), :])
            wvr_sb = [cp.tile([128, HID], BF16, name=f"wvr{j}") for j in range(NCH)]
            for j in range(NCH):
                nc.gpsimd.dma_start(out=wvr_sb[j], in_=wvrd.ap()[128 * j:128 * (j + 1), :])
            wot_sb = [cp.tile([128, HID], BF16, name=f"wot{j}") for j in range(NCH)]
            for j in range(NCH):
                nc.gpsimd.dma_start(out=wot_sb[j], in_=wotd.ap()[128 * j:128 * (j + 1), :])
            wrbd_sb = cp.tile([128, 128], BF16)
            nc.scalar.dma_start(out=wrbd_sb, in_=wrbdd.ap())
            ident_sb = cp.tile([128, 128], BF16)
            nc.scalar.dma_start(out=ident_sb, in_=identd.ap())
            segs_sb = cp.tile([128, 8 * NCH], BF16)
            nc.scalar.dma_start(out=segs_sb, in_=segsd.ap())
            sels_sb = cp.tile([8, HID], F32R)
            nc.scalar.dma_start(out=sels_sb, in_=selsd.ap().bitcast(F32R))
            wbcol_sb = cp.tile([128, NCH], F32)
            nc.scalar.dma_start(out=wbcol_sb, in_=wbcold.ap())
            if apply_bo:
                ones1_sb = cp.tile([1, 128], F32R)
                nc.scalar.dma_start(out=ones1_sb, in_=onesd.ap().bitcast(F32R))
                bo_sb = cp.tile([1, HID], F32R)
                nc.scalar.dma_start(out=bo_sb, in_=bod.ap().bitcast(F32R))
            if apply_affine:
                lng_sb = cp.tile([128, HID], BF16)
                nc.gpsimd.dma_start(out=lng_sb, in_=lngd.ap())
                lnb_sb = cp.tile([128, HID], BF16)
                nc.gpsimd.dma_start(out=lnb_sb, in_=lnbd.ap())

            EDT = BF16

            # -------- per-graph state (software-pipelined emission) --------
            st = {}

            def phase_load(g):
                s = {}
                s["xtb"] = bp.tile([128, NCH * L], BF16, name=f"xtb{g}", tag="xtb",
                                   bufs=3)
                xtb_src = bass.AP(
                    tensor=xtb.ap().tensor, offset=g * L,
                    ap=[[G * L, 128], [128 * G * L, NCH], [1, L]])
                nc.sync.dma_start(
                    out=s["xtb"].rearrange("p (i l) -> p i l", i=NCH), in_=xtb_src)
                if use_fp8:
                    s["xt8"] = bp.tile([128, NCH * L], FP8, name=f"xt8{g}",
                                       tag="xt8", bufs=2)
                    xt8_src = bass.AP(
                        tensor=xt8.ap().tensor, offset=g * L,
                        ap=[[G * L, 128], [128 * G * L, NCH], [1, L]])
                    nc.scalar.dma_start(
                        out=s["xt8"].rearrange("p (i l) -> p i l", i=NCH),
                        in_=xt8_src)
                    s["dxt8"] = bp.tile([128, NCH * L], FP8, name=f"dxt8{g}",
                                        tag="dxt8", bufs=2)
                    dxt8_src = bass.AP(
                        tensor=dxt8.ap().tensor, offset=g * L,
                        ap=[[G * L, 128], [128 * G * L, NCH], [1, L]])
                    nc.gpsimd.dma_start(
                        out=s["dxt8"].rearrange("p (i l) -> p i l", i=NCH),
                        in_=dxt8_src)
                st[g] = s

            def proj_half(g, w8_sb, w_sb, j, n0, pp):
                if use_fp8:
                    w8, dw8 = w8_sb
                    xt8_3d = st[g]["xt8"].rearrange("p (i l) -> p i l", i=NCH)
                    dxt8_3d = st[g]["dxt8"].rearrange("p (i l) -> p i l", i=NCH)
                    # psum = X8 @ W8 + X8 @ dW8 + dX8 @ W8  (~bf16 accuracy)
                    plan = [(w8, xt8_3d), (dw8, xt8_3d), (w8, dxt8_3d)]
                    nmm = len(plan) * 2
                    k = 0
                    for wsb, xsb in plan:
                        for p in range(2):
                            lhs = wsb[:, (2 * j + p) * 256:(2 * j + p + 1) * 256] \
                                .rearrange("p (two f) -> p two f", two=2)
                            rhs = xsb[:, 2 * p:2 * p + 2, n0:n0 + 512]
                            nc.tensor.matmul(pp, lhs, rhs, start=(k == 0),
                                             stop=(k == nmm - 1),
                                             perf_mode=PM.DoubleRow)
                            k += 1
                else:
                    for i in range(NCH):
                        nc.tensor.matmul(
                            pp, w_sb[i][:, 128 * j:128 * (j + 1)],
                            st[g]["xtb"][:, i * L + n0: i * L + n0 + 512],
                            start=(i == 0), stop=(i == NCH - 1))

            def proj_stage(g, tag, w8_sb, w_sb, scale_ap):
                """projection + exp + m for one of Q/K; then seg-sums + recip."""
                e_all = bp.tile([128, NCH * L], EDT, name=f"e{tag}{g}", tag="e",
                                bufs=3)
                m_tiles = []
                for j in range(NCH):
                    mt = sp.tile([128, L], BF16, name=f"m{tag}{g}{j}", tag="scr",
                                 bufs=4)
                    m_tiles.append(mt)
                for j in range(NCH):
                    for n0 in (0, 512):
                        pp = ppool.tile([128, 512], F32, name=f"pp{tag}{g}{j}{n0}",
                                        tag="pp", bufs=5)
                        proj_half(g, w8_sb, w_sb, j, n0, pp)
                        nc.scalar.activation(
                            out=e_all[:, j * L + n0: j * L + n0 + 512], in_=pp,
                            func=AT.Exp, scale=scale_ap[:, j:j + 1])
                        nc.vector.tensor_mul(
                            out=m_tiles[j][:, n0:n0 + 512],
                            in0=e_all[:, j * L + n0: j * L + n0 + 512], in1=pp)
                sos = []
                for hi, n0 in enumerate((0, 512)):
                    so = ppool.tile([16, 512], F32, name=f"so{tag}{g}{n0}",
                                    tag="rs", bufs=3, padded_shape=[128, 512])
                    for j in range(NCH):
                        nc.tensor.matmul(
                            so[0:8, :], segs_sb[:, 8 * j:8 * (j + 1)],
                            e_all[:, j * L + n0: j * L + n0 + 512],
                            start=(j == 0), stop=(j == NCH - 1))
                    sos.append(so)
                rt = sp.tile([8, 1024], F32, name=f"rt{tag}{g}", tag="rt", bufs=3)
                nc.vector.reciprocal_approx_fast(out=rt[:, 0:512], in_=sos[0][0:8, :])
                nc.vector.reciprocal_approx_fast(out=rt[:, 512:1024], in_=sos[1][0:8, :])
                rtr = sp.tile([8, 1024], F32R, name=f"rtr{tag}{g}", tag="rtr", bufs=3)
                nc.vector.tensor_copy(out=rtr, in_=rt)
                st[g][f"m{tag}"] = m_tiles
                st[g][f"rt{tag}"] = rtr

            def rbc_stage(g, tag):
                """rbc expand + stt accumulate; returns summed [128, NCH] tile."""
                rtr = st[g][f"rt{tag}"]
                m_tiles = st[g][f"m{tag}"]
                parts = sp.tile([128, 2 * NCH], F32, name=f"pts{tag}{g}",
                                tag=f"pts_{tag}")
                for j in range(NCH):
                    for hi, n0 in enumerate((0, 512)):
                        rbc = ppool.tile([128, 512], F32, name=f"rbc{tag}{g}{j}{n0}",
                                         tag="rs", bufs=3)
                        nc.tensor.matmul(rbc, sels_sb[:, 128 * j:128 * (j + 1)],
                                         rtr[:, n0:n0 + 512])
                        nc.vector.scalar_tensor_tensor(
                            out=m_tiles[j][:, n0:n0 + 512],
                            in0=m_tiles[j][:, n0:n0 + 512],
                            scalar=1.0 / SWV, in1=rbc,
                            op0=OP.mult, op1=OP.mult,
                            accum_out=parts[:, hi * NCH + j:hi * NCH + j + 1])
                tot = sp.tile([128, NCH], F32, name=f"tot{tag}{g}", tag=f"tot{tag}")
                nc.gpsimd.tensor_add(out=tot, in0=parts[:, 0:NCH],
                                     in1=parts[:, NCH:2 * NCH])
                return tot

            def phase_A(g):
                proj_stage(g, "a", (wq8_sb, dwq8_sb) if use_fp8 else None,
                           None if use_fp8 else wqt_sb, wacol_sb)

            def phase_rbcA(g):
                gq = rbc_stage(g, "a")
                gqwb = sp.tile([128, NCH], F32, name=f"gqwb{g}", tag="gqwb")
                nc.gpsimd.tensor_mul(out=gqwb, in0=gq, in1=wbcol_sb)
                st[g]["gq"] = gq
                st[g]["gqwb"] = gqwb

            def phase_K(g):
                proj_stage(g, "b", (wk8_sb, dwk8_sb) if use_fp8 else None,
                           None if use_fp8 else wkt_sb, st[g]["gqwb"])

            def phase_rbcB(g):
                acc = rbc_stage(g, "b")
                gk = sp.tile([128, NCH], F32, name=f"gk{g}", tag="gk")
                nc.gpsimd.tensor_mul(out=gk, in0=acc, in1=st[g]["gq"])
                gkwr = sp.tile([128, NCH * 128], BF16, name=f"gkwr{g}", tag="gkwr")
                for j in range(NCH):
                    nc.gpsimd.tensor_scalar_mul(
                        out=gkwr[:, 128 * j:128 * (j + 1)], in0=wrbd_sb,
                        scalar1=gk[:, j:j + 1])
                st[g]["gkwr"] = gkwr

            def phase_prep(g):
                gkwr = st[g]["gkwr"]
                w3_sb = []
                for i in range(NCH):
                    ppw = ppool.tile([128, 512], F32, name=f"ppw{g}{i}", tag="pp",
                                     bufs=5)
                    for j in range(NCH):
                        nc.tensor.matmul(
                            ppw[:, 128 * j:128 * (j + 1)],
                            wvr_sb[j][:, 128 * i:128 * (i + 1)],
                            gkwr[:, 128 * j:128 * (j + 1)],
                            start=True, stop=False)
                        nc.tensor.matmul(
                            ppw[:, 128 * j:128 * (j + 1)], ident_sb,
                            wqt_sb[i][:, 128 * j:128 * (j + 1)],
                            start=False, stop=True)
                    w3 = sp.tile([128, 512], BF16, name=f"w3{g}{i}", tag="w3",
                                 bufs=8)
                    nc.scalar.copy(out=w3, in_=ppw)
                    w3_sb.append(w3)
                st[g]["w3"] = w3_sb

            def phase_stream(g):
                w3_sb = st[g]["w3"]
                xtb_all = st[g]["xtb"]
                att_all = bp.tile([128, NCH * L], BF16, name=f"att{g}", tag="att",
                                  bufs=2)
                for j in range(NCH):
                    for n0 in (0, 512):
                        ppv = ppool.tile([128, 512], F32, name=f"ppv{g}{j}{n0}",
                                         tag="pp", bufs=5)
                        for i in range(NCH):
                            nc.tensor.matmul(
                                ppv, w3_sb[i][:, 128 * j:128 * (j + 1)],
                                xtb_all[:, i * L + n0: i * L + n0 + 512],
                                start=(i == 0), stop=(i == NCH - 1))
                        nc.scalar.activation(
                            out=att_all[:, j * L + n0: j * L + n0 + 512], in_=ppv,
                            func=AT.Relu)
                st[g]["att"] = att_all

            def ln_apply(g, obs, mv_all, rstd_all, t):
                och = sp.tile([128, HID], BF16, name=f"och{g}{t}", tag="och",
                              bufs=4)
                nc.gpsimd.tensor_scalar(
                    out=och, in0=obs[t], scalar1=mv_all[:, 2 * t:2 * t + 1],
                    scalar2=rstd_all[:, t:t + 1], op0=OP.subtract, op1=OP.mult)
                if apply_affine:
                    nc.vector.tensor_mul(out=och, in0=och, in1=lng_sb)
                    nc.vector.tensor_add(out=och, in0=och, in1=lnb_sb)
                nc.sync.dma_start(
                    out=outd.ap()[g * L + 128 * t: g * L + 128 * (t + 1), :],
                    in_=och)

            def phase_Wo(g, tail=False):
                att_all = st[g]["att"]
                mv_all = sp.tile([128, 2 * NT], F32, name=f"mv{g}", tag="mv")
                rstd_all = sp.tile([128, NT], F32, name=f"rstd{g}", tag="rstd")
                vf = sp.tile([128, NT], F32, name=f"vf{g}", tag="vf")
                lnv = sp.tile([128, NT], F32, name=f"lnv{g}", tag="lnv")
                obs = []
                for t in range(NT):
                    o_ps = ppool.tile([128, HID], F32, name=f"ops{g}{t}", tag="pp",
                                      bufs=5)
                    last = NCH - 1
                    for j in range(NCH):
                        nc.tensor.matmul(
                            o_ps, att_all[:, j * L + 128 * t: j * L + 128 * (t + 1)],
                            wot_sb[j], start=(j == 0),
                            stop=(j == last and not apply_bo))
                    if apply_bo:
                        nc.tensor.matmul(o_ps, ones1_sb, bo_sb, start=False,
                                         stop=True)
                    ob = sp.tile([128, HID], BF16, name=f"ob{g}{t}", tag="ob",
                                 bufs=NT + 2)
                    nc.scalar.copy(out=ob, in_=o_ps)
                    stats = sp.tile([128, 6], F32, name=f"sst{g}{t}", tag="sst")
                    nc.vector.bn_stats(out=stats, in_=ob)
                    nc.vector.bn_aggr(out=mv_all[:, 2 * t:2 * t + 2], in_=stats)
                    obs.append(ob)
                    if tail:
                        nc.vector.tensor_scalar_add(
                            out=vf[:, t:t + 1], in0=mv_all[:, 2 * t + 1:2 * t + 2],
                            scalar1=EPS)
                        nc.scalar.activation(out=lnv[:, t:t + 1], in_=vf[:, t:t + 1],
                                             func=AT.Ln)
                        nc.scalar.activation(out=rstd_all[:, t:t + 1],
                                             in_=lnv[:, t:t + 1], func=AT.Exp,
                                             scale=-0.5)
                        ln_apply(g, obs, mv_all, rstd_all, t)
                if not tail:
                    nc.gpsimd.tensor_scalar_add(out=vf, in0=mv_all[:, 1:2 * NT:2],
                                                scalar1=EPS)
                    nc.scalar.activation(out=lnv, in_=vf, func=AT.Ln)
                    nc.scalar.activation(out=rstd_all, in_=lnv, func=AT.Exp, scale=-0.5)
                    for t in range(NT):
                        ln_apply(g, obs, mv_all, rstd_all, t)
                del st[g]

            # -------- modulo schedule: stream/Wo of g-1 fill g's stt chains ----
            phase_load(0)
            for g in range(G):
                if g + 1 < G:
                    phase_load(g + 1)
                phase_A(g)
                phase_rbcA(g)
                if g > 0:
                    phase_stream(g - 1)
                phase_K(g)
                phase_rbcB(g)
                if g > 0:
                    phase_Wo(g - 1)
                phase_prep(g)
            phase_stream(G - 1)
            phase_Wo(G - 1, tail=True)

    _bacc_mod.get_activation_tables = _gat
    try:
        nc.compile()
    finally:
        _bacc_mod.get_activation_tables = _orig_gat
    return nc


_NC_CACHE = {}


def _get_nc(apply_bo, apply_affine):
    key = (apply_bo, apply_affine, USE_FP8)
    if key not in _NC_CACHE:
        _NC_CACHE[key] = _build(apply_bo, apply_affine, USE_FP8)
    return _NC_CACHE[key]


def _host_consts(Wq, Wk, Wv, Wr, w_alpha, w_beta, Wo, bo, ln_g, ln_b):
    import ml_dtypes
    bf = ml_dtypes.bfloat16
    f8 = ml_dtypes.float8_e4m3fn

    wqt = np.ascontiguousarray(Wq.T)                       # [h, e]
    wvr = np.ascontiguousarray(Wv)                         # [d, h]
    wot = np.ascontiguousarray(Wo.T)
    wrt = Wr.T.astype(np.float32)                          # WrT[d, e] = Wr[e, d]
    wrbd = np.zeros((128, 128), np.float32)
    wrbd[:64, :64] = wrt; wrbd[64:, 64:] = wrt
    ident = np.eye(128, dtype=np.float32)
    wa_vec = np.tile(w_alpha, HEADS) * SCALE               # [512]
    wb_vec = np.tile(w_beta, HEADS) * SCALE
    SWV = SW if USE_FP8 else 1.0
    wacol = (wa_vec / SWV).reshape(NCH, 128).T.copy()      # [128, NCH]
    wbcol = (wb_vec / SWV).reshape(NCH, 128).T.copy()

    segs = np.zeros((128, 8 * NCH), np.float32)
    sels = np.zeros((8, HID), np.float32)
    for j in range(NCH):
        for p in range(128):
            segs[p, 8 * j + 2 * j + p // 64] = 1.0
        for m in range(HID):
            if m // 128 == j:
                sels[2 * j + (m % 128) // 64, m] = 1.0
    segs8 = np.zeros((128, 2 * 32), np.float32)
    for p in range(2):
        for d in range(128):
            hA = 4 * p + d // 64          # head of (chunk 2p, partition d)
            hB = 4 * p + 2 + d // 64      # head of (chunk 2p+1, partition d)
            segs8[d, 32 * p + hA] = 1.0
            segs8[d, 32 * p + 16 + hB] = 1.0

    common = {"wqt": wqt.astype(bf), "wvr": wvr.astype(bf),
              "wot": wot.astype(bf), "wrbd": wrbd.astype(bf),
              "ident": ident.astype(bf), "segs": segs.astype(bf),
              "segs8": segs8.astype(f8),
              "sels": sels, "wacol": wacol.astype(np.float32),
              "wbcol": wbcol.astype(np.float32)}

    if USE_FP8:
        def pack_dr(WT):   # WT [h, e] -> [128, NCH*2*256] DoubleRow stationary
            out = np.zeros((128, NCH * 2 * 256), np.float32)
            for j in range(NCH):
                for p in range(2):
                    blkA = WT[256 * p:256 * p + 128, 128 * j:128 * (j + 1)]
                    blkB = WT[256 * p + 128:256 * (p + 1), 128 * j:128 * (j + 1)]
                    c0 = (2 * j + p) * 256
                    out[:, c0:c0 + 128] = blkA
                    out[:, c0 + 128:c0 + 256] = blkB
            return out
        wq_pk = pack_dr(SW * Wq.T)
        wk_pk = pack_dr(SW * Wk.T)
        wq8 = wq_pk.astype(f8)
        wk8 = wk_pk.astype(f8)
        common["wq8"] = wq8
        common["wk8"] = wk8
        common["dwq8"] = (wq_pk - wq8.astype(np.float32)).astype(f8)
        common["dwk8"] = (wk_pk - wk8.astype(np.float32)).astype(f8)
    else:
        common["wkt"] = np.ascontiguousarray(Wk.T).astype(bf)

    apply_bo = not np.allclose(bo, 0.0)
    apply_affine = not (np.allclose(ln_g, 1.0) and np.allclose(ln_b, 0.0))
    if apply_bo:
        common["bo"] = bo.reshape(1, HID).astype(np.float32)
        common["ones1"] = np.ones((1, 128), np.float32)
    if apply_affine:
        common["ln_g"] = np.tile(ln_g, (128, 1)).astype(bf)
        common["ln_b"] = np.tile(ln_b, (128, 1)).astype(bf)
    return common, apply_bo, apply_affine


def kernel(edge_attr, batch_scopes, Wq, Wk, Wv, Wr, w_alpha, w_beta, Wo, bo,
           ln_g, ln_b):
    from concourse import bass_utils
    import ml_dtypes

    edge_attr = np.asarray(edge_attr, dtype=np.float32)
    scopes = np.asarray(batch_scopes)
    Wq = np.asarray(Wq, np.float32); Wk = np.asarray(Wk, np.float32)
    Wv = np.asarray(Wv, np.float32); Wr = np.asarray(Wr, np.float32)
    Wo = np.asarray(Wo, np.float32)
    w_alpha = np.asarray(w_alpha, np.float32); w_beta = np.asarray(w_beta, np.float32)
    bo = np.asarray(bo, np.float32)
    ln_g = np.asarray(ln_g, np.float32); ln_b = np.asarray(ln_b, np.float32)

    assert np.all(scopes[:, 1] == L), "equal-length contiguous scopes expected"
    starts = scopes[:, 0].astype(np.int64)

    common, apply_bo, apply_affine = _host_consts(
        Wq, Wk, Wv, Wr, w_alpha, w_beta, Wo, bo, ln_g, ln_b)
    nc = _get_nc(apply_bo, apply_affine)

    bf = ml_dtypes.bfloat16
    f8 = ml_dtypes.float8_e4m3fn
    in_maps = []
    for c in range(NCORES):
        rows = np.concatenate([
            np.arange(starts[c * G + g], starts[c * G + g] + L)
            for g in range(G)])
        xslab = edge_attr[rows]                       # [G*L, 512]
        xt = np.ascontiguousarray(xslab.T)
        m = {"xtb": xt.astype(bf), **common}
        if USE_FP8:
            x8 = xt.astype(f8)
            m["xt8"] = x8
            m["dxt8"] = (xt - x8.astype(np.float32)).astype(f8)
        in_maps.append(m)

    res = bass_utils.run_bass_kernel_spmd(nc, in_maps, core_ids=list(range(NCORES)))
    out = np.concatenate([r["out"] for r in res.results], axis=0)
    return out.astype(np.float32)


# revision 26
# speedup vs baseline: 1.1157x; 1.0273x over previous
"""BondFastAttention Trainium2 kernel (self-contained), v2.

Shapes (hardcoded from the problem spec):
  edge_attr [65536, 512] fp32, B=64 graphs x L=1024 bonds, HID=512, 8 heads x D=64.
  8 NeuronCores, data-parallel over graphs: G=8 graphs per core.

Device layout: transposed domain - features on partitions, tokens on free dim
for Q/K/kvout; tokens on partitions for the Wo/LayerNorm stage.

Key structure vs v1:
  - The V projection, Wr matmul, gk scaling and +q add are all folded into a
    single per-graph combined weight W''' = Wv^T (gk . Wr^T) + Wq^T, built on
    the PE (16 small matmuls + identity-add), so one X-stream produces
    relu-input directly.
  - Projection PSUM is consumed in place (ACT exp, Pool multiply) - no
    psum->sbuf copies for q/k/v.
  - Optional fp8 path: Q/K projections and softmax seg-sums run as fp8
    DoubleRow matmuls (K=256 per pass).
  - Output is written bf16 and upcast to f32 on the host.
"""
import numpy as np

HID = 512
HEADS = 8
D = 64
B = 64
L = 1024
SCALE = D ** -0.5
EPS = 1e-5
NCORES = 8
G = B // NCORES          # graphs per core
NCH = HID // 128         # 4 feature chunks (2 heads each)
NT = L // 128            # 8 token chunks
SW = 16.0                # fp8 weight prescale

USE_FP8 = True


def _build(apply_bo: bool, apply_affine: bool, use_fp8: bool):
    import concourse.bass as bass
    from concourse import bacc
    import concourse.mybir as mybir
    from concourse.tile import TileContext

    F32 = mybir.dt.float32
    F32R = mybir.dt.float32r
    BF16 = mybir.dt.bfloat16
    FP8 = mybir.dt.float8e4
    AT = mybir.ActivationFunctionType
    OP = mybir.AluOpType
    PM = mybir.MatmulPerfMode

    nc = bacc.Bacc()

    import concourse.bacc as _bacc_mod
    _orig_gat = _bacc_mod.get_activation_tables

    def _gat(arch):
        # Keep dict order but strip our funcs from every other set, so the
        # table-load pass assigns all of them to natural_log_exp_and_others
        # -> a single physical table load.
        t = _orig_gat(arch)
        ours = {AT.Exp, AT.Ln, AT.Copy, AT.Relu, AT.Identity}
        out = {}
        for k, funcs in t.items():
            if k == "natural_log_exp_and_others":
                out[k] = funcs
            else:
                out[k] = {f for f in funcs if f not in ours}
        return out

    # ---------------- dram tensors ----------------
    xtb = nc.dram_tensor("xtb", [HID, G * L], BF16, kind="ExternalInput")
    segs8d = nc.dram_tensor("segs8", [128, 2 * 32], FP8, kind="ExternalInput")
    if use_fp8:
        xt8 = nc.dram_tensor("xt8", [HID, G * L], FP8, kind="ExternalInput")
        dxt8 = nc.dram_tensor("dxt8", [HID, G * L], FP8, kind="ExternalInput")
        wq8d = nc.dram_tensor("wq8", [128, NCH * 2 * 256], FP8, kind="ExternalInput")
        wk8d = nc.dram_tensor("wk8", [128, NCH * 2 * 256], FP8, kind="ExternalInput")
        dwq8d = nc.dram_tensor("dwq8", [128, NCH * 2 * 256], FP8, kind="ExternalInput")
        dwk8d = nc.dram_tensor("dwk8", [128, NCH * 2 * 256], FP8, kind="ExternalInput")
    else:
        wktd = nc.dram_tensor("wkt", [HID, HID], BF16, kind="ExternalInput")
    wqtd = nc.dram_tensor("wqt", [HID, HID], BF16, kind="ExternalInput")
    wvrd = nc.dram_tensor("wvr", [HID, HID], BF16, kind="ExternalInput")
    wotd = nc.dram_tensor("wot", [HID, HID], BF16, kind="ExternalInput")
    wrbdd = nc.dram_tensor("wrbd", [128, 128], BF16, kind="ExternalInput")
    identd = nc.dram_tensor("ident", [128, 128], BF16, kind="ExternalInput")
    segsd = nc.dram_tensor("segs", [128, 8 * NCH], BF16, kind="ExternalInput")
    selsd = nc.dram_tensor("sels", [8, HID], F32, kind="ExternalInput")
    wacold = nc.dram_tensor("wacol", [128, NCH], F32, kind="ExternalInput")
    wbcold = nc.dram_tensor("wbcol", [128, NCH], F32, kind="ExternalInput")
    if apply_bo:
        bod = nc.dram_tensor("bo", [1, HID], F32, kind="ExternalInput")
        onesd = nc.dram_tensor("ones1", [1, 128], F32, kind="ExternalInput")
    if apply_affine:
        lngd = nc.dram_tensor("ln_g", [128, HID], BF16, kind="ExternalInput")
        lnbd = nc.dram_tensor("ln_b", [128, HID], BF16, kind="ExternalInput")
    outd = nc.dram_tensor("out", [G * L, HID], BF16, kind="ExternalOutput")

    SWV = SW if use_fp8 else 1.0

    with TileContext(nc) as tc:
        with tc.tile_pool(name="consts", bufs=1) as cp, \
             tc.tile_pool(name="big", bufs=1) as bp, \
             tc.tile_pool(name="small", bufs=2) as sp, \
             tc.tile_pool(name="psum", bufs=1, space="PSUM") as ppool:

            # ---- constants to SBUF ----
            # Phase-0-critical consts first; bulk consts go AFTER the first
            # graph's X loads are queued, spread across SP/ACT/Pool DGEs.
            wacol_sb = cp.tile([128, NCH], F32)
            nc.sync.dma_start(out=wacol_sb, in_=wacold.ap())
            if use_fp8:
                wq8_sb = cp.tile([128, NCH * 2 * 256], FP8)
                nc.sync.dma_start(out=wq8_sb, in_=wq8d.ap())
                wk8_sb = cp.tile([128, NCH * 2 * 256], FP8)
                nc.scalar.dma_start(out=wk8_sb, in_=wk8d.ap())
                dwq8_sb = cp.tile([128, NCH * 2 * 256], FP8)
                nc.gpsimd.dma_start(out=dwq8_sb, in_=dwq8d.ap())
                dwk8_sb = cp.tile([128, NCH * 2 * 256], FP8)
                nc.scalar.dma_start(out=dwk8_sb, in_=dwk8d.ap())
            else:
                wkt_sb = [cp.tile([128, HID], BF16, name=f"wkt{i}") for i in range(NCH)]
            wqt_sb = [cp.tile([128, HID], BF16, name=f"wqt{i}") for i in range(NCH)]
            segs_sb = cp.tile([128, 8 * NCH], BF16)
            nc.scalar.dma_start(out=segs_sb, in_=segsd.ap())
            sels_sb = cp.tile([8, HID], F32R)
            nc.sync.dma_start(out=sels_sb, in_=selsd.ap().bitcast(F32R))

            def load_bulk_consts():
                for i in range(NCH):
                    nc.sync.dma_start(out=wqt_sb[i],
                                      in_=wqtd.ap()[128 * i:128 * (i + 1), :])
                if not use_fp8:
                    for i in range(NCH):
                        nc.scalar.dma_start(
                            out=wkt_sb[i], in_=wktd.ap()[128 * i:128 * (i + 1), :])
                for j in range(NCH):
                    nc.gpsimd.dma_start(out=wvr_sb[j],
                                        in_=wvrd.ap()[128 * j:128 * (j + 1), :])
                for j in range(NCH):
                    nc.gpsimd.dma_start(out=wot_sb[j],
                                        in_=wotd.ap()[128 * j:128 * (j + 1), :])
                nc.scalar.dma_start(out=wrbd_sb, in_=wrbdd.ap())
                nc.scalar.dma_start(out=ident_sb, in_=identd.ap())
                nc.scalar.dma_start(out=segs8_sb, in_=segs8d.ap())
                nc.scalar.dma_start(out=wbcol_sb, in_=wbcold.ap())
                if apply_bo:
                    nc.scalar.dma_start(out=ones1_sb, in_=onesd.ap().bitcast(F32R))
                    nc.scalar.dma_start(out=bo_sb, in_=bod.ap().bitcast(F32R))
                if apply_affine:
                    nc.gpsimd.dma_start(out=lng_sb, in_=lngd.ap())
                    nc.gpsimd.dma_start(out=lnb_sb, in_=lnbd.ap())

            wvr_sb = [cp.tile([128, HID], BF16, name=f"wvr{j}") for j in range(NCH)]
            wot_sb = [cp.tile([128, HID], BF16, name=f"wot{j}") for j in range(NCH)]
            wrbd_sb = cp.tile([128, 128], BF16)
            ident_sb = cp.tile([128, 128], BF16)
            segs8_sb = cp.tile([128, 2 * 32], FP8)
            wbcol_sb = cp.tile([128, NCH], F32)
            if apply_bo:
                ones1_sb = cp.tile([1, 128], F32R)
                bo_sb = cp.tile([1, HID], F32R)
            if apply_affine:
                lng_sb = cp.tile([128, HID], BF16)
                lnb_sb = cp.tile([128, HID], BF16)

            EDT = BF16

            # -------- per-graph state (software-pipelined emission) --------
            st = {}

            def phase_load(g):
                s = {}
                s["xtb"] = bp.tile([128, NCH * L], BF16, name=f"xtb{g}", tag="xtb",
                                   bufs=4)
                xtb_src = bass.AP(
                    tensor=xtb.ap().tensor, offset=g * L,
                    ap=[[G * L, 128], [128 * G * L, NCH], [1, L]])
                nc.sync.dma_start(
                    out=s["xtb"].rearrange("p (i l) -> p i l", i=NCH), in_=xtb_src)
                if use_fp8:
                    s["xt8"] = bp.tile([128, NCH * L], FP8, name=f"xt8{g}",
                                       tag="xt8", bufs=3)
                    xt8_src = bass.AP(
                        tensor=xt8.ap().tensor, offset=g * L,
                        ap=[[G * L, 128], [128 * G * L, NCH], [1, L]])
                    nc.scalar.dma_start(
                        out=s["xt8"].rearrange("p (i l) -> p i l", i=NCH),
                        in_=xt8_src)
                    s["dxt8"] = bp.tile([128, NCH * L], FP8, name=f"dxt8{g}",
                                        tag="dxt8", bufs=3)
                    dxt8_src = bass.AP(
                        tensor=dxt8.ap().tensor, offset=g * L,
                        ap=[[G * L, 128], [128 * G * L, NCH], [1, L]])
                    nc.gpsimd.dma_start(
                        out=s["dxt8"].rearrange("p (i l) -> p i l", i=NCH),
                        in_=dxt8_src)
                st[g] = s

            def proj_half(g, w8_sb, w_sb, j, n0, pp):
                if use_fp8:
                    w8, dw8 = w8_sb
                    xt8_3d = st[g]["xt8"].rearrange("p (i l) -> p i l", i=NCH)
                    dxt8_3d = st[g]["dxt8"].rearrange("p (i l) -> p i l", i=NCH)
                    # psum = X8 @ W8 + X8 @ dW8 + dX8 @ W8  (~bf16 accuracy)
                    plan = [(w8, xt8_3d), (dw8, xt8_3d), (w8, dxt8_3d)]
                    nmm = len(plan) * 2
                    k = 0
                    for wsb, xsb in plan:
                        for p in range(2):
                            lhs = wsb[:, (2 * j + p) * 256:(2 * j + p + 1) * 256] \
                                .rearrange("p (two f) -> p two f", two=2)
                            rhs = xsb[:, 2 * p:2 * p + 2, n0:n0 + 512]
                            nc.tensor.matmul(pp, lhs, rhs, start=(k == 0),
                                             stop=(k == nmm - 1),
                                             perf_mode=PM.DoubleRow)
                            k += 1
                else:
                    for i in range(NCH):
                        nc.tensor.matmul(
                            pp, w_sb[i][:, 128 * j:128 * (j + 1)],
                            st[g]["xtb"][:, i * L + n0: i * L + n0 + 512],
                            start=(i == 0), stop=(i == NCH - 1))

            def proj_stage(g, tag, w8_sb, w_sb, scale_ap):
                """projection + exp + m for one of Q/K; then seg-sums + recip."""
                e_all = bp.tile([128, NCH * L], EDT, name=f"e{tag}{g}", tag="e",
                                bufs=3)
                m_tiles = []
                for j in range(NCH):
                    mt = sp.tile([128, L], BF16, name=f"m{tag}{g}{j}", tag="scr",
                                 bufs=8)
                    m_tiles.append(mt)
                for j in range(NCH):
                    for n0 in (0, 512):
                        pp = ppool.tile([128, 512], F32, name=f"pp{tag}{g}{j}{n0}",
                                        tag="pp", bufs=5)
                        proj_half(g, w8_sb, w_sb, j, n0, pp)
                        nc.scalar.activation(
                            out=e_all[:, j * L + n0: j * L + n0 + 512], in_=pp,
                            func=AT.Exp, scale=scale_ap[:, j:j + 1])
                        nc.vector.tensor_mul(
                            out=m_tiles[j][:, n0:n0 + 512],
                            in0=e_all[:, j * L + n0: j * L + n0 + 512], in1=pp)
                sos = []
                for hi, n0 in enumerate((0, 512)):
                    so = ppool.tile([16, 512], F32, name=f"so{tag}{g}{n0}",
                                    tag="rs", bufs=3, padded_shape=[128, 512])
                    for j in range(NCH):
                        nc.tensor.matmul(
                            so[0:8, :], segs_sb[:, 8 * j:8 * (j + 1)],
                            e_all[:, j * L + n0: j * L + n0 + 512],
                            start=(j == 0), stop=(j == NCH - 1))
                    sos.append(so)
                rt = sp.tile([8, 1024], F32, name=f"rt{tag}{g}", tag="rt", bufs=3)
                nc.vector.reciprocal_approx_fast(out=rt[:, 0:512], in_=sos[0][0:8, :])
                nc.vector.reciprocal_approx_fast(out=rt[:, 512:1024], in_=sos[1][0:8, :])
                rtr = sp.tile([8, 1024], F32R, name=f"rtr{tag}{g}", tag="rtr", bufs=3)
                nc.vector.tensor_copy(out=rtr, in_=rt)
                st[g][f"m{tag}"] = m_tiles
                st[g][f"rt{tag}"] = rtr

            def rbc_stage(g, tag):
                """rbc expand + stt accumulate; returns summed [128, NCH] tile."""
                rtr = st[g][f"rt{tag}"]
                m_tiles = st[g][f"m{tag}"]
                parts = sp.tile([128, 2 * NCH], F32, name=f"pts{tag}{g}",
                                tag=f"pts_{tag}")
                for j in range(NCH):
                    for hi, n0 in enumerate((0, 512)):
                        rbc = ppool.tile([128, 512], F32, name=f"rbc{tag}{g}{j}{n0}",
                                         tag="rs", bufs=3)
                        nc.tensor.matmul(rbc, sels_sb[:, 128 * j:128 * (j + 1)],
                                         rtr[:, n0:n0 + 512])
                        nc.vector.scalar_tensor_tensor(
                            out=m_tiles[j][:, n0:n0 + 512],
                            in0=m_tiles[j][:, n0:n0 + 512],
                            scalar=1.0 / SWV, in1=rbc,
                            op0=OP.mult, op1=OP.mult,
                            accum_out=parts[:, hi * NCH + j:hi * NCH + j + 1])
                tot = sp.tile([128, NCH], F32, name=f"tot{tag}{g}", tag=f"tot{tag}")
                nc.gpsimd.tensor_add(out=tot, in0=parts[:, 0:NCH],
                                     in1=parts[:, NCH:2 * NCH])
                return tot

            def phase_A(g):
                proj_stage(g, "a", (wq8_sb, dwq8_sb) if use_fp8 else None,
                           None if use_fp8 else wqt_sb, wacol_sb)

            def phase_rbcA(g):
                gq = rbc_stage(g, "a")
                gqwb = sp.tile([128, NCH], F32, name=f"gqwb{g}", tag="gqwb")
                nc.gpsimd.tensor_mul(out=gqwb, in0=gq, in1=wbcol_sb)
                st[g]["gq"] = gq
                st[g]["gqwb"] = gqwb

            def phase_K(g):
                proj_stage(g, "b", (wk8_sb, dwk8_sb) if use_fp8 else None,
                           None if use_fp8 else wkt_sb, st[g]["gqwb"])

            def phase_rbcB(g):
                acc = rbc_stage(g, "b")
                gk = sp.tile([128, NCH], F32, name=f"gk{g}", tag="gk")
                nc.gpsimd.tensor_mul(out=gk, in0=acc, in1=st[g]["gq"])
                gkwr = sp.tile([128, NCH * 128], BF16, name=f"gkwr{g}", tag="gkwr")
                for j in range(NCH):
                    nc.gpsimd.tensor_scalar_mul(
                        out=gkwr[:, 128 * j:128 * (j + 1)], in0=wrbd_sb,
                        scalar1=gk[:, j:j + 1])
                st[g]["gkwr"] = gkwr

            def phase_prep(g):
                gkwr = st[g]["gkwr"]
                w3_sb = []
                for i in range(NCH):
                    ppw = ppool.tile([128, 512], F32, name=f"ppw{g}{i}", tag="pp",
                                     bufs=5)
                    for j in range(NCH):
                        nc.tensor.matmul(
                            ppw[:, 128 * j:128 * (j + 1)],
                            wvr_sb[j][:, 128 * i:128 * (i + 1)],
                            gkwr[:, 128 * j:128 * (j + 1)],
                            start=True, stop=False)
                        nc.tensor.matmul(
                            ppw[:, 128 * j:128 * (j + 1)], ident_sb,
                            wqt_sb[i][:, 128 * j:128 * (j + 1)],
                            start=False, stop=True)
                    w3 = sp.tile([128, 512], BF16, name=f"w3{g}{i}", tag="w3",
                                 bufs=8)
                    nc.scalar.copy(out=w3, in_=ppw)
                    w3_sb.append(w3)
                st[g]["w3"] = w3_sb

            def phase_stream(g):
                w3_sb = st[g]["w3"]
                xtb_all = st[g]["xtb"]
                att_all = bp.tile([128, NCH * L], BF16, name=f"att{g}", tag="att",
                                  bufs=2)
                for j in range(NCH):
                    for n0 in (0, 512):
                        ppv = ppool.tile([128, 512], F32, name=f"ppv{g}{j}{n0}",
                                         tag="pp", bufs=5)
                        for i in range(NCH):
                            nc.tensor.matmul(
                                ppv, w3_sb[i][:, 128 * j:128 * (j + 1)],
                                xtb_all[:, i * L + n0: i * L + n0 + 512],
                                start=(i == 0), stop=(i == NCH - 1))
                        nc.scalar.activation(
                            out=att_all[:, j * L + n0: j * L + n0 + 512], in_=ppv,
                            func=AT.Relu)
                st[g]["att"] = att_all

            def ln_apply(g, obs, mv_all, rstd_all, t):
                och = sp.tile([128, HID], BF16, name=f"och{g}{t}", tag="och",
                              bufs=4)
                nc.gpsimd.tensor_scalar(
                    out=och, in0=obs[t], scalar1=mv_all[:, 2 * t:2 * t + 1],
                    scalar2=rstd_all[:, t:t + 1], op0=OP.subtract, op1=OP.mult)
                if apply_affine:
                    nc.vector.tensor_mul(out=och, in0=och, in1=lng_sb)
                    nc.vector.tensor_add(out=och, in0=och, in1=lnb_sb)
                nc.sync.dma_start(
                    out=outd.ap()[g * L + 128 * t: g * L + 128 * (t + 1), :],
                    in_=och)

            def phase_Wo(g, tail=False):
                att_all = st[g]["att"]
                mv_all = sp.tile([128, 2 * NT], F32, name=f"mv{g}", tag="mv")
                rstd_all = sp.tile([128, NT], F32, name=f"rstd{g}", tag="rstd")
                vf = sp.tile([128, NT], F32, name=f"vf{g}", tag="vf")
                lnv = sp.tile([128, NT], F32, name=f"lnv{g}", tag="lnv")
                obs = []
                for t in range(NT):
                    o_ps = ppool.tile([128, HID], F32, name=f"ops{g}{t}", tag="pp",
                                      bufs=5)
                    last = NCH - 1
                    for j in range(NCH):
                        nc.tensor.matmul(
                            o_ps, att_all[:, j * L + 128 * t: j * L + 128 * (t + 1)],
                            wot_sb[j], start=(j == 0),
                            stop=(j == last and not apply_bo))
                    if apply_bo:
                        nc.tensor.matmul(o_ps, ones1_sb, bo_sb, start=False,
                                         stop=True)
                    ob = sp.tile([128, HID], BF16, name=f"ob{g}{t}", tag="ob",
                                 bufs=NT + 2)
                    nc.scalar.copy(out=ob, in_=o_ps)
                    stats = sp.tile([128, 6], F32, name=f"sst{g}{t}", tag="sst")
                    nc.vector.bn_stats(out=stats, in_=ob)
                    nc.vector.bn_aggr(out=mv_all[:, 2 * t:2 * t + 2], in_=stats)
                    obs.append(ob)
                    if tail:
                        nc.vector.tensor_scalar_add(
                            out=vf[:, t:t + 1], in0=mv_all[:, 2 * t + 1:2 * t + 2],
                            scalar1=EPS)
                        nc.scalar.activation(out=lnv[:, t:t + 1], in_=vf[:, t:t + 1],
                                             func=AT.Ln)
                        nc.scalar.activation(out=rstd_all[:, t:t + 1],
                                             in_=lnv[:, t:t + 1], func=AT.Exp,
                                             scale=-0.5)
                        ln_apply(g, obs, mv_all, rstd_all, t)
                if not tail:
                    nc.gpsimd.tensor_scalar_add(out=vf, in0=mv_all[:, 1:2 * NT:2],
                                                scalar1=EPS)
                    nc.scalar.activation(out=lnv, in_=vf, func=AT.Ln)
                    nc.scalar.activation(out=rstd_all, in_=lnv, func=AT.Exp, scale=-0.5)
                    for t in range(NT):
                        ln_apply(g, obs, mv_all, rstd_all, t)
                del st[g]

            # -------- modulo schedule (A shifted one slot early) --------
            # per iter g: A(g+1) fills sttA(g); stream/Wo(g-1) fill sttB(g);
            # rbcA(g+1) at iter end once recipA(g+1) is ready.
            phase_load(0)
            phase_load(1)
            load_bulk_consts()
            phase_A(0)
            phase_rbcA(0)
            for g in range(G):
                if g + 2 < G:
                    phase_load(g + 2)
                if g + 1 < G:
                    phase_A(g + 1)
                phase_K(g)
                phase_rbcB(g)
                if g > 0:
                    phase_stream(g - 1)
                    phase_Wo(g - 1)
                phase_prep(g)
                if g + 1 < G:
                    phase_rbcA(g + 1)
            phase_stream(G - 1)
            phase_Wo(G - 1, tail=True)

    _bacc_mod.get_activation_tables = _gat
    try:
        nc.compile()
    finally:
        _bacc_mod.get_activation_tables = _orig_gat
    return nc


_NC_CACHE = {}


def _get_nc(apply_bo, apply_affine):
    key = (apply_bo, apply_affine, USE_FP8)
    if key not in _NC_CACHE:
        _NC_CACHE[key] = _build(apply_bo, apply_affine, USE_FP8)
    return _NC_CACHE[key]


def _host_consts(Wq, Wk, Wv, Wr, w_alpha, w_beta, Wo, bo, ln_g, ln_b):
    import ml_dtypes
    bf = ml_dtypes.bfloat16
    f8 = ml_dtypes.float8_e4m3fn

    wqt = np.ascontiguousarray(Wq.T)                       # [h, e]
    wvr = np.ascontiguousarray(Wv)                         # [d, h]
    wot = np.ascontiguousarray(Wo.T)
    wrt = Wr.T.astype(np.float32)                          # WrT[d, e] = Wr[e, d]
    wrbd = np.zeros((128, 128), np.float32)
    wrbd[:64, :64] = wrt; wrbd[64:, 64:] = wrt
    ident = np.eye(128, dtype=np.float32)
    wa_vec = np.tile(w_alpha, HEADS) * SCALE               # [512]
    wb_vec = np.tile(w_beta, HEADS) * SCALE
    SWV = SW if USE_FP8 else 1.0
    wacol = (wa_vec / SWV).reshape(NCH, 128).T.copy()      # [128, NCH]
    wbcol = (wb_vec / SWV).reshape(NCH, 128).T.copy()

    segs = np.zeros((128, 8 * NCH), np.float32)
    sels = np.zeros((8, HID), np.float32)
    for j in range(NCH):
        for p in range(128):
            segs[p, 8 * j + 2 * j + p // 64] = 1.0
        for m in range(HID):
            if m // 128 == j:
                sels[2 * j + (m % 128) // 64, m] = 1.0
    segs8 = np.zeros((128, 2 * 32), np.float32)
    for p in range(2):
        for d in range(128):
            hA = 4 * p + d // 64          # head of (chunk 2p, partition d)
            hB = 4 * p + 2 + d // 64      # head of (chunk 2p+1, partition d)
            segs8[d, 32 * p + hA] = 1.0
            segs8[d, 32 * p + 16 + hB] = 1.0

    common = {"wqt": wqt.astype(bf), "wvr": wvr.astype(bf),
              "wot": wot.astype(bf), "wrbd": wrbd.astype(bf),
              "ident": ident.astype(bf), "segs": segs.astype(bf),
              "segs8": segs8.astype(f8),
              "sels": sels, "wacol": wacol.astype(np.float32),
              "wbcol": wbcol.astype(np.float32)}

    if USE_FP8:
        def pack_dr(WT):   # WT [h, e] -> [128, NCH*2*256] DoubleRow stationary
            out = np.zeros((128, NCH * 2 * 256), np.float32)
            for j in range(NCH):
                for p in range(2):
                    blkA = WT[256 * p:256 * p + 128, 128 * j:128 * (j + 1)]
                    blkB = WT[256 * p + 128:256 * (p + 1), 128 * j:128 * (j + 1)]
                    c0 = (2 * j + p) * 256
                    out[:, c0:c0 + 128] = blkA
                    out[:, c0 + 128:c0 + 256] = blkB
            return out
        wq_pk = pack_dr(SW * Wq.T)
        wk_pk = pack_dr(SW * Wk.T)
        wq8 = wq_pk.astype(f8)
        wk8 = wk_pk.astype(f8)
        common["wq8"] = wq8
        common["wk8"] = wk8
        common["dwq8"] = (wq_pk - wq8.astype(np.float32)).astype(f8)
        common["dwk8"] = (wk_pk - wk8.astype(np.float32)).astype(f8)
    else:
        common["wkt"] = np.ascontiguousarray(Wk.T).astype(bf)

    apply_bo = not np.allclose(bo, 0.0)
    apply_affine = not (np.allclose(ln_g, 1.0) and np.allclose(ln_b, 0.0))
    if apply_bo:
        common["bo"] = bo.reshape(1, HID).astype(np.float32)
        common["ones1"] = np.ones((1, 128), np.float32)
    if apply_affine:
        common["ln_g"] = np.tile(ln_g, (128, 1)).astype(bf)
        common["ln_b"] = np.tile(ln_b, (128, 1)).astype(bf)
    return common, apply_bo, apply_affine


def kernel(edge_attr, batch_scopes, Wq, Wk, Wv, Wr, w_alpha, w_beta, Wo, bo,
           ln_g, ln_b):
    from concourse import bass_utils
    import ml_dtypes

    edge_attr = np.asarray(edge_attr, dtype=np.float32)
    scopes = np.asarray(batch_scopes)
    Wq = np.asarray(Wq, np.float32); Wk = np.asarray(Wk, np.float32)
    Wv = np.asarray(Wv, np.float32); Wr = np.asarray(Wr, np.float32)
    Wo = np.asarray(Wo, np.float32)
    w_alpha = np.asarray(w_alpha, np.float32); w_beta = np.asarray(w_beta, np.float32)
    bo = np.asarray(bo, np.float32)
    ln_g = np.asarray(ln_g, np.float32); ln_b = np.asarray(ln_b, np.float32)

    assert np.all(scopes[:, 1] == L), "equal-length contiguous scopes expected"
    starts = scopes[:, 0].astype(np.int64)

    common, apply_bo, apply_affine = _host_consts(
        Wq, Wk, Wv, Wr, w_alpha, w_beta, Wo, bo, ln_g, ln_b)
    nc = _get_nc(apply_bo, apply_affine)

    bf = ml_dtypes.bfloat16
    f8 = ml_dtypes.float8_e4m3fn
    in_maps = []
    for c in range(NCORES):
        rows = np.concatenate([
            np.arange(starts[c * G + g], starts[c * G + g] + L)
            for g in range(G)])
        xslab = edge_attr[rows]                       # [G*L, 512]
        xt = np.ascontiguousarray(xslab.T)
        m = {"xtb": xt.astype(bf), **common}
        if USE_FP8:
            x8 = xt.astype(f8)
            m["xt8"] = x8
            m["dxt8"] = (xt - x8.astype(np.float32)).astype(f8)
        in_maps.append(m)

    res = bass_utils.run_bass_kernel_spmd(nc, in_maps, core_ids=list(range(NCORES)))
    out = np.concatenate([r["out"] for r in res.results], axis=0)
    return out.astype(np.float32)


# revision 27
# speedup vs baseline: 1.1381x; 1.0201x over previous
"""BondFastAttention Trainium2 kernel (self-contained), v2.

Shapes (hardcoded from the problem spec):
  edge_attr [65536, 512] fp32, B=64 graphs x L=1024 bonds, HID=512, 8 heads x D=64.
  8 NeuronCores, data-parallel over graphs: G=8 graphs per core.

Device layout: transposed domain - features on partitions, tokens on free dim
for Q/K/kvout; tokens on partitions for the Wo/LayerNorm stage.

Key structure vs v1:
  - The V projection, Wr matmul, gk scaling and +q add are all folded into a
    single per-graph combined weight W''' = Wv^T (gk . Wr^T) + Wq^T, built on
    the PE (16 small matmuls + identity-add), so one X-stream produces
    relu-input directly.
  - Projection PSUM is consumed in place (ACT exp, Pool multiply) - no
    psum->sbuf copies for q/k/v.
  - Optional fp8 path: Q/K projections and softmax seg-sums run as fp8
    DoubleRow matmuls (K=256 per pass).
  - Output is written bf16 and upcast to f32 on the host.
"""
import numpy as np

HID = 512
HEADS = 8
D = 64
B = 64
L = 1024
SCALE = D ** -0.5
EPS = 1e-5
NCORES = 8
G = B // NCORES          # graphs per core
NCH = HID // 128         # 4 feature chunks (2 heads each)
NT = L // 128            # 8 token chunks
SW = 16.0                # fp8 weight prescale

USE_FP8 = True


def _build(apply_bo: bool, apply_affine: bool, use_fp8: bool):
    import concourse.bass as bass
    from concourse import bacc
    import concourse.mybir as mybir
    from concourse.tile import TileContext

    F32 = mybir.dt.float32
    F32R = mybir.dt.float32r
    BF16 = mybir.dt.bfloat16
    FP8 = mybir.dt.float8e4
    AT = mybir.ActivationFunctionType
    OP = mybir.AluOpType
    PM = mybir.MatmulPerfMode

    nc = bacc.Bacc()

    import concourse.bacc as _bacc_mod
    _orig_gat = _bacc_mod.get_activation_tables

    def _gat(arch):
        # Keep dict order but strip our funcs from every other set, so the
        # table-load pass assigns all of them to natural_log_exp_and_others
        # -> a single physical table load.
        t = _orig_gat(arch)
        ours = {AT.Exp, AT.Ln, AT.Copy, AT.Relu, AT.Identity}
        out = {}
        for k, funcs in t.items():
            if k == "natural_log_exp_and_others":
                out[k] = funcs
            else:
                out[k] = {f for f in funcs if f not in ours}
        return out

    # ---------------- dram tensors ----------------
    xtb = nc.dram_tensor("xtb", [HID, G * L], BF16, kind="ExternalInput")
    segs8d = nc.dram_tensor("segs8", [128, 2 * 32], FP8, kind="ExternalInput")
    if use_fp8:
        xt8 = nc.dram_tensor("xt8", [HID, G * L], FP8, kind="ExternalInput")
        dxt8 = nc.dram_tensor("dxt8", [HID, G * L], FP8, kind="ExternalInput")
        wq8d = nc.dram_tensor("wq8", [128, NCH * 2 * 256], FP8, kind="ExternalInput")
        wk8d = nc.dram_tensor("wk8", [128, NCH * 2 * 256], FP8, kind="ExternalInput")
        dwq8d = nc.dram_tensor("dwq8", [128, NCH * 2 * 256], FP8, kind="ExternalInput")
        dwk8d = nc.dram_tensor("dwk8", [128, NCH * 2 * 256], FP8, kind="ExternalInput")
    else:
        wktd = nc.dram_tensor("wkt", [HID, HID], BF16, kind="ExternalInput")
    wqtd = nc.dram_tensor("wqt", [HID, HID], BF16, kind="ExternalInput")
    wvrd = nc.dram_tensor("wvr", [HID, HID], BF16, kind="ExternalInput")
    wotd = nc.dram_tensor("wot", [HID, HID], BF16, kind="ExternalInput")
    wrbdd = nc.dram_tensor("wrbd", [128, 128], BF16, kind="ExternalInput")
    identd = nc.dram_tensor("ident", [128, 128], BF16, kind="ExternalInput")
    segsd = nc.dram_tensor("segs", [128, 8 * NCH], BF16, kind="ExternalInput")
    selsd = nc.dram_tensor("sels", [8, HID], F32, kind="ExternalInput")
    wacold = nc.dram_tensor("wacol", [128, NCH], F32, kind="ExternalInput")
    wbcold = nc.dram_tensor("wbcol", [128, NCH], F32, kind="ExternalInput")
    if apply_bo:
        bod = nc.dram_tensor("bo", [1, HID], F32, kind="ExternalInput")
        onesd = nc.dram_tensor("ones1", [1, 128], F32, kind="ExternalInput")
    if apply_affine:
        lngd = nc.dram_tensor("ln_g", [128, HID], BF16, kind="ExternalInput")
        lnbd = nc.dram_tensor("ln_b", [128, HID], BF16, kind="ExternalInput")
    outd = nc.dram_tensor("out", [G * L, HID], BF16, kind="ExternalOutput")

    SWV = SW if use_fp8 else 1.0

    with TileContext(nc) as tc:
        with tc.tile_pool(name="consts", bufs=1) as cp, \
             tc.tile_pool(name="big", bufs=1) as bp, \
             tc.tile_pool(name="small", bufs=2) as sp, \
             tc.tile_pool(name="psum", bufs=1, space="PSUM") as ppool:

            # ---- constants to SBUF ----
            # Phase-0-critical consts first; bulk consts go AFTER the first
            # graph's X loads are queued, spread across SP/ACT/Pool DGEs.
            wacol_sb = cp.tile([128, NCH], F32)
            nc.sync.dma_start(out=wacol_sb, in_=wacold.ap())
            if use_fp8:
                wq8_sb = cp.tile([128, NCH * 2 * 256], FP8)
                nc.sync.dma_start(out=wq8_sb, in_=wq8d.ap())
                dwq8_sb = cp.tile([128, NCH * 2 * 256], FP8)
                nc.gpsimd.dma_start(out=dwq8_sb, in_=dwq8d.ap())
                wk8_sb = cp.tile([128, NCH * 2 * 256], FP8)
                dwk8_sb = cp.tile([128, NCH * 2 * 256], FP8)
            else:
                wkt_sb = [cp.tile([128, HID], BF16, name=f"wkt{i}") for i in range(NCH)]
            wqt_sb = [cp.tile([128, HID], BF16, name=f"wqt{i}") for i in range(NCH)]
            segs_sb = cp.tile([128, 8 * NCH], BF16)
            sels_sb = cp.tile([8, HID], F32R)

            def load_bulk_consts():
                nc.scalar.dma_start(out=segs_sb, in_=segsd.ap())
                nc.sync.dma_start(out=sels_sb, in_=selsd.ap().bitcast(F32R))
                if use_fp8:
                    nc.scalar.dma_start(out=wk8_sb, in_=wk8d.ap())
                    nc.scalar.dma_start(out=dwk8_sb, in_=dwk8d.ap())
                for i in range(NCH):
                    nc.sync.dma_start(out=wqt_sb[i],
                                      in_=wqtd.ap()[128 * i:128 * (i + 1), :])
                if not use_fp8:
                    for i in range(NCH):
                        nc.scalar.dma_start(
                            out=wkt_sb[i], in_=wktd.ap()[128 * i:128 * (i + 1), :])
                for j in range(NCH):
                    nc.gpsimd.dma_start(out=wvr_sb[j],
                                        in_=wvrd.ap()[128 * j:128 * (j + 1), :])
                for j in range(NCH):
                    nc.gpsimd.dma_start(out=wot_sb[j],
                                        in_=wotd.ap()[128 * j:128 * (j + 1), :])
                nc.scalar.dma_start(out=wrbd_sb, in_=wrbdd.ap())
                nc.scalar.dma_start(out=ident_sb, in_=identd.ap())
                nc.scalar.dma_start(out=segs8_sb, in_=segs8d.ap())
                nc.scalar.dma_start(out=wbcol_sb, in_=wbcold.ap())
                if apply_bo:
                    nc.scalar.dma_start(out=ones1_sb, in_=onesd.ap().bitcast(F32R))
                    nc.scalar.dma_start(out=bo_sb, in_=bod.ap().bitcast(F32R))
                if apply_affine:
                    nc.gpsimd.dma_start(out=lng_sb, in_=lngd.ap())
                    nc.gpsimd.dma_start(out=lnb_sb, in_=lnbd.ap())

            wvr_sb = [cp.tile([128, HID], BF16, name=f"wvr{j}") for j in range(NCH)]
            wot_sb = [cp.tile([128, HID], BF16, name=f"wot{j}") for j in range(NCH)]
            wrbd_sb = cp.tile([128, 128], BF16)
            ident_sb = cp.tile([128, 128], BF16)
            segs8_sb = cp.tile([128, 2 * 32], FP8)
            wbcol_sb = cp.tile([128, NCH], F32)
            if apply_bo:
                ones1_sb = cp.tile([1, 128], F32R)
                bo_sb = cp.tile([1, HID], F32R)
            if apply_affine:
                lng_sb = cp.tile([128, HID], BF16)
                lnb_sb = cp.tile([128, HID], BF16)

            EDT = BF16

            # -------- per-graph state (software-pipelined emission) --------
            st = {}

            def phase_load(g):
                s = {}
                s["xtb"] = bp.tile([128, NCH * L], BF16, name=f"xtb{g}", tag="xtb",
                                   bufs=4)
                xtb_src = bass.AP(
                    tensor=xtb.ap().tensor, offset=g * L,
                    ap=[[G * L, 128], [128 * G * L, NCH], [1, L]])
                nc.sync.dma_start(
                    out=s["xtb"].rearrange("p (i l) -> p i l", i=NCH), in_=xtb_src)
                if use_fp8:
                    s["xt8"] = bp.tile([128, NCH * L], FP8, name=f"xt8{g}",
                                       tag="xt8", bufs=3)
                    xt8_src = bass.AP(
                        tensor=xt8.ap().tensor, offset=g * L,
                        ap=[[G * L, 128], [128 * G * L, NCH], [1, L]])
                    nc.scalar.dma_start(
                        out=s["xt8"].rearrange("p (i l) -> p i l", i=NCH),
                        in_=xt8_src)
                    s["dxt8"] = bp.tile([128, NCH * L], FP8, name=f"dxt8{g}",
                                        tag="dxt8", bufs=3)
                    dxt8_src = bass.AP(
                        tensor=dxt8.ap().tensor, offset=g * L,
                        ap=[[G * L, 128], [128 * G * L, NCH], [1, L]])
                    nc.gpsimd.dma_start(
                        out=s["dxt8"].rearrange("p (i l) -> p i l", i=NCH),
                        in_=dxt8_src)
                st[g] = s

            def proj_half(g, w8_sb, w_sb, j, n0, pp):
                if use_fp8:
                    w8, dw8 = w8_sb
                    xt8_3d = st[g]["xt8"].rearrange("p (i l) -> p i l", i=NCH)
                    dxt8_3d = st[g]["dxt8"].rearrange("p (i l) -> p i l", i=NCH)
                    # psum = X8 @ W8 + X8 @ dW8 + dX8 @ W8  (~bf16 accuracy)
                    plan = [(w8, xt8_3d), (dw8, xt8_3d), (w8, dxt8_3d)]
                    nmm = len(plan) * 2
                    k = 0
                    for wsb, xsb in plan:
                        for p in range(2):
                            lhs = wsb[:, (2 * j + p) * 256:(2 * j + p + 1) * 256] \
                                .rearrange("p (two f) -> p two f", two=2)
                            rhs = xsb[:, 2 * p:2 * p + 2, n0:n0 + 512]
                            nc.tensor.matmul(pp, lhs, rhs, start=(k == 0),
                                             stop=(k == nmm - 1),
                                             perf_mode=PM.DoubleRow)
                            k += 1
                else:
                    for i in range(NCH):
                        nc.tensor.matmul(
                            pp, w_sb[i][:, 128 * j:128 * (j + 1)],
                            st[g]["xtb"][:, i * L + n0: i * L + n0 + 512],
                            start=(i == 0), stop=(i == NCH - 1))

            def proj_stage(g, tag, w8_sb, w_sb, scale_ap):
                """projection + exp + m for one of Q/K; then seg-sums + recip."""
                e_all = bp.tile([128, NCH * L], EDT, name=f"e{tag}{g}", tag="e",
                                bufs=3)
                m_tiles = []
                for j in range(NCH):
                    mt = sp.tile([128, L], BF16, name=f"m{tag}{g}{j}", tag="scr",
                                 bufs=8)
                    m_tiles.append(mt)
                for j in range(NCH):
                    for n0 in (0, 512):
                        pp = ppool.tile([128, 512], F32, name=f"pp{tag}{g}{j}{n0}",
                                        tag="pp", bufs=5)
                        proj_half(g, w8_sb, w_sb, j, n0, pp)
                        nc.scalar.activation(
                            out=e_all[:, j * L + n0: j * L + n0 + 512], in_=pp,
                            func=AT.Exp, scale=scale_ap[:, j:j + 1])
                        nc.vector.tensor_mul(
                            out=m_tiles[j][:, n0:n0 + 512],
                            in0=e_all[:, j * L + n0: j * L + n0 + 512], in1=pp)
                sos = []
                for hi, n0 in enumerate((0, 512)):
                    so = ppool.tile([16, 512], F32, name=f"so{tag}{g}{n0}",
                                    tag="rs", bufs=3, padded_shape=[128, 512])
                    for j in range(NCH):
                        nc.tensor.matmul(
                            so[0:8, :], segs_sb[:, 8 * j:8 * (j + 1)],
                            e_all[:, j * L + n0: j * L + n0 + 512],
                            start=(j == 0), stop=(j == NCH - 1))
                    sos.append(so)
                rt = sp.tile([8, 1024], F32, name=f"rt{tag}{g}", tag="rt", bufs=3)
                nc.vector.reciprocal_approx_fast(out=rt[:, 0:512], in_=sos[0][0:8, :])
                nc.vector.reciprocal_approx_fast(out=rt[:, 512:1024], in_=sos[1][0:8, :])
                rtr = sp.tile([8, 1024], F32R, name=f"rtr{tag}{g}", tag="rtr", bufs=3)
                nc.vector.tensor_copy(out=rtr, in_=rt)
                st[g][f"m{tag}"] = m_tiles
                st[g][f"rt{tag}"] = rtr

            def rbc_stage(g, tag):
                """rbc expand + stt accumulate; returns summed [128, NCH] tile."""
                rtr = st[g][f"rt{tag}"]
                m_tiles = st[g][f"m{tag}"]
                parts = sp.tile([128, 2 * NCH], F32, name=f"pts{tag}{g}",
                                tag=f"pts_{tag}")
                for j in range(NCH):
                    for hi, n0 in enumerate((0, 512)):
                        rbc = ppool.tile([128, 512], F32, name=f"rbc{tag}{g}{j}{n0}",
                                         tag="rs", bufs=3)
                        nc.tensor.matmul(rbc, sels_sb[:, 128 * j:128 * (j + 1)],
                                         rtr[:, n0:n0 + 512])
                        nc.vector.scalar_tensor_tensor(
                            out=m_tiles[j][:, n0:n0 + 512],
                            in0=m_tiles[j][:, n0:n0 + 512],
                            scalar=1.0 / SWV, in1=rbc,
                            op0=OP.mult, op1=OP.mult,
                            accum_out=parts[:, hi * NCH + j:hi * NCH + j + 1])
                tot = sp.tile([128, NCH], F32, name=f"tot{tag}{g}", tag=f"tot{tag}")
                nc.gpsimd.tensor_add(out=tot, in0=parts[:, 0:NCH],
                                     in1=parts[:, NCH:2 * NCH])
                return tot

            def phase_A(g):
                proj_stage(g, "a", (wq8_sb, dwq8_sb) if use_fp8 else None,
                           None if use_fp8 else wqt_sb, wacol_sb)

            def phase_rbcA(g):
                gq = rbc_stage(g, "a")
                gqwb = sp.tile([128, NCH], F32, name=f"gqwb{g}", tag="gqwb")
                nc.gpsimd.tensor_mul(out=gqwb, in0=gq, in1=wbcol_sb)
                st[g]["gq"] = gq
                st[g]["gqwb"] = gqwb

            def phase_K(g):
                proj_stage(g, "b", (wk8_sb, dwk8_sb) if use_fp8 else None,
                           None if use_fp8 else wkt_sb, st[g]["gqwb"])

            def phase_rbcB(g):
                acc = rbc_stage(g, "b")
                gk = sp.tile([128, NCH], F32, name=f"gk{g}", tag="gk")
                nc.gpsimd.tensor_mul(out=gk, in0=acc, in1=st[g]["gq"])
                gkwr = sp.tile([128, NCH * 128], BF16, name=f"gkwr{g}", tag="gkwr")
                for j in range(NCH):
                    nc.gpsimd.tensor_scalar_mul(
                        out=gkwr[:, 128 * j:128 * (j + 1)], in0=wrbd_sb,
                        scalar1=gk[:, j:j + 1])
                st[g]["gkwr"] = gkwr

            def phase_prep(g):
                gkwr = st[g]["gkwr"]
                w3_sb = []
                for i in range(NCH):
                    ppw = ppool.tile([128, 512], F32, name=f"ppw{g}{i}", tag="pp",
                                     bufs=5)
                    for j in range(NCH):
                        nc.tensor.matmul(
                            ppw[:, 128 * j:128 * (j + 1)],
                            wvr_sb[j][:, 128 * i:128 * (i + 1)],
                            gkwr[:, 128 * j:128 * (j + 1)],
                            start=True, stop=False)
                        nc.tensor.matmul(
                            ppw[:, 128 * j:128 * (j + 1)], ident_sb,
                            wqt_sb[i][:, 128 * j:128 * (j + 1)],
                            start=False, stop=True)
                    w3 = sp.tile([128, 512], BF16, name=f"w3{g}{i}", tag="w3",
                                 bufs=8)
                    nc.scalar.copy(out=w3, in_=ppw)
                    w3_sb.append(w3)
                st[g]["w3"] = w3_sb

            def phase_stream(g):
                w3_sb = st[g]["w3"]
                xtb_all = st[g]["xtb"]
                att_all = bp.tile([128, NCH * L], BF16, name=f"att{g}", tag="att",
                                  bufs=2)
                for j in range(NCH):
                    for n0 in (0, 512):
                        ppv = ppool.tile([128, 512], F32, name=f"ppv{g}{j}{n0}",
                                         tag="pp", bufs=5)
                        for i in range(NCH):
                            nc.tensor.matmul(
                                ppv, w3_sb[i][:, 128 * j:128 * (j + 1)],
                                xtb_all[:, i * L + n0: i * L + n0 + 512],
                                start=(i == 0), stop=(i == NCH - 1))
                        nc.scalar.activation(
                            out=att_all[:, j * L + n0: j * L + n0 + 512], in_=ppv,
                            func=AT.Relu)
                st[g]["att"] = att_all

            def ln_apply(g, obs, mv_all, rstd_all, t):
                och = sp.tile([128, HID], BF16, name=f"och{g}{t}", tag="och",
                              bufs=4)
                nc.gpsimd.tensor_scalar(
                    out=och, in0=obs[t], scalar1=mv_all[:, 2 * t:2 * t + 1],
                    scalar2=rstd_all[:, t:t + 1], op0=OP.subtract, op1=OP.mult)
                if apply_affine:
                    nc.vector.tensor_mul(out=och, in0=och, in1=lng_sb)
                    nc.vector.tensor_add(out=och, in0=och, in1=lnb_sb)
                nc.sync.dma_start(
                    out=outd.ap()[g * L + 128 * t: g * L + 128 * (t + 1), :],
                    in_=och)

            def phase_Wo(g, tail=False):
                att_all = st[g]["att"]
                mv_all = sp.tile([128, 2 * NT], F32, name=f"mv{g}", tag="mv")
                rstd_all = sp.tile([128, NT], F32, name=f"rstd{g}", tag="rstd")
                vf = sp.tile([128, NT], F32, name=f"vf{g}", tag="vf")
                lnv = sp.tile([128, NT], F32, name=f"lnv{g}", tag="lnv")
                obs = []
                for t in range(NT):
                    o_ps = ppool.tile([128, HID], F32, name=f"ops{g}{t}", tag="pp",
                                      bufs=5)
                    last = NCH - 1
                    for j in range(NCH):
                        nc.tensor.matmul(
                            o_ps, att_all[:, j * L + 128 * t: j * L + 128 * (t + 1)],
                            wot_sb[j], start=(j == 0),
                            stop=(j == last and not apply_bo))
                    if apply_bo:
                        nc.tensor.matmul(o_ps, ones1_sb, bo_sb, start=False,
                                         stop=True)
                    ob = sp.tile([128, HID], BF16, name=f"ob{g}{t}", tag="ob",
                                 bufs=NT + 2)
                    nc.scalar.copy(out=ob, in_=o_ps)
                    stats = sp.tile([128, 6], F32, name=f"sst{g}{t}", tag="sst")
                    nc.vector.bn_stats(out=stats, in_=ob)
                    nc.vector.bn_aggr(out=mv_all[:, 2 * t:2 * t + 2], in_=stats)
                    obs.append(ob)
                    if tail:
                        nc.vector.tensor_scalar_add(
                            out=vf[:, t:t + 1], in0=mv_all[:, 2 * t + 1:2 * t + 2],
                            scalar1=EPS)
                        nc.scalar.activation(out=lnv[:, t:t + 1], in_=vf[:, t:t + 1],
                                             func=AT.Ln)
                        nc.scalar.activation(out=rstd_all[:, t:t + 1],
                                             in_=lnv[:, t:t + 1], func=AT.Exp,
                                             scale=-0.5)
                        ln_apply(g, obs, mv_all, rstd_all, t)
                if not tail:
                    nc.gpsimd.tensor_scalar_add(out=vf, in0=mv_all[:, 1:2 * NT:2],
                                                scalar1=EPS)
                    nc.scalar.activation(out=lnv, in_=vf, func=AT.Ln)
                    nc.scalar.activation(out=rstd_all, in_=lnv, func=AT.Exp, scale=-0.5)
                    for t in range(NT):
                        ln_apply(g, obs, mv_all, rstd_all, t)
                del st[g]

            # -------- modulo schedule (A shifted one slot early) --------
            # per iter g: A(g+1) fills sttA(g); stream/Wo(g-1) fill sttB(g);
            # rbcA(g+1) at iter end once recipA(g+1) is ready.
            phase_load(0)
            phase_load(1)
            load_bulk_consts()
            phase_A(0)
            phase_rbcA(0)
            for g in range(G):
                if g + 2 < G:
                    phase_load(g + 2)
                if g + 1 < G:
                    phase_A(g + 1)
                phase_K(g)
                phase_rbcB(g)
                if g > 0:
                    phase_stream(g - 1)
                    phase_Wo(g - 1)
                phase_prep(g)
                if g + 1 < G:
                    phase_rbcA(g + 1)
            phase_stream(G - 1)
            phase_Wo(G - 1, tail=True)

    _bacc_mod.get_activation_tables = _gat
    try:
        nc.compile()
    finally:
        _bacc_mod.get_activation_tables = _orig_gat
    return nc


_NC_CACHE = {}


def _get_nc(apply_bo, apply_affine):
    key = (apply_bo, apply_affine, USE_FP8)
    if key not in _NC_CACHE:
        _NC_CACHE[key] = _build(apply_bo, apply_affine, USE_FP8)
    return _NC_CACHE[key]


def _host_consts(Wq, Wk, Wv, Wr, w_alpha, w_beta, Wo, bo, ln_g, ln_b):
    import ml_dtypes
    bf = ml_dtypes.bfloat16
    f8 = ml_dtypes.float8_e4m3fn

    wqt = np.ascontiguousarray(Wq.T)                       # [h, e]
    wvr = np.ascontiguousarray(Wv)                         # [d, h]
    wot = np.ascontiguousarray(Wo.T)
    wrt = Wr.T.astype(np.float32)                          # WrT[d, e] = Wr[e, d]
    wrbd = np.zeros((128, 128), np.float32)
    wrbd[:64, :64] = wrt; wrbd[64:, 64:] = wrt
    ident = np.eye(128, dtype=np.float32)
    wa_vec = np.tile(w_alpha, HEADS) * SCALE               # [512]
    wb_vec = np.tile(w_beta, HEADS) * SCALE
    SWV = SW if USE_FP8 else 1.0
    wacol = (wa_vec / SWV).reshape(NCH, 128).T.copy()      # [128, NCH]
    wbcol = (wb_vec / SWV).reshape(NCH, 128).T.copy()

    segs = np.zeros((128, 8 * NCH), np.float32)
    sels = np.zeros((8, HID), np.float32)
    for j in range(NCH):
        for p in range(128):
            segs[p, 8 * j + 2 * j + p // 64] = 1.0
        for m in range(HID):
            if m // 128 == j:
                sels[2 * j + (m % 128) // 64, m] = 1.0
    segs8 = np.zeros((128, 2 * 32), np.float32)
    for p in range(2):
        for d in range(128):
            hA = 4 * p + d // 64          # head of (chunk 2p, partition d)
            hB = 4 * p + 2 + d // 64      # head of (chunk 2p+1, partition d)
            segs8[d, 32 * p + hA] = 1.0
            segs8[d, 32 * p + 16 + hB] = 1.0

    common = {"wqt": wqt.astype(bf), "wvr": wvr.astype(bf),
              "wot": wot.astype(bf), "wrbd": wrbd.astype(bf),
              "ident": ident.astype(bf), "segs": segs.astype(bf),
              "segs8": segs8.astype(f8),
              "sels": sels, "wacol": wacol.astype(np.float32),
              "wbcol": wbcol.astype(np.float32)}

    if USE_FP8:
        def pack_dr(WT):   # WT [h, e] -> [128, NCH*2*256] DoubleRow stationary
            out = np.zeros((128, NCH * 2 * 256), np.float32)
            for j in range(NCH):
                for p in range(2):
                    blkA = WT[256 * p:256 * p + 128, 128 * j:128 * (j + 1)]
                    blkB = WT[256 * p + 128:256 * (p + 1), 128 * j:128 * (j + 1)]
                    c0 = (2 * j + p) * 256
                    out[:, c0:c0 + 128] = blkA
                    out[:, c0 + 128:c0 + 256] = blkB
            return out
        wq_pk = pack_dr(SW * Wq.T)
        wk_pk = pack_dr(SW * Wk.T)
        wq8 = wq_pk.astype(f8)
        wk8 = wk_pk.astype(f8)
        common["wq8"] = wq8
        common["wk8"] = wk8
        common["dwq8"] = (wq_pk - wq8.astype(np.float32)).astype(f8)
        common["dwk8"] = (wk_pk - wk8.astype(np.float32)).astype(f8)
    else:
        common["wkt"] = np.ascontiguousarray(Wk.T).astype(bf)

    apply_bo = not np.allclose(bo, 0.0)
    apply_affine = not (np.allclose(ln_g, 1.0) and np.allclose(ln_b, 0.0))
    if apply_bo:
        common["bo"] = bo.reshape(1, HID).astype(np.float32)
        common["ones1"] = np.ones((1, 128), np.float32)
    if apply_affine:
        common["ln_g"] = np.tile(ln_g, (128, 1)).astype(bf)
        common["ln_b"] = np.tile(ln_b, (128, 1)).astype(bf)
    return common, apply_bo, apply_affine


def kernel(edge_attr, batch_scopes, Wq, Wk, Wv, Wr, w_alpha, w_beta, Wo, bo,
           ln_g, ln_b):
    from concourse import bass_utils
    import ml_dtypes

    edge_attr = np.asarray(edge_attr, dtype=np.float32)
    scopes = np.asarray(batch_scopes)
    Wq = np.asarray(Wq, np.float32); Wk = np.asarray(Wk, np.float32)
    Wv = np.asarray(Wv, np.float32); Wr = np.asarray(Wr, np.float32)
    Wo = np.asarray(Wo, np.float32)
    w_alpha = np.asarray(w_alpha, np.float32); w_beta = np.asarray(w_beta, np.float32)
    bo = np.asarray(bo, np.float32)
    ln_g = np.asarray(ln_g, np.float32); ln_b = np.asarray(ln_b, np.float32)

    assert np.all(scopes[:, 1] == L), "equal-length contiguous scopes expected"
    starts = scopes[:, 0].astype(np.int64)

    common, apply_bo, apply_affine = _host_consts(
        Wq, Wk, Wv, Wr, w_alpha, w_beta, Wo, bo, ln_g, ln_b)
    nc = _get_nc(apply_bo, apply_affine)

    bf = ml_dtypes.bfloat16
    f8 = ml_dtypes.float8_e4m3fn
    in_maps = []
    for c in range(NCORES):
        rows = np.concatenate([
            np.arange(starts[c * G + g], starts[c * G + g] + L)
            for g in range(G)])
        xslab = edge_attr[rows]                       # [G*L, 512]
        xt = np.ascontiguousarray(xslab.T)
        m = {"xtb": xt.astype(bf), **common}
        if USE_FP8:
            x8 = xt.astype(f8)
            m["xt8"] = x8
            m["dxt8"] = (xt - x8.astype(np.float32)).astype(f8)
        in_maps.append(m)

    res = bass_utils.run_bass_kernel_spmd(nc, in_maps, core_ids=list(range(NCORES)))
    out = np.concatenate([r["out"] for r in res.results], axis=0)
    return out.astype(np.float32)


# revision 33
# speedup vs baseline: 1.1434x; 1.0047x over previous
"""BondFastAttention Trainium2 kernel (self-contained), v2.

Shapes (hardcoded from the problem spec):
  edge_attr [65536, 512] fp32, B=64 graphs x L=1024 bonds, HID=512, 8 heads x D=64.
  8 NeuronCores, data-parallel over graphs: G=8 graphs per core.

Device layout: transposed domain - features on partitions, tokens on free dim
for Q/K/kvout; tokens on partitions for the Wo/LayerNorm stage.

Key structure vs v1:
  - The V projection, Wr matmul, gk scaling and +q add are all folded into a
    single per-graph combined weight W''' = Wv^T (gk . Wr^T) + Wq^T, built on
    the PE (16 small matmuls + identity-add), so one X-stream produces
    relu-input directly.
  - Projection PSUM is consumed in place (ACT exp, Pool multiply) - no
    psum->sbuf copies for q/k/v.
  - Optional fp8 path: Q/K projections and softmax seg-sums run as fp8
    DoubleRow matmuls (K=256 per pass).
  - Output is written bf16 and upcast to f32 on the host.
"""
import numpy as np

HID = 512
HEADS = 8
D = 64
B = 64
L = 1024
SCALE = D ** -0.5
EPS = 1e-5
NCORES = 8
G = B // NCORES          # graphs per core
NCH = HID // 128         # 4 feature chunks (2 heads each)
NT = L // 128            # 8 token chunks
SW = 16.0                # fp8 weight prescale

USE_FP8 = True


def _build(apply_bo: bool, apply_affine: bool, use_fp8: bool):
    import concourse.bass as bass
    from concourse import bacc
    import concourse.mybir as mybir
    from concourse.tile import TileContext

    F32 = mybir.dt.float32
    F32R = mybir.dt.float32r
    BF16 = mybir.dt.bfloat16
    FP8 = mybir.dt.float8e4
    AT = mybir.ActivationFunctionType
    OP = mybir.AluOpType
    PM = mybir.MatmulPerfMode

    nc = bacc.Bacc()

    import concourse.bacc as _bacc_mod
    _orig_gat = _bacc_mod.get_activation_tables

    def _gat(arch):
        # Keep dict order but strip our funcs from every other set, so the
        # table-load pass assigns all of them to natural_log_exp_and_others
        # -> a single physical table load.
        t = _orig_gat(arch)
        ours = {AT.Exp, AT.Ln, AT.Copy, AT.Relu, AT.Identity}
        out = {}
        for k, funcs in t.items():
            if k == "natural_log_exp_and_others":
                out[k] = funcs
            else:
                out[k] = {f for f in funcs if f not in ours}
        return out

    # ---------------- dram tensors ----------------
    xtb = nc.dram_tensor("xtb", [HID, G * L], BF16, kind="ExternalInput")
    segs8d = nc.dram_tensor("segs8", [128, 2 * 32], FP8, kind="ExternalInput")
    if use_fp8:
        xt8 = nc.dram_tensor("xt8", [HID, G * L], FP8, kind="ExternalInput")
        dxt8 = nc.dram_tensor("dxt8", [HID, G * L], FP8, kind="ExternalInput")
        wq8d = nc.dram_tensor("wq8", [128, NCH * 2 * 256], FP8, kind="ExternalInput")
        wk8d = nc.dram_tensor("wk8", [128, NCH * 2 * 256], FP8, kind="ExternalInput")
        dwq8d = nc.dram_tensor("dwq8", [128, NCH * 2 * 256], FP8, kind="ExternalInput")
        dwk8d = nc.dram_tensor("dwk8", [128, NCH * 2 * 256], FP8, kind="ExternalInput")
    else:
        wktd = nc.dram_tensor("wkt", [HID, HID], BF16, kind="ExternalInput")
    wqtd = nc.dram_tensor("wqt", [HID, HID], BF16, kind="ExternalInput")
    wvrd = nc.dram_tensor("wvr", [HID, HID], BF16, kind="ExternalInput")
    wotd = nc.dram_tensor("wot", [HID, HID], BF16, kind="ExternalInput")
    wrbdd = nc.dram_tensor("wrbd", [128, 128], BF16, kind="ExternalInput")
    identd = nc.dram_tensor("ident", [128, 128], BF16, kind="ExternalInput")
    segsd = nc.dram_tensor("segs", [128, 8 * NCH], BF16, kind="ExternalInput")
    selsd = nc.dram_tensor("sels", [8, HID], F32, kind="ExternalInput")
    wacold = nc.dram_tensor("wacol", [128, NCH], F32, kind="ExternalInput")
    wbcold = nc.dram_tensor("wbcol", [128, NCH], F32, kind="ExternalInput")
    if apply_bo:
        bod = nc.dram_tensor("bo", [1, HID], F32, kind="ExternalInput")
        onesd = nc.dram_tensor("ones1", [1, 128], F32, kind="ExternalInput")
    if apply_affine:
        lngd = nc.dram_tensor("ln_g", [128, HID], BF16, kind="ExternalInput")
        lnbd = nc.dram_tensor("ln_b", [128, HID], BF16, kind="ExternalInput")
    outd = nc.dram_tensor("out", [G * L, HID], BF16, kind="ExternalOutput")

    SWV = SW if use_fp8 else 1.0

    with TileContext(nc) as tc:
        with tc.tile_pool(name="consts", bufs=1) as cp, \
             tc.tile_pool(name="big", bufs=1) as bp, \
             tc.tile_pool(name="small", bufs=2) as sp, \
             tc.tile_pool(name="psum", bufs=1, space="PSUM") as ppool:

            # ---- constants to SBUF ----
            # Phase-0-critical consts first; bulk consts go AFTER the first
            # graph's X loads are queued, spread across SP/ACT/Pool DGEs.
            wacol_sb = cp.tile([128, NCH], F32)
            nc.sync.dma_start(out=wacol_sb, in_=wacold.ap())
            if use_fp8:
                wq8_sb = cp.tile([128, NCH * 2 * 256], FP8)
                nc.sync.dma_start(out=wq8_sb, in_=wq8d.ap())
                dwq8_sb = cp.tile([128, NCH * 2 * 256], FP8)
                nc.gpsimd.dma_start(out=dwq8_sb, in_=dwq8d.ap())
                wk8_sb = cp.tile([128, NCH * 2 * 256], FP8)
                dwk8_sb = cp.tile([128, NCH * 2 * 256], FP8)
            else:
                wkt_sb = [cp.tile([128, HID], BF16, name=f"wkt{i}") for i in range(NCH)]
            wqt_sb = [cp.tile([128, HID], BF16, name=f"wqt{i}") for i in range(NCH)]
            segs_sb = cp.tile([128, 8 * NCH], BF16)
            sels_sb = cp.tile([8, HID], F32R)

            def load_bulk_consts():
                nc.scalar.dma_start(out=segs_sb, in_=segsd.ap())
                nc.sync.dma_start(out=sels_sb, in_=selsd.ap().bitcast(F32R))
                if use_fp8:
                    nc.scalar.dma_start(out=wk8_sb, in_=wk8d.ap())
                    nc.scalar.dma_start(out=dwk8_sb, in_=dwk8d.ap())
                for i in range(NCH):
                    nc.sync.dma_start(out=wqt_sb[i],
                                      in_=wqtd.ap()[128 * i:128 * (i + 1), :])
                if not use_fp8:
                    for i in range(NCH):
                        nc.scalar.dma_start(
                            out=wkt_sb[i], in_=wktd.ap()[128 * i:128 * (i + 1), :])
                for j in range(NCH):
                    nc.gpsimd.dma_start(out=wvr_sb[j],
                                        in_=wvrd.ap()[128 * j:128 * (j + 1), :])
                for j in range(NCH):
                    nc.gpsimd.dma_start(out=wot_sb[j],
                                        in_=wotd.ap()[128 * j:128 * (j + 1), :])
                nc.scalar.dma_start(out=wrbd_sb, in_=wrbdd.ap())
                nc.scalar.dma_start(out=ident_sb, in_=identd.ap())
                nc.scalar.dma_start(out=segs8_sb, in_=segs8d.ap())
                nc.scalar.dma_start(out=wbcol_sb, in_=wbcold.ap())
                if apply_bo:
                    nc.scalar.dma_start(out=ones1_sb, in_=onesd.ap().bitcast(F32R))
                    nc.scalar.dma_start(out=bo_sb, in_=bod.ap().bitcast(F32R))
                if apply_affine:
                    nc.gpsimd.dma_start(out=lng_sb, in_=lngd.ap())
                    nc.gpsimd.dma_start(out=lnb_sb, in_=lnbd.ap())

            wvr_sb = [cp.tile([128, HID], BF16, name=f"wvr{j}") for j in range(NCH)]
            wot_sb = [cp.tile([128, HID], BF16, name=f"wot{j}") for j in range(NCH)]
            wrbd_sb = cp.tile([128, 128], BF16)
            ident_sb = cp.tile([128, 128], BF16)
            segs8_sb = cp.tile([128, 2 * 32], FP8)
            wbcol_sb = cp.tile([128, NCH], F32)
            if apply_bo:
                ones1_sb = cp.tile([1, 128], F32R)
                bo_sb = cp.tile([1, HID], F32R)
            if apply_affine:
                lng_sb = cp.tile([128, HID], BF16)
                lnb_sb = cp.tile([128, HID], BF16)

            EDT = BF16

            # -------- per-graph state (software-pipelined emission) --------
            st = {}

            def phase_load(g):
                s = {}
                s["xtb"] = bp.tile([128, NCH * L], BF16, name=f"xtb{g}", tag="xtb",
                                   bufs=4)
                xtb_src = bass.AP(
                    tensor=xtb.ap().tensor, offset=g * L,
                    ap=[[G * L, 128], [128 * G * L, NCH], [1, L]])
                nc.sync.dma_start(
                    out=s["xtb"].rearrange("p (i l) -> p i l", i=NCH), in_=xtb_src)
                if use_fp8:
                    s["xt8"] = bp.tile([128, NCH * L], FP8, name=f"xt8{g}",
                                       tag="xt8", bufs=3)
                    xt8_src = bass.AP(
                        tensor=xt8.ap().tensor, offset=g * L,
                        ap=[[G * L, 128], [128 * G * L, NCH], [1, L]])
                    nc.scalar.dma_start(
                        out=s["xt8"].rearrange("p (i l) -> p i l", i=NCH),
                        in_=xt8_src)
                    s["dxt8"] = bp.tile([128, NCH * L], FP8, name=f"dxt8{g}",
                                        tag="dxt8", bufs=3)
                    dxt8_src = bass.AP(
                        tensor=dxt8.ap().tensor, offset=g * L,
                        ap=[[G * L, 128], [128 * G * L, NCH], [1, L]])
                    nc.gpsimd.dma_start(
                        out=s["dxt8"].rearrange("p (i l) -> p i l", i=NCH),
                        in_=dxt8_src)
                st[g] = s

            def proj_half(g, w8_sb, w_sb, j, n0, pp):
                if use_fp8:
                    w8, dw8 = w8_sb
                    xt8_3d = st[g]["xt8"].rearrange("p (i l) -> p i l", i=NCH)
                    dxt8_3d = st[g]["dxt8"].rearrange("p (i l) -> p i l", i=NCH)
                    # psum = X8 @ W8 + X8 @ dW8 + dX8 @ W8  (~bf16 accuracy)
                    plan = [(w8, xt8_3d), (dw8, xt8_3d), (w8, dxt8_3d)]
                    nmm = len(plan) * 2
                    k = 0
                    for wsb, xsb in plan:
                        for p in range(2):
                            lhs = wsb[:, (2 * j + p) * 256:(2 * j + p + 1) * 256] \
                                .rearrange("p (two f) -> p two f", two=2)
                            rhs = xsb[:, 2 * p:2 * p + 2, n0:n0 + 512]
                            nc.tensor.matmul(pp, lhs, rhs, start=(k == 0),
                                             stop=(k == nmm - 1),
                                             perf_mode=PM.DoubleRow)
                            k += 1
                else:
                    for i in range(NCH):
                        nc.tensor.matmul(
                            pp, w_sb[i][:, 128 * j:128 * (j + 1)],
                            st[g]["xtb"][:, i * L + n0: i * L + n0 + 512],
                            start=(i == 0), stop=(i == NCH - 1))

            def proj_stage(g, tag, w8_sb, w_sb, scale_ap):
                """projection + exp + m for one of Q/K; then seg-sums + recip."""
                e_all = bp.tile([128, NCH * L], EDT, name=f"e{tag}{g}", tag="e",
                                bufs=3)
                m_tiles = []
                for j in range(NCH):
                    mt = sp.tile([128, L], BF16, name=f"m{tag}{g}{j}", tag="scr",
                                 bufs=8)
                    m_tiles.append(mt)
                for j in range(NCH):
                    for n0 in (0, 512):
                        pp = ppool.tile([128, 512], F32, name=f"pp{tag}{g}{j}{n0}",
                                        tag="pp", bufs=5)
                        proj_half(g, w8_sb, w_sb, j, n0, pp)
                        nc.scalar.activation(
                            out=e_all[:, j * L + n0: j * L + n0 + 512], in_=pp,
                            func=AT.Exp, scale=scale_ap[:, j:j + 1])
                        nc.vector.tensor_mul(
                            out=m_tiles[j][:, n0:n0 + 512],
                            in0=e_all[:, j * L + n0: j * L + n0 + 512], in1=pp)
                sos = []
                for hi, n0 in enumerate((0, 512)):
                    so = ppool.tile([16, 512], F32, name=f"so{tag}{g}{n0}",
                                    tag="rs", bufs=3, padded_shape=[128, 512])
                    for j in range(NCH):
                        nc.tensor.matmul(
                            so[0:8, :], segs_sb[:, 8 * j:8 * (j + 1)],
                            e_all[:, j * L + n0: j * L + n0 + 512],
                            start=(j == 0), stop=(j == NCH - 1))
                    sos.append(so)
                rt = sp.tile([8, 1024], F32, name=f"rt{tag}{g}", tag="rt", bufs=3)
                nc.vector.reciprocal_approx_fast(out=rt[:, 0:512], in_=sos[0][0:8, :])
                nc.vector.reciprocal_approx_fast(out=rt[:, 512:1024], in_=sos[1][0:8, :])
                rtr = sp.tile([8, 1024], F32R, name=f"rtr{tag}{g}", tag="rtr", bufs=3)
                nc.scalar.copy(out=rtr, in_=rt)
                st[g][f"m{tag}"] = m_tiles
                st[g][f"rt{tag}"] = rtr

            def rbc_stage(g, tag):
                """rbc expand + stt accumulate; returns summed [128, NCH] tile."""
                rtr = st[g][f"rt{tag}"]
                m_tiles = st[g][f"m{tag}"]
                parts = sp.tile([128, 2 * NCH], F32, name=f"pts{tag}{g}",
                                tag=f"pts_{tag}")
                for j in range(NCH):
                    for hi, n0 in enumerate((0, 512)):
                        rbc = ppool.tile([128, 512], F32, name=f"rbc{tag}{g}{j}{n0}",
                                         tag="rs", bufs=3)
                        nc.tensor.matmul(rbc, sels_sb[:, 128 * j:128 * (j + 1)],
                                         rtr[:, n0:n0 + 512])
                        nc.vector.scalar_tensor_tensor(
                            out=m_tiles[j][:, n0:n0 + 512],
                            in0=m_tiles[j][:, n0:n0 + 512],
                            scalar=1.0 / SWV, in1=rbc,
                            op0=OP.mult, op1=OP.mult,
                            accum_out=parts[:, hi * NCH + j:hi * NCH + j + 1])
                tot = sp.tile([128, NCH], F32, name=f"tot{tag}{g}", tag=f"tot{tag}")
                nc.gpsimd.tensor_add(out=tot, in0=parts[:, 0:NCH],
                                     in1=parts[:, NCH:2 * NCH])
                return tot

            def phase_A(g):
                proj_stage(g, "a", (wq8_sb, dwq8_sb) if use_fp8 else None,
                           None if use_fp8 else wqt_sb, wacol_sb)

            def phase_rbcA(g):
                gq = rbc_stage(g, "a")
                gqwb = sp.tile([128, NCH], F32, name=f"gqwb{g}", tag="gqwb")
                nc.gpsimd.tensor_mul(out=gqwb, in0=gq, in1=wbcol_sb)
                st[g]["gq"] = gq
                st[g]["gqwb"] = gqwb

            def phase_K(g):
                proj_stage(g, "b", (wk8_sb, dwk8_sb) if use_fp8 else None,
                           None if use_fp8 else wkt_sb, st[g]["gqwb"])

            def phase_rbcB(g):
                acc = rbc_stage(g, "b")
                gk = sp.tile([128, NCH], F32, name=f"gk{g}", tag="gk")
                nc.gpsimd.tensor_mul(out=gk, in0=acc, in1=st[g]["gq"])
                gkwr = sp.tile([128, NCH * 128], BF16, name=f"gkwr{g}", tag="gkwr")
                for j in range(NCH):
                    nc.gpsimd.tensor_scalar_mul(
                        out=gkwr[:, 128 * j:128 * (j + 1)], in0=wrbd_sb,
                        scalar1=gk[:, j:j + 1])
                st[g]["gkwr"] = gkwr

            def phase_prep(g):
                gkwr = st[g]["gkwr"]
                w3_sb = []
                for i in range(NCH):
                    ppw = ppool.tile([128, 512], F32, name=f"ppw{g}{i}", tag="pp",
                                     bufs=5)
                    for j in range(NCH):
                        nc.tensor.matmul(
                            ppw[:, 128 * j:128 * (j + 1)],
                            wvr_sb[j][:, 128 * i:128 * (i + 1)],
                            gkwr[:, 128 * j:128 * (j + 1)],
                            start=True, stop=False)
                        nc.tensor.matmul(
                            ppw[:, 128 * j:128 * (j + 1)], ident_sb,
                            wqt_sb[i][:, 128 * j:128 * (j + 1)],
                            start=False, stop=True)
                    w3 = sp.tile([128, 512], BF16, name=f"w3{g}{i}", tag="w3",
                                 bufs=8)
                    nc.scalar.copy(out=w3, in_=ppw)
                    w3_sb.append(w3)
                st[g]["w3"] = w3_sb

            def phase_stream(g):
                w3_sb = st[g]["w3"]
                xtb_all = st[g]["xtb"]
                att_all = bp.tile([128, NCH * L], BF16, name=f"att{g}", tag="att",
                                  bufs=2)
                for j in range(NCH):
                    for n0 in (0, 512):
                        ppv = ppool.tile([128, 512], F32, name=f"ppv{g}{j}{n0}",
                                         tag="pp", bufs=5)
                        for i in range(NCH):
                            nc.tensor.matmul(
                                ppv, w3_sb[i][:, 128 * j:128 * (j + 1)],
                                xtb_all[:, i * L + n0: i * L + n0 + 512],
                                start=(i == 0), stop=(i == NCH - 1))
                        nc.scalar.activation(
                            out=att_all[:, j * L + n0: j * L + n0 + 512], in_=ppv,
                            func=AT.Relu)
                st[g]["att"] = att_all

            def ln_apply(g, obs, mv_all, rstd_all, t):
                och = sp.tile([128, HID], BF16, name=f"och{g}{t}", tag="och",
                              bufs=4)
                nc.gpsimd.tensor_scalar(
                    out=och, in0=obs[t], scalar1=mv_all[:, 2 * t:2 * t + 1],
                    scalar2=rstd_all[:, t:t + 1], op0=OP.subtract, op1=OP.mult)
                if apply_affine:
                    nc.vector.tensor_mul(out=och, in0=och, in1=lng_sb)
                    nc.vector.tensor_add(out=och, in0=och, in1=lnb_sb)
                nc.sync.dma_start(
                    out=outd.ap()[g * L + 128 * t: g * L + 128 * (t + 1), :],
                    in_=och)

            def phase_Wo(g, tail=False):
                att_all = st[g]["att"]
                mv_all = sp.tile([128, 2 * NT], F32, name=f"mv{g}", tag="mv")
                rstd_all = sp.tile([128, NT], F32, name=f"rstd{g}", tag="rstd")
                vf = sp.tile([128, NT], F32, name=f"vf{g}", tag="vf")
                lnv = sp.tile([128, NT], F32, name=f"lnv{g}", tag="lnv")
                obs = []
                for t in range(NT):
                    o_ps = ppool.tile([128, HID], F32, name=f"ops{g}{t}", tag="pp",
                                      bufs=5)
                    last = NCH - 1
                    for j in range(NCH):
                        nc.tensor.matmul(
                            o_ps, att_all[:, j * L + 128 * t: j * L + 128 * (t + 1)],
                            wot_sb[j], start=(j == 0),
                            stop=(j == last and not apply_bo))
                    if apply_bo:
                        nc.tensor.matmul(o_ps, ones1_sb, bo_sb, start=False,
                                         stop=True)
                    ob = sp.tile([128, HID], BF16, name=f"ob{g}{t}", tag="ob",
                                 bufs=NT + 2)
                    nc.scalar.copy(out=ob, in_=o_ps)
                    stats = sp.tile([128, 6], F32, name=f"sst{g}{t}", tag="sst")
                    nc.vector.bn_stats(out=stats, in_=ob)
                    nc.vector.bn_aggr(out=mv_all[:, 2 * t:2 * t + 2], in_=stats)
                    obs.append(ob)
                    if tail:
                        nc.vector.tensor_scalar_add(
                            out=vf[:, t:t + 1], in0=mv_all[:, 2 * t + 1:2 * t + 2],
                            scalar1=EPS)
                        nc.scalar.activation(out=lnv[:, t:t + 1], in_=vf[:, t:t + 1],
                                             func=AT.Ln)
                        nc.scalar.activation(out=rstd_all[:, t:t + 1],
                                             in_=lnv[:, t:t + 1], func=AT.Exp,
                                             scale=-0.5)
                        ln_apply(g, obs, mv_all, rstd_all, t)
                if not tail:
                    nc.gpsimd.tensor_scalar_add(out=vf, in0=mv_all[:, 1:2 * NT:2],
                                                scalar1=EPS)
                    nc.scalar.activation(out=lnv, in_=vf, func=AT.Ln)
                    nc.scalar.activation(out=rstd_all, in_=lnv, func=AT.Exp, scale=-0.5)
                    for t in range(NT):
                        ln_apply(g, obs, mv_all, rstd_all, t)
                del st[g]

            # -------- modulo schedule (A shifted one slot early) --------
            # per iter g: A(g+1) fills sttA(g); stream/Wo(g-1) fill sttB(g);
            # rbcA(g+1) at iter end once recipA(g+1) is ready.
            phase_load(0)
            phase_load(1)
            load_bulk_consts()
            phase_A(0)
            phase_rbcA(0)
            for g in range(G):
                if g + 2 < G:
                    phase_load(g + 2)
                if g + 1 < G:
                    phase_A(g + 1)
                phase_K(g)
                phase_rbcB(g)
                if g > 0:
                    phase_stream(g - 1)
                    phase_Wo(g - 1)
                phase_prep(g)
                if g + 1 < G:
                    phase_rbcA(g + 1)
            phase_stream(G - 1)
            phase_Wo(G - 1, tail=True)

    _bacc_mod.get_activation_tables = _gat
    try:
        nc.compile()
    finally:
        _bacc_mod.get_activation_tables = _orig_gat
    return nc


_NC_CACHE = {}


def _get_nc(apply_bo, apply_affine):
    key = (apply_bo, apply_affine, USE_FP8)
    if key not in _NC_CACHE:
        _NC_CACHE[key] = _build(apply_bo, apply_affine, USE_FP8)
    return _NC_CACHE[key]


def _host_consts(Wq, Wk, Wv, Wr, w_alpha, w_beta, Wo, bo, ln_g, ln_b):
    import ml_dtypes
    bf = ml_dtypes.bfloat16
    f8 = ml_dtypes.float8_e4m3fn

    wqt = np.ascontiguousarray(Wq.T)                       # [h, e]
    wvr = np.ascontiguousarray(Wv)                         # [d, h]
    wot = np.ascontiguousarray(Wo.T)
    wrt = Wr.T.astype(np.float32)                          # WrT[d, e] = Wr[e, d]
    wrbd = np.zeros((128, 128), np.float32)
    wrbd[:64, :64] = wrt; wrbd[64:, 64:] = wrt
    ident = np.eye(128, dtype=np.float32)
    wa_vec = np.tile(w_alpha, HEADS) * SCALE               # [512]
    wb_vec = np.tile(w_beta, HEADS) * SCALE
    SWV = SW if USE_FP8 else 1.0
    wacol = (wa_vec / SWV).reshape(NCH, 128).T.copy()      # [128, NCH]
    wbcol = (wb_vec / SWV).reshape(NCH, 128).T.copy()

    segs = np.zeros((128, 8 * NCH), np.float32)
    sels = np.zeros((8, HID), np.float32)
    for j in range(NCH):
        for p in range(128):
            segs[p, 8 * j + 2 * j + p // 64] = 1.0
        for m in range(HID):
            if m // 128 == j:
                sels[2 * j + (m % 128) // 64, m] = 1.0
    segs8 = np.zeros((128, 2 * 32), np.float32)
    for p in range(2):
        for d in range(128):
            hA = 4 * p + d // 64          # head of (chunk 2p, partition d)
            hB = 4 * p + 2 + d // 64      # head of (chunk 2p+1, partition d)
            segs8[d, 32 * p + hA] = 1.0
            segs8[d, 32 * p + 16 + hB] = 1.0

    common = {"wqt": wqt.astype(bf), "wvr": wvr.astype(bf),
              "wot": wot.astype(bf), "wrbd": wrbd.astype(bf),
              "ident": ident.astype(bf), "segs": segs.astype(bf),
              "segs8": segs8.astype(f8),
              "sels": sels, "wacol": wacol.astype(np.float32),
              "wbcol": wbcol.astype(np.float32)}

    if USE_FP8:
        def pack_dr(WT):   # WT [h, e] -> [128, NCH*2*256] DoubleRow stationary
            out = np.zeros((128, NCH * 2 * 256), np.float32)
            for j in range(NCH):
                for p in range(2):
                    blkA = WT[256 * p:256 * p + 128, 128 * j:128 * (j + 1)]
                    blkB = WT[256 * p + 128:256 * (p + 1), 128 * j:128 * (j + 1)]
                    c0 = (2 * j + p) * 256
                    out[:, c0:c0 + 128] = blkA
                    out[:, c0 + 128:c0 + 256] = blkB
            return out
        wq_pk = pack_dr(SW * Wq.T)
        wk_pk = pack_dr(SW * Wk.T)
        wq8 = wq_pk.astype(f8)
        wk8 = wk_pk.astype(f8)
        common["wq8"] = wq8
        common["wk8"] = wk8
        common["dwq8"] = (wq_pk - wq8.astype(np.float32)).astype(f8)
        common["dwk8"] = (wk_pk - wk8.astype(np.float32)).astype(f8)
    else:
        common["wkt"] = np.ascontiguousarray(Wk.T).astype(bf)

    apply_bo = not np.allclose(bo, 0.0)
    apply_affine = not (np.allclose(ln_g, 1.0) and np.allclose(ln_b, 0.0))
    if apply_bo:
        common["bo"] = bo.reshape(1, HID).astype(np.float32)
        common["ones1"] = np.ones((1, 128), np.float32)
    if apply_affine:
        common["ln_g"] = np.tile(ln_g, (128, 1)).astype(bf)
        common["ln_b"] = np.tile(ln_b, (128, 1)).astype(bf)
    return common, apply_bo, apply_affine


def kernel(edge_attr, batch_scopes, Wq, Wk, Wv, Wr, w_alpha, w_beta, Wo, bo,
           ln_g, ln_b):
    from concourse import bass_utils
    import ml_dtypes

    edge_attr = np.asarray(edge_attr, dtype=np.float32)
    scopes = np.asarray(batch_scopes)
    Wq = np.asarray(Wq, np.float32); Wk = np.asarray(Wk, np.float32)
    Wv = np.asarray(Wv, np.float32); Wr = np.asarray(Wr, np.float32)
    Wo = np.asarray(Wo, np.float32)
    w_alpha = np.asarray(w_alpha, np.float32); w_beta = np.asarray(w_beta, np.float32)
    bo = np.asarray(bo, np.float32)
    ln_g = np.asarray(ln_g, np.float32); ln_b = np.asarray(ln_b, np.float32)

    assert np.all(scopes[:, 1] == L), "equal-length contiguous scopes expected"
    starts = scopes[:, 0].astype(np.int64)

    common, apply_bo, apply_affine = _host_consts(
        Wq, Wk, Wv, Wr, w_alpha, w_beta, Wo, bo, ln_g, ln_b)
    nc = _get_nc(apply_bo, apply_affine)

    bf = ml_dtypes.bfloat16
    f8 = ml_dtypes.float8_e4m3fn
    in_maps = []
    for c in range(NCORES):
        rows = np.concatenate([
            np.arange(starts[c * G + g], starts[c * G + g] + L)
            for g in range(G)])
        xslab = edge_attr[rows]                       # [G*L, 512]
        xt = np.ascontiguousarray(xslab.T)
        m = {"xtb": xt.astype(bf), **common}
        if USE_FP8:
            x8 = xt.astype(f8)
            m["xt8"] = x8
            m["dxt8"] = (xt - x8.astype(np.float32)).astype(f8)
        in_maps.append(m)

    res = bass_utils.run_bass_kernel_spmd(nc, in_maps, core_ids=list(range(NCORES)))
    out = np.concatenate([r["out"] for r in res.results], axis=0)
    return out.astype(np.float32)


# revision 34
# speedup vs baseline: 1.1522x; 1.0077x over previous
"""BondFastAttention Trainium2 kernel (self-contained), v2.

Shapes (hardcoded from the problem spec):
  edge_attr [65536, 512] fp32, B=64 graphs x L=1024 bonds, HID=512, 8 heads x D=64.
  8 NeuronCores, data-parallel over graphs: G=8 graphs per core.

Device layout: transposed domain - features on partitions, tokens on free dim
for Q/K/kvout; tokens on partitions for the Wo/LayerNorm stage.

Key structure vs v1:
  - The V projection, Wr matmul, gk scaling and +q add are all folded into a
    single per-graph combined weight W''' = Wv^T (gk . Wr^T) + Wq^T, built on
    the PE (16 small matmuls + identity-add), so one X-stream produces
    relu-input directly.
  - Projection PSUM is consumed in place (ACT exp, Pool multiply) - no
    psum->sbuf copies for q/k/v.
  - Optional fp8 path: Q/K projections and softmax seg-sums run as fp8
    DoubleRow matmuls (K=256 per pass).
  - Output is written bf16 and upcast to f32 on the host.
"""
import numpy as np

HID = 512
HEADS = 8
D = 64
B = 64
L = 1024
SCALE = D ** -0.5
EPS = 1e-5
NCORES = 8
G = B // NCORES          # graphs per core
NCH = HID // 128         # 4 feature chunks (2 heads each)
NT = L // 128            # 8 token chunks
SW = 16.0                # fp8 weight prescale

USE_FP8 = True


def _build(apply_bo: bool, apply_affine: bool, use_fp8: bool):
    import concourse.bass as bass
    from concourse import bacc
    import concourse.mybir as mybir
    from concourse.tile import TileContext

    F32 = mybir.dt.float32
    F32R = mybir.dt.float32r
    BF16 = mybir.dt.bfloat16
    FP8 = mybir.dt.float8e4
    AT = mybir.ActivationFunctionType
    OP = mybir.AluOpType
    PM = mybir.MatmulPerfMode

    nc = bacc.Bacc()

    import concourse.bacc as _bacc_mod
    _orig_gat = _bacc_mod.get_activation_tables

    def _gat(arch):
        # Keep dict order but strip our funcs from every other set, so the
        # table-load pass assigns all of them to natural_log_exp_and_others
        # -> a single physical table load.
        t = _orig_gat(arch)
        ours = {AT.Exp, AT.Ln, AT.Copy, AT.Relu, AT.Identity}
        out = {}
        for k, funcs in t.items():
            if k == "natural_log_exp_and_others":
                out[k] = funcs
            else:
                out[k] = {f for f in funcs if f not in ours}
        return out

    # ---------------- dram tensors ----------------
    xtb = nc.dram_tensor("xtb", [HID, G * L], BF16, kind="ExternalInput")
    segs8d = nc.dram_tensor("segs8", [128, 2 * 32], FP8, kind="ExternalInput")
    if use_fp8:
        xt8 = nc.dram_tensor("xt8", [HID, G * L], FP8, kind="ExternalInput")
        dxt8 = nc.dram_tensor("dxt8", [HID, G * L], FP8, kind="ExternalInput")
        wq8d = nc.dram_tensor("wq8", [128, NCH * 2 * 256], FP8, kind="ExternalInput")
        wk8d = nc.dram_tensor("wk8", [128, NCH * 2 * 256], FP8, kind="ExternalInput")
        dwq8d = nc.dram_tensor("dwq8", [128, NCH * 2 * 256], FP8, kind="ExternalInput")
        dwk8d = nc.dram_tensor("dwk8", [128, NCH * 2 * 256], FP8, kind="ExternalInput")
    else:
        wktd = nc.dram_tensor("wkt", [HID, HID], BF16, kind="ExternalInput")
    wqtd = nc.dram_tensor("wqt", [HID, HID], BF16, kind="ExternalInput")
    wvrd = nc.dram_tensor("wvr", [HID, HID], BF16, kind="ExternalInput")
    wotd = nc.dram_tensor("wot", [HID, HID], BF16, kind="ExternalInput")
    wrbdd = nc.dram_tensor("wrbd", [128, 128], BF16, kind="ExternalInput")
    identd = nc.dram_tensor("ident", [128, 128], BF16, kind="ExternalInput")
    segsd = nc.dram_tensor("segs", [128, 8 * NCH], BF16, kind="ExternalInput")
    selsd = nc.dram_tensor("sels", [8, HID], F32, kind="ExternalInput")
    wacold = nc.dram_tensor("wacol", [128, NCH], F32, kind="ExternalInput")
    wbcold = nc.dram_tensor("wbcol", [128, NCH], F32, kind="ExternalInput")
    if apply_bo:
        bod = nc.dram_tensor("bo", [1, HID], F32, kind="ExternalInput")
        onesd = nc.dram_tensor("ones1", [1, 128], F32, kind="ExternalInput")
    if apply_affine:
        lngd = nc.dram_tensor("ln_g", [128, HID], BF16, kind="ExternalInput")
        lnbd = nc.dram_tensor("ln_b", [128, HID], BF16, kind="ExternalInput")
    outd = nc.dram_tensor("out", [G * L, HID], BF16, kind="ExternalOutput")

    SWV = SW if use_fp8 else 1.0

    with TileContext(nc) as tc:
        with tc.tile_pool(name="consts", bufs=1) as cp, \
             tc.tile_pool(name="big", bufs=1) as bp, \
             tc.tile_pool(name="small", bufs=2) as sp, \
             tc.tile_pool(name="psum", bufs=1, space="PSUM") as ppool:

            # ---- constants to SBUF ----
            # Phase-0-critical consts first; bulk consts go AFTER the first
            # graph's X loads are queued, spread across SP/ACT/Pool DGEs.
            wacol_sb = cp.tile([128, NCH], F32)
            nc.sync.dma_start(out=wacol_sb, in_=wacold.ap())
            if use_fp8:
                wq8_sb = cp.tile([128, NCH * 2 * 256], FP8)
                nc.sync.dma_start(out=wq8_sb, in_=wq8d.ap())
                dwq8_sb = cp.tile([128, NCH * 2 * 256], FP8)
                nc.gpsimd.dma_start(out=dwq8_sb, in_=dwq8d.ap())
                wk8_sb = cp.tile([128, NCH * 2 * 256], FP8)
                dwk8_sb = cp.tile([128, NCH * 2 * 256], FP8)
            else:
                wkt_sb = [cp.tile([128, HID], BF16, name=f"wkt{i}") for i in range(NCH)]
            wqt_sb = [cp.tile([128, HID], BF16, name=f"wqt{i}") for i in range(NCH)]
            segs_sb = cp.tile([128, 8 * NCH], BF16)
            sels_sb = cp.tile([8, HID], F32R)

            def load_bulk_consts():
                nc.scalar.dma_start(out=segs_sb, in_=segsd.ap())
                nc.sync.dma_start(out=sels_sb, in_=selsd.ap().bitcast(F32R))
                if use_fp8:
                    nc.scalar.dma_start(out=wk8_sb, in_=wk8d.ap())
                    nc.scalar.dma_start(out=dwk8_sb, in_=dwk8d.ap())
                for i in range(NCH):
                    nc.sync.dma_start(out=wqt_sb[i],
                                      in_=wqtd.ap()[128 * i:128 * (i + 1), :])
                if not use_fp8:
                    for i in range(NCH):
                        nc.scalar.dma_start(
                            out=wkt_sb[i], in_=wktd.ap()[128 * i:128 * (i + 1), :])
                for j in range(NCH):
                    nc.gpsimd.dma_start(out=wvr_sb[j],
                                        in_=wvrd.ap()[128 * j:128 * (j + 1), :])
                for j in range(NCH):
                    nc.gpsimd.dma_start(out=wot_sb[j],
                                        in_=wotd.ap()[128 * j:128 * (j + 1), :])
                nc.scalar.dma_start(out=wrbd_sb, in_=wrbdd.ap())
                nc.scalar.dma_start(out=ident_sb, in_=identd.ap())
                nc.scalar.dma_start(out=segs8_sb, in_=segs8d.ap())
                nc.scalar.dma_start(out=wbcol_sb, in_=wbcold.ap())
                if apply_bo:
                    nc.scalar.dma_start(out=ones1_sb, in_=onesd.ap().bitcast(F32R))
                    nc.scalar.dma_start(out=bo_sb, in_=bod.ap().bitcast(F32R))
                if apply_affine:
                    nc.gpsimd.dma_start(out=lng_sb, in_=lngd.ap())
                    nc.gpsimd.dma_start(out=lnb_sb, in_=lnbd.ap())

            wvr_sb = [cp.tile([128, HID], BF16, name=f"wvr{j}") for j in range(NCH)]
            wot_sb = [cp.tile([128, HID], BF16, name=f"wot{j}") for j in range(NCH)]
            wrbd_sb = cp.tile([128, 128], BF16)
            ident_sb = cp.tile([128, 128], BF16)
            segs8_sb = cp.tile([128, 2 * 32], FP8)
            wbcol_sb = cp.tile([128, NCH], F32)
            if apply_bo:
                ones1_sb = cp.tile([1, 128], F32R)
                bo_sb = cp.tile([1, HID], F32R)
            if apply_affine:
                lng_sb = cp.tile([128, HID], BF16)
                lnb_sb = cp.tile([128, HID], BF16)

            EDT = BF16

            # -------- per-graph state (software-pipelined emission) --------
            st = {}

            def phase_load(g):
                s = {}
                s["xtb"] = bp.tile([128, NCH * L], BF16, name=f"xtb{g}", tag="xtb",
                                   bufs=4)
                xtb_src = bass.AP(
                    tensor=xtb.ap().tensor, offset=g * L,
                    ap=[[G * L, 128], [128 * G * L, NCH], [1, L]])
                nc.sync.dma_start(
                    out=s["xtb"].rearrange("p (i l) -> p i l", i=NCH), in_=xtb_src)
                if use_fp8:
                    s["xt8"] = bp.tile([128, NCH * L], FP8, name=f"xt8{g}",
                                       tag="xt8", bufs=3)
                    xt8_src = bass.AP(
                        tensor=xt8.ap().tensor, offset=g * L,
                        ap=[[G * L, 128], [128 * G * L, NCH], [1, L]])
                    nc.scalar.dma_start(
                        out=s["xt8"].rearrange("p (i l) -> p i l", i=NCH),
                        in_=xt8_src)
                    s["dxt8"] = bp.tile([128, NCH * L], FP8, name=f"dxt8{g}",
                                        tag="dxt8", bufs=3)
                    dxt8_src = bass.AP(
                        tensor=dxt8.ap().tensor, offset=g * L,
                        ap=[[G * L, 128], [128 * G * L, NCH], [1, L]])
                    nc.gpsimd.dma_start(
                        out=s["dxt8"].rearrange("p (i l) -> p i l", i=NCH),
                        in_=dxt8_src)
                st[g] = s

            def proj_half(g, w8_sb, w_sb, j, n0, pp):
                if use_fp8:
                    w8, dw8 = w8_sb
                    xt8_3d = st[g]["xt8"].rearrange("p (i l) -> p i l", i=NCH)
                    dxt8_3d = st[g]["dxt8"].rearrange("p (i l) -> p i l", i=NCH)
                    # psum = X8 @ W8 + X8 @ dW8 + dX8 @ W8  (~bf16 accuracy)
                    plan = [(w8, xt8_3d), (dw8, xt8_3d), (w8, dxt8_3d)]
                    nmm = len(plan) * 2
                    k = 0
                    for wsb, xsb in plan:
                        for p in range(2):
                            lhs = wsb[:, (2 * j + p) * 256:(2 * j + p + 1) * 256] \
                                .rearrange("p (two f) -> p two f", two=2)
                            rhs = xsb[:, 2 * p:2 * p + 2, n0:n0 + 512]
                            nc.tensor.matmul(pp, lhs, rhs, start=(k == 0),
                                             stop=(k == nmm - 1),
                                             perf_mode=PM.DoubleRow)
                            k += 1
                else:
                    for i in range(NCH):
                        nc.tensor.matmul(
                            pp, w_sb[i][:, 128 * j:128 * (j + 1)],
                            st[g]["xtb"][:, i * L + n0: i * L + n0 + 512],
                            start=(i == 0), stop=(i == NCH - 1))

            def proj_stage(g, tag, w8_sb, w_sb, scale_ap):
                """projection + exp + m for one of Q/K; then seg-sums + recip."""
                e_all = bp.tile([128, NCH * L], EDT, name=f"e{tag}{g}", tag="e",
                                bufs=3)
                m_tiles = []
                for j in range(NCH):
                    mt = sp.tile([128, L], BF16, name=f"m{tag}{g}{j}", tag="scr",
                                 bufs=8)
                    m_tiles.append(mt)
                for j in range(NCH):
                    for n0 in (0, 512):
                        pp = ppool.tile([128, 512], F32, name=f"pp{tag}{g}{j}{n0}",
                                        tag="pp", bufs=6)
                        proj_half(g, w8_sb, w_sb, j, n0, pp)
                        nc.scalar.activation(
                            out=e_all[:, j * L + n0: j * L + n0 + 512], in_=pp,
                            func=AT.Exp, scale=scale_ap[:, j:j + 1])
                        nc.vector.tensor_mul(
                            out=m_tiles[j][:, n0:n0 + 512],
                            in0=e_all[:, j * L + n0: j * L + n0 + 512], in1=pp)
                sos = []
                for hi, n0 in enumerate((0, 512)):
                    so = ppool.tile([16, 512], F32, name=f"so{tag}{g}{n0}",
                                    tag="rs", bufs=2, padded_shape=[128, 512])
                    for j in range(NCH):
                        nc.tensor.matmul(
                            so[0:8, :], segs_sb[:, 8 * j:8 * (j + 1)],
                            e_all[:, j * L + n0: j * L + n0 + 512],
                            start=(j == 0), stop=(j == NCH - 1))
                    sos.append(so)
                rt = sp.tile([8, 1024], F32, name=f"rt{tag}{g}", tag="rt", bufs=3)
                nc.vector.reciprocal_approx_fast(out=rt[:, 0:512], in_=sos[0][0:8, :])
                nc.vector.reciprocal_approx_fast(out=rt[:, 512:1024], in_=sos[1][0:8, :])
                rtr = sp.tile([8, 1024], F32R, name=f"rtr{tag}{g}", tag="rtr", bufs=3)
                nc.scalar.copy(out=rtr, in_=rt)
                st[g][f"m{tag}"] = m_tiles
                st[g][f"rt{tag}"] = rtr

            def rbc_stage(g, tag):
                """rbc expand + stt accumulate; returns summed [128, NCH] tile."""
                rtr = st[g][f"rt{tag}"]
                m_tiles = st[g][f"m{tag}"]
                parts = sp.tile([128, 2 * NCH], F32, name=f"pts{tag}{g}",
                                tag=f"pts_{tag}")
                for j in range(NCH):
                    for hi, n0 in enumerate((0, 512)):
                        rbc = ppool.tile([128, 512], F32, name=f"rbc{tag}{g}{j}{n0}",
                                         tag="rs", bufs=2)
                        nc.tensor.matmul(rbc, sels_sb[:, 128 * j:128 * (j + 1)],
                                         rtr[:, n0:n0 + 512])
                        nc.vector.scalar_tensor_tensor(
                            out=m_tiles[j][:, n0:n0 + 512],
                            in0=m_tiles[j][:, n0:n0 + 512],
                            scalar=1.0 / SWV, in1=rbc,
                            op0=OP.mult, op1=OP.mult,
                            accum_out=parts[:, hi * NCH + j:hi * NCH + j + 1])
                tot = sp.tile([128, NCH], F32, name=f"tot{tag}{g}", tag=f"tot{tag}")
                nc.gpsimd.tensor_add(out=tot, in0=parts[:, 0:NCH],
                                     in1=parts[:, NCH:2 * NCH])
                return tot

            def phase_A(g):
                proj_stage(g, "a", (wq8_sb, dwq8_sb) if use_fp8 else None,
                           None if use_fp8 else wqt_sb, wacol_sb)

            def phase_rbcA(g):
                gq = rbc_stage(g, "a")
                gqwb = sp.tile([128, NCH], F32, name=f"gqwb{g}", tag="gqwb")
                nc.gpsimd.tensor_mul(out=gqwb, in0=gq, in1=wbcol_sb)
                st[g]["gq"] = gq
                st[g]["gqwb"] = gqwb

            def phase_K(g):
                proj_stage(g, "b", (wk8_sb, dwk8_sb) if use_fp8 else None,
                           None if use_fp8 else wkt_sb, st[g]["gqwb"])

            def phase_rbcB(g):
                acc = rbc_stage(g, "b")
                gk = sp.tile([128, NCH], F32, name=f"gk{g}", tag="gk")
                nc.gpsimd.tensor_mul(out=gk, in0=acc, in1=st[g]["gq"])
                gkwr = sp.tile([128, NCH * 128], BF16, name=f"gkwr{g}", tag="gkwr")
                for j in range(NCH):
                    nc.gpsimd.tensor_scalar_mul(
                        out=gkwr[:, 128 * j:128 * (j + 1)], in0=wrbd_sb,
                        scalar1=gk[:, j:j + 1])
                st[g]["gkwr"] = gkwr

            def phase_prep(g):
                gkwr = st[g]["gkwr"]
                w3_sb = []
                for i in range(NCH):
                    ppw = ppool.tile([128, 512], F32, name=f"ppw{g}{i}", tag="pp",
                                     bufs=6)
                    for j in range(NCH):
                        nc.tensor.matmul(
                            ppw[:, 128 * j:128 * (j + 1)],
                            wvr_sb[j][:, 128 * i:128 * (i + 1)],
                            gkwr[:, 128 * j:128 * (j + 1)],
                            start=True, stop=False)
                        nc.tensor.matmul(
                            ppw[:, 128 * j:128 * (j + 1)], ident_sb,
                            wqt_sb[i][:, 128 * j:128 * (j + 1)],
                            start=False, stop=True)
                    w3 = sp.tile([128, 512], BF16, name=f"w3{g}{i}", tag="w3",
                                 bufs=8)
                    nc.scalar.copy(out=w3, in_=ppw)
                    w3_sb.append(w3)
                st[g]["w3"] = w3_sb

            def phase_stream(g):
                w3_sb = st[g]["w3"]
                xtb_all = st[g]["xtb"]
                att_all = bp.tile([128, NCH * L], BF16, name=f"att{g}", tag="att",
                                  bufs=2)
                for j in range(NCH):
                    for n0 in (0, 512):
                        ppv = ppool.tile([128, 512], F32, name=f"ppv{g}{j}{n0}",
                                         tag="pp", bufs=6)
                        for i in range(NCH):
                            nc.tensor.matmul(
                                ppv, w3_sb[i][:, 128 * j:128 * (j + 1)],
                                xtb_all[:, i * L + n0: i * L + n0 + 512],
                                start=(i == 0), stop=(i == NCH - 1))
                        nc.scalar.activation(
                            out=att_all[:, j * L + n0: j * L + n0 + 512], in_=ppv,
                            func=AT.Relu)
                st[g]["att"] = att_all

            def ln_apply(g, obs, mv_all, rstd_all, t):
                och = sp.tile([128, HID], BF16, name=f"och{g}{t}", tag="och",
                              bufs=4)
                nc.gpsimd.tensor_scalar(
                    out=och, in0=obs[t], scalar1=mv_all[:, 2 * t:2 * t + 1],
                    scalar2=rstd_all[:, t:t + 1], op0=OP.subtract, op1=OP.mult)
                if apply_affine:
                    nc.vector.tensor_mul(out=och, in0=och, in1=lng_sb)
                    nc.vector.tensor_add(out=och, in0=och, in1=lnb_sb)
                nc.sync.dma_start(
                    out=outd.ap()[g * L + 128 * t: g * L + 128 * (t + 1), :],
                    in_=och)

            def phase_Wo(g, tail=False):
                att_all = st[g]["att"]
                mv_all = sp.tile([128, 2 * NT], F32, name=f"mv{g}", tag="mv")
                rstd_all = sp.tile([128, NT], F32, name=f"rstd{g}", tag="rstd")
                vf = sp.tile([128, NT], F32, name=f"vf{g}", tag="vf")
                lnv = sp.tile([128, NT], F32, name=f"lnv{g}", tag="lnv")
                obs = []
                for t in range(NT):
                    o_ps = ppool.tile([128, HID], F32, name=f"ops{g}{t}", tag="pp",
                                      bufs=6)
                    last = NCH - 1
                    for j in range(NCH):
                        nc.tensor.matmul(
                            o_ps, att_all[:, j * L + 128 * t: j * L + 128 * (t + 1)],
                            wot_sb[j], start=(j == 0),
                            stop=(j == last and not apply_bo))
                    if apply_bo:
                        nc.tensor.matmul(o_ps, ones1_sb, bo_sb, start=False,
                                         stop=True)
                    ob = sp.tile([128, HID], BF16, name=f"ob{g}{t}", tag="ob",
                                 bufs=NT + 2)
                    nc.scalar.copy(out=ob, in_=o_ps)
                    stats = sp.tile([128, 6], F32, name=f"sst{g}{t}", tag="sst")
                    nc.vector.bn_stats(out=stats, in_=ob)
                    nc.vector.bn_aggr(out=mv_all[:, 2 * t:2 * t + 2], in_=stats)
                    obs.append(ob)
                    if tail:
                        nc.vector.tensor_scalar_add(
                            out=vf[:, t:t + 1], in0=mv_all[:, 2 * t + 1:2 * t + 2],
                            scalar1=EPS)
                        nc.scalar.activation(out=lnv[:, t:t + 1], in_=vf[:, t:t + 1],
                                             func=AT.Ln)
                        nc.scalar.activation(out=rstd_all[:, t:t + 1],
                                             in_=lnv[:, t:t + 1], func=AT.Exp,
                                             scale=-0.5)
                        ln_apply(g, obs, mv_all, rstd_all, t)
                if not tail:
                    nc.gpsimd.tensor_scalar_add(out=vf, in0=mv_all[:, 1:2 * NT:2],
                                                scalar1=EPS)
                    nc.scalar.activation(out=lnv, in_=vf, func=AT.Ln)
                    nc.scalar.activation(out=rstd_all, in_=lnv, func=AT.Exp, scale=-0.5)
                    for t in range(NT):
                        ln_apply(g, obs, mv_all, rstd_all, t)
                del st[g]

            # -------- modulo schedule (A shifted one slot early) --------
            # per iter g: A(g+1) fills sttA(g); stream/Wo(g-1) fill sttB(g);
            # rbcA(g+1) at iter end once recipA(g+1) is ready.
            phase_load(0)
            phase_load(1)
            load_bulk_consts()
            phase_A(0)
            phase_rbcA(0)
            for g in range(G):
                if g + 2 < G:
                    phase_load(g + 2)
                if g + 1 < G:
                    phase_A(g + 1)
                phase_K(g)
                phase_rbcB(g)
                if g > 0:
                    phase_stream(g - 1)
                    phase_Wo(g - 1)
                phase_prep(g)
                if g + 1 < G:
                    phase_rbcA(g + 1)
            phase_stream(G - 1)
            phase_Wo(G - 1, tail=True)

    _bacc_mod.get_activation_tables = _gat
    try:
        nc.compile()
    finally:
        _bacc_mod.get_activation_tables = _orig_gat
    return nc


_NC_CACHE = {}


def _get_nc(apply_bo, apply_affine):
    key = (apply_bo, apply_affine, USE_FP8)
    if key not in _NC_CACHE:
        _NC_CACHE[key] = _build(apply_bo, apply_affine, USE_FP8)
    return _NC_CACHE[key]


def _host_consts(Wq, Wk, Wv, Wr, w_alpha, w_beta, Wo, bo, ln_g, ln_b):
    import ml_dtypes
    bf = ml_dtypes.bfloat16
    f8 = ml_dtypes.float8_e4m3fn

    wqt = np.ascontiguousarray(Wq.T)                       # [h, e]
    wvr = np.ascontiguousarray(Wv)                         # [d, h]
    wot = np.ascontiguousarray(Wo.T)
    wrt = Wr.T.astype(np.float32)                          # WrT[d, e] = Wr[e, d]
    wrbd = np.zeros((128, 128), np.float32)
    wrbd[:64, :64] = wrt; wrbd[64:, 64:] = wrt
    ident = np.eye(128, dtype=np.float32)
    wa_vec = np.tile(w_alpha, HEADS) * SCALE               # [512]
    wb_vec = np.tile(w_beta, HEADS) * SCALE
    SWV = SW if USE_FP8 else 1.0
    wacol = (wa_vec / SWV).reshape(NCH, 128).T.copy()      # [128, NCH]
    wbcol = (wb_vec / SWV).reshape(NCH, 128).T.copy()

    segs = np.zeros((128, 8 * NCH), np.float32)
    sels = np.zeros((8, HID), np.float32)
    for j in range(NCH):
        for p in range(128):
            segs[p, 8 * j + 2 * j + p // 64] = 1.0
        for m in range(HID):
            if m // 128 == j:
                sels[2 * j + (m % 128) // 64, m] = 1.0
    segs8 = np.zeros((128, 2 * 32), np.float32)
    for p in range(2):
        for d in range(128):
            hA = 4 * p + d // 64          # head of (chunk 2p, partition d)
            hB = 4 * p + 2 + d // 64      # head of (chunk 2p+1, partition d)
            segs8[d, 32 * p + hA] = 1.0
            segs8[d, 32 * p + 16 + hB] = 1.0

    common = {"wqt": wqt.astype(bf), "wvr": wvr.astype(bf),
              "wot": wot.astype(bf), "wrbd": wrbd.astype(bf),
              "ident": ident.astype(bf), "segs": segs.astype(bf),
              "segs8": segs8.astype(f8),
              "sels": sels, "wacol": wacol.astype(np.float32),
              "wbcol": wbcol.astype(np.float32)}

    if USE_FP8:
        def pack_dr(WT):   # WT [h, e] -> [128, NCH*2*256] DoubleRow stationary
            out = np.zeros((128, NCH * 2 * 256), np.float32)
            for j in range(NCH):
                for p in range(2):
                    blkA = WT[256 * p:256 * p + 128, 128 * j:128 * (j + 1)]
                    blkB = WT[256 * p + 128:256 * (p + 1), 128 * j:128 * (j + 1)]
                    c0 = (2 * j + p) * 256
                    out[:, c0:c0 + 128] = blkA
                    out[:, c0 + 128:c0 + 256] = blkB
            return out
        wq_pk = pack_dr(SW * Wq.T)
        wk_pk = pack_dr(SW * Wk.T)
        wq8 = wq_pk.astype(f8)
        wk8 = wk_pk.astype(f8)
        common["wq8"] = wq8
        common["wk8"] = wk8
        common["dwq8"] = (wq_pk - wq8.astype(np.float32)).astype(f8)
        common["dwk8"] = (wk_pk - wk8.astype(np.float32)).astype(f8)
    else:
        common["wkt"] = np.ascontiguousarray(Wk.T).astype(bf)

    apply_bo = not np.allclose(bo, 0.0)
    apply_affine = not (np.allclose(ln_g, 1.0) and np.allclose(ln_b, 0.0))
    if apply_bo:
        common["bo"] = bo.reshape(1, HID).astype(np.float32)
        common["ones1"] = np.ones((1, 128), np.float32)
    if apply_affine:
        common["ln_g"] = np.tile(ln_g, (128, 1)).astype(bf)
        common["ln_b"] = np.tile(ln_b, (128, 1)).astype(bf)
    return common, apply_bo, apply_affine


def kernel(edge_attr, batch_scopes, Wq, Wk, Wv, Wr, w_alpha, w_beta, Wo, bo,
           ln_g, ln_b):
    from concourse import bass_utils
    import ml_dtypes

    edge_attr = np.asarray(edge_attr, dtype=np.float32)
    scopes = np.asarray(batch_scopes)
    Wq = np.asarray(Wq, np.float32); Wk = np.asarray(Wk, np.float32)
    Wv = np.asarray(Wv, np.float32); Wr = np.asarray(Wr, np.float32)
    Wo = np.asarray(Wo, np.float32)
    w_alpha = np.asarray(w_alpha, np.float32); w_beta = np.asarray(w_beta, np.float32)
    bo = np.asarray(bo, np.float32)
    ln_g = np.asarray(ln_g, np.float32); ln_b = np.asarray(ln_b, np.float32)

    assert np.all(scopes[:, 1] == L), "equal-length contiguous scopes expected"
    starts = scopes[:, 0].astype(np.int64)

    common, apply_bo, apply_affine = _host_consts(
        Wq, Wk, Wv, Wr, w_alpha, w_beta, Wo, bo, ln_g, ln_b)
    nc = _get_nc(apply_bo, apply_affine)

    bf = ml_dtypes.bfloat16
    f8 = ml_dtypes.float8_e4m3fn
    in_maps = []
    for c in range(NCORES):
        rows = np.concatenate([
            np.arange(starts[c * G + g], starts[c * G + g] + L)
            for g in range(G)])
        xslab = edge_attr[rows]                       # [G*L, 512]
        xt = np.ascontiguousarray(xslab.T)
        m = {"xtb": xt.astype(bf), **common}
        if USE_FP8:
            x8 = xt.astype(f8)
            m["xt8"] = x8
            m["dxt8"] = (xt - x8.astype(np.float32)).astype(f8)
        in_maps.append(m)

    res = bass_utils.run_bass_kernel_spmd(nc, in_maps, core_ids=list(range(NCORES)))
    out = np.concatenate([r["out"] for r in res.results], axis=0)
    return out.astype(np.float32)


# revision 35
# speedup vs baseline: 1.1730x; 1.0180x over previous
"""BondFastAttention Trainium2 kernel (self-contained), v2.

Shapes (hardcoded from the problem spec):
  edge_attr [65536, 512] fp32, B=64 graphs x L=1024 bonds, HID=512, 8 heads x D=64.
  8 NeuronCores, data-parallel over graphs: G=8 graphs per core.

Device layout: transposed domain - features on partitions, tokens on free dim
for Q/K/kvout; tokens on partitions for the Wo/LayerNorm stage.

Key structure vs v1:
  - The V projection, Wr matmul, gk scaling and +q add are all folded into a
    single per-graph combined weight W''' = Wv^T (gk . Wr^T) + Wq^T, built on
    the PE (16 small matmuls + identity-add), so one X-stream produces
    relu-input directly.
  - Projection PSUM is consumed in place (ACT exp, Pool multiply) - no
    psum->sbuf copies for q/k/v.
  - Optional fp8 path: Q/K projections and softmax seg-sums run as fp8
    DoubleRow matmuls (K=256 per pass).
  - Output is written bf16 and upcast to f32 on the host.
"""
import numpy as np

HID = 512
HEADS = 8
D = 64
B = 64
L = 1024
SCALE = D ** -0.5
EPS = 1e-5
NCORES = 8
G = B // NCORES          # graphs per core
NCH = HID // 128         # 4 feature chunks (2 heads each)
NT = L // 128            # 8 token chunks
SW = 16.0                # fp8 weight prescale

USE_FP8 = True


def _build(apply_bo: bool, apply_affine: bool, use_fp8: bool):
    import concourse.bass as bass
    from concourse import bacc
    import concourse.mybir as mybir
    from concourse.tile import TileContext

    F32 = mybir.dt.float32
    F32R = mybir.dt.float32r
    BF16 = mybir.dt.bfloat16
    FP8 = mybir.dt.float8e4
    AT = mybir.ActivationFunctionType
    OP = mybir.AluOpType
    PM = mybir.MatmulPerfMode

    nc = bacc.Bacc()

    import concourse.bacc as _bacc_mod
    _orig_gat = _bacc_mod.get_activation_tables

    def _gat(arch):
        # Keep dict order but strip our funcs from every other set, so the
        # table-load pass assigns all of them to natural_log_exp_and_others
        # -> a single physical table load.
        t = _orig_gat(arch)
        ours = {AT.Exp, AT.Ln, AT.Copy, AT.Relu, AT.Identity}
        out = {}
        for k, funcs in t.items():
            if k == "natural_log_exp_and_others":
                out[k] = funcs
            else:
                out[k] = {f for f in funcs if f not in ours}
        return out

    # ---------------- dram tensors ----------------
    xtb = nc.dram_tensor("xtb", [HID, G * L], BF16, kind="ExternalInput")
    segs8d = nc.dram_tensor("segs8", [128, 2 * 32], FP8, kind="ExternalInput")
    if use_fp8:
        xt8 = nc.dram_tensor("xt8", [HID, G * L], FP8, kind="ExternalInput")
        dxt8 = nc.dram_tensor("dxt8", [HID, G * L], FP8, kind="ExternalInput")
        wq8d = nc.dram_tensor("wq8", [128, NCH * 2 * 256], FP8, kind="ExternalInput")
        wk8d = nc.dram_tensor("wk8", [128, NCH * 2 * 256], FP8, kind="ExternalInput")
        dwq8d = nc.dram_tensor("dwq8", [128, NCH * 2 * 256], FP8, kind="ExternalInput")
        dwk8d = nc.dram_tensor("dwk8", [128, NCH * 2 * 256], FP8, kind="ExternalInput")
    else:
        wktd = nc.dram_tensor("wkt", [HID, HID], BF16, kind="ExternalInput")
    wqtd = nc.dram_tensor("wqt", [HID, HID], BF16, kind="ExternalInput")
    wvrd = nc.dram_tensor("wvr", [HID, HID], BF16, kind="ExternalInput")
    wotd = nc.dram_tensor("wot", [HID, HID], BF16, kind="ExternalInput")
    wrbdd = nc.dram_tensor("wrbd", [128, 128], BF16, kind="ExternalInput")
    identd = nc.dram_tensor("ident", [128, 128], BF16, kind="ExternalInput")
    segsd = nc.dram_tensor("segs", [128, 8 * NCH], BF16, kind="ExternalInput")
    selsd = nc.dram_tensor("sels", [8, HID], F32, kind="ExternalInput")
    wacold = nc.dram_tensor("wacol", [128, NCH], F32, kind="ExternalInput")
    wbcold = nc.dram_tensor("wbcol", [128, NCH], F32, kind="ExternalInput")
    if apply_bo:
        bod = nc.dram_tensor("bo", [1, HID], F32, kind="ExternalInput")
        onesd = nc.dram_tensor("ones1", [1, 128], F32, kind="ExternalInput")
    if apply_affine:
        lngd = nc.dram_tensor("ln_g", [128, HID], BF16, kind="ExternalInput")
        lnbd = nc.dram_tensor("ln_b", [128, HID], BF16, kind="ExternalInput")
    outd = nc.dram_tensor("out", [G * L, HID], BF16, kind="ExternalOutput")

    SWV = SW if use_fp8 else 1.0

    with TileContext(nc) as tc:
        with tc.tile_pool(name="consts", bufs=1) as cp, \
             tc.tile_pool(name="big", bufs=1) as bp, \
             tc.tile_pool(name="small", bufs=2) as sp, \
             tc.tile_pool(name="psum", bufs=1, space="PSUM") as ppool:

            # ---- constants to SBUF ----
            # Phase-0-critical consts first; bulk consts go AFTER the first
            # graph's X loads are queued, spread across SP/ACT/Pool DGEs.
            wacol_sb = cp.tile([128, NCH], F32)
            nc.sync.dma_start(out=wacol_sb, in_=wacold.ap())
            if use_fp8:
                wq8_sb = cp.tile([128, NCH * 2 * 256], FP8)
                nc.sync.dma_start(out=wq8_sb, in_=wq8d.ap())
                dwq8_sb = cp.tile([128, NCH * 2 * 256], FP8)
                nc.gpsimd.dma_start(out=dwq8_sb, in_=dwq8d.ap())
                wk8_sb = cp.tile([128, NCH * 2 * 256], FP8)
                dwk8_sb = cp.tile([128, NCH * 2 * 256], FP8)
            else:
                wkt_sb = [cp.tile([128, HID], BF16, name=f"wkt{i}") for i in range(NCH)]
            wqt_sb = [cp.tile([128, HID], BF16, name=f"wqt{i}") for i in range(NCH)]
            segs_sb = cp.tile([128, 8 * NCH], BF16)
            sels_sb = cp.tile([8, HID], F32R)

            def load_bulk_consts():
                nc.scalar.dma_start(out=segs_sb, in_=segsd.ap())
                nc.sync.dma_start(out=sels_sb, in_=selsd.ap().bitcast(F32R))
                if use_fp8:
                    nc.scalar.dma_start(out=wk8_sb, in_=wk8d.ap())
                    nc.scalar.dma_start(out=dwk8_sb, in_=dwk8d.ap())
                for i in range(NCH):
                    nc.sync.dma_start(out=wqt_sb[i],
                                      in_=wqtd.ap()[128 * i:128 * (i + 1), :])
                if not use_fp8:
                    for i in range(NCH):
                        nc.scalar.dma_start(
                            out=wkt_sb[i], in_=wktd.ap()[128 * i:128 * (i + 1), :])
                for j in range(NCH):
                    nc.gpsimd.dma_start(out=wvr_sb[j],
                                        in_=wvrd.ap()[128 * j:128 * (j + 1), :])
                for j in range(NCH):
                    nc.gpsimd.dma_start(out=wot_sb[j],
                                        in_=wotd.ap()[128 * j:128 * (j + 1), :])
                nc.scalar.dma_start(out=wrbd_sb, in_=wrbdd.ap())
                nc.scalar.dma_start(out=ident_sb, in_=identd.ap())
                nc.scalar.dma_start(out=segs8_sb, in_=segs8d.ap())
                nc.scalar.dma_start(out=wbcol_sb, in_=wbcold.ap())
                if apply_bo:
                    nc.scalar.dma_start(out=ones1_sb, in_=onesd.ap().bitcast(F32R))
                    nc.scalar.dma_start(out=bo_sb, in_=bod.ap().bitcast(F32R))
                if apply_affine:
                    nc.gpsimd.dma_start(out=lng_sb, in_=lngd.ap())
                    nc.gpsimd.dma_start(out=lnb_sb, in_=lnbd.ap())

            wvr_sb = [cp.tile([128, HID], BF16, name=f"wvr{j}") for j in range(NCH)]
            wot_sb = [cp.tile([128, HID], BF16, name=f"wot{j}") for j in range(NCH)]
            wrbd_sb = cp.tile([128, 128], BF16)
            ident_sb = cp.tile([128, 128], BF16)
            segs8_sb = cp.tile([128, 2 * 32], FP8)
            wbcol_sb = cp.tile([128, NCH], F32)
            if apply_bo:
                ones1_sb = cp.tile([1, 128], F32R)
                bo_sb = cp.tile([1, HID], F32R)
            if apply_affine:
                lng_sb = cp.tile([128, HID], BF16)
                lnb_sb = cp.tile([128, HID], BF16)

            EDT = BF16

            # -------- per-graph state (software-pipelined emission) --------
            st = {}

            def phase_load(g):
                s = {}
                s["xtb"] = bp.tile([128, NCH * L], BF16, name=f"xtb{g}", tag="xtb",
                                   bufs=4)
                xtb_src = bass.AP(
                    tensor=xtb.ap().tensor, offset=g * L,
                    ap=[[G * L, 128], [128 * G * L, NCH], [1, L]])
                nc.sync.dma_start(
                    out=s["xtb"].rearrange("p (i l) -> p i l", i=NCH), in_=xtb_src)
                if use_fp8:
                    s["xt8"] = bp.tile([128, NCH * L], FP8, name=f"xt8{g}",
                                       tag="xt8", bufs=3)
                    xt8_src = bass.AP(
                        tensor=xt8.ap().tensor, offset=g * L,
                        ap=[[G * L, 128], [128 * G * L, NCH], [1, L]])
                    nc.scalar.dma_start(
                        out=s["xt8"].rearrange("p (i l) -> p i l", i=NCH),
                        in_=xt8_src)
                    s["dxt8"] = bp.tile([128, NCH * L], FP8, name=f"dxt8{g}",
                                        tag="dxt8", bufs=3)
                    dxt8_src = bass.AP(
                        tensor=dxt8.ap().tensor, offset=g * L,
                        ap=[[G * L, 128], [128 * G * L, NCH], [1, L]])
                    nc.gpsimd.dma_start(
                        out=s["dxt8"].rearrange("p (i l) -> p i l", i=NCH),
                        in_=dxt8_src)
                st[g] = s

            def proj_half(g, w8_sb, w_sb, j, n0, pp):
                if use_fp8:
                    w8, dw8 = w8_sb
                    xt8_3d = st[g]["xt8"].rearrange("p (i l) -> p i l", i=NCH)
                    dxt8_3d = st[g]["dxt8"].rearrange("p (i l) -> p i l", i=NCH)
                    # psum = X8 @ W8 + X8 @ dW8 + dX8 @ W8  (~bf16 accuracy)
                    plan = [(w8, xt8_3d), (dw8, xt8_3d), (w8, dxt8_3d)]
                    nmm = len(plan) * 2
                    k = 0
                    for wsb, xsb in plan:
                        for p in range(2):
                            lhs = wsb[:, (2 * j + p) * 256:(2 * j + p + 1) * 256] \
                                .rearrange("p (two f) -> p two f", two=2)
                            rhs = xsb[:, 2 * p:2 * p + 2, n0:n0 + 512]
                            nc.tensor.matmul(pp, lhs, rhs, start=(k == 0),
                                             stop=(k == nmm - 1),
                                             perf_mode=PM.DoubleRow)
                            k += 1
                else:
                    for i in range(NCH):
                        nc.tensor.matmul(
                            pp, w_sb[i][:, 128 * j:128 * (j + 1)],
                            st[g]["xtb"][:, i * L + n0: i * L + n0 + 512],
                            start=(i == 0), stop=(i == NCH - 1))

            def proj_stage(g, tag, w8_sb, w_sb, scale_ap):
                """projection + exp + m for one of Q/K; then seg-sums + recip."""
                e_all = bp.tile([128, NCH * L], EDT, name=f"e{tag}{g}", tag="e",
                                bufs=3)
                m_tiles = []
                for j in range(NCH):
                    mt = sp.tile([128, L], BF16, name=f"m{tag}{g}{j}", tag="scr",
                                 bufs=8)
                    m_tiles.append(mt)
                for j in range(NCH):
                    for n0 in (0, 512):
                        pp = ppool.tile([128, 512], F32, name=f"pp{tag}{g}{j}{n0}",
                                        tag="pp", bufs=6)
                        proj_half(g, w8_sb, w_sb, j, n0, pp)
                        nc.scalar.activation(
                            out=e_all[:, j * L + n0: j * L + n0 + 512], in_=pp,
                            func=AT.Exp, scale=scale_ap[:, j:j + 1])
                        nc.vector.tensor_mul(
                            out=m_tiles[j][:, n0:n0 + 512],
                            in0=e_all[:, j * L + n0: j * L + n0 + 512], in1=pp)
                sos = []
                for hi, n0 in enumerate((0, 512)):
                    so = ppool.tile([16, 512], F32, name=f"so{tag}{g}{n0}",
                                    tag="rs", bufs=2, padded_shape=[128, 512])
                    for j in range(NCH):
                        nc.tensor.matmul(
                            so[0:8, :], segs_sb[:, 8 * j:8 * (j + 1)],
                            e_all[:, j * L + n0: j * L + n0 + 512],
                            start=(j == 0), stop=(j == NCH - 1))
                    sos.append(so)
                rt = sp.tile([8, 1024], F32, name=f"rt{tag}{g}", tag="rt", bufs=3)
                nc.vector.reciprocal_approx_fast(out=rt[:, 0:512], in_=sos[0][0:8, :])
                nc.vector.reciprocal_approx_fast(out=rt[:, 512:1024], in_=sos[1][0:8, :])
                rtr = sp.tile([8, 1024], F32R, name=f"rtr{tag}{g}", tag="rtr", bufs=3)
                nc.scalar.copy(out=rtr, in_=rt)
                st[g][f"m{tag}"] = m_tiles
                st[g][f"rt{tag}"] = rtr

            def rbc_stage(g, tag):
                """rbc expand + stt accumulate; returns summed [128, NCH] tile."""
                rtr = st[g][f"rt{tag}"]
                m_tiles = st[g][f"m{tag}"]
                parts = sp.tile([128, 2 * NCH], F32, name=f"pts{tag}{g}",
                                tag=f"pts_{tag}")
                for j in range(NCH):
                    for hi, n0 in enumerate((0, 512)):
                        rbc = ppool.tile([128, 512], F32, name=f"rbc{tag}{g}{j}{n0}",
                                         tag="rs", bufs=2)
                        nc.tensor.matmul(rbc, sels_sb[:, 128 * j:128 * (j + 1)],
                                         rtr[:, n0:n0 + 512])
                        nc.vector.scalar_tensor_tensor(
                            out=m_tiles[j][:, n0:n0 + 512],
                            in0=m_tiles[j][:, n0:n0 + 512],
                            scalar=1.0 / SWV, in1=rbc,
                            op0=OP.mult, op1=OP.mult,
                            accum_out=parts[:, hi * NCH + j:hi * NCH + j + 1])
                tot = sp.tile([128, NCH], F32, name=f"tot{tag}{g}", tag=f"tot{tag}")
                nc.gpsimd.tensor_add(out=tot, in0=parts[:, 0:NCH],
                                     in1=parts[:, NCH:2 * NCH])
                return tot

            def phase_A(g):
                proj_stage(g, "a", (wq8_sb, dwq8_sb) if use_fp8 else None,
                           None if use_fp8 else wqt_sb, wacol_sb)

            def phase_rbcA(g):
                gq = rbc_stage(g, "a")
                gqwb = sp.tile([128, NCH], F32, name=f"gqwb{g}", tag="gqwb")
                nc.gpsimd.tensor_mul(out=gqwb, in0=gq, in1=wbcol_sb)
                st[g]["gq"] = gq
                st[g]["gqwb"] = gqwb

            def phase_K(g):
                proj_stage(g, "b", (wk8_sb, dwk8_sb) if use_fp8 else None,
                           None if use_fp8 else wkt_sb, st[g]["gqwb"])

            def phase_rbcB(g):
                acc = rbc_stage(g, "b")
                gk = sp.tile([128, NCH], F32, name=f"gk{g}", tag="gk")
                nc.gpsimd.tensor_mul(out=gk, in0=acc, in1=st[g]["gq"])
                gkwr = sp.tile([128, NCH * 128], BF16, name=f"gkwr{g}", tag="gkwr")
                for j in range(NCH):
                    nc.gpsimd.tensor_scalar_mul(
                        out=gkwr[:, 128 * j:128 * (j + 1)], in0=wrbd_sb,
                        scalar1=gk[:, j:j + 1])
                st[g]["gkwr"] = gkwr

            def phase_prep(g):
                gkwr = st[g]["gkwr"]
                w3_sb = []
                for i in range(NCH):
                    ppw = ppool.tile([128, 512], F32, name=f"ppw{g}{i}", tag="pp",
                                     bufs=6)
                    for j in range(NCH):
                        nc.tensor.matmul(
                            ppw[:, 128 * j:128 * (j + 1)],
                            wvr_sb[j][:, 128 * i:128 * (i + 1)],
                            gkwr[:, 128 * j:128 * (j + 1)],
                            start=True, stop=False)
                        nc.tensor.matmul(
                            ppw[:, 128 * j:128 * (j + 1)], ident_sb,
                            wqt_sb[i][:, 128 * j:128 * (j + 1)],
                            start=False, stop=True)
                    w3 = sp.tile([128, 512], BF16, name=f"w3{g}{i}", tag="w3",
                                 bufs=8)
                    nc.scalar.copy(out=w3, in_=ppw)
                    w3_sb.append(w3)
                st[g]["w3"] = w3_sb

            def phase_stream(g):
                w3_sb = st[g]["w3"]
                xtb_all = st[g]["xtb"]
                att_all = bp.tile([128, NCH * L], BF16, name=f"att{g}", tag="att",
                                  bufs=2)
                for j in range(NCH):
                    for n0 in (0, 512):
                        ppv = ppool.tile([128, 512], F32, name=f"ppv{g}{j}{n0}",
                                         tag="pp", bufs=6)
                        for i in range(NCH):
                            nc.tensor.matmul(
                                ppv, w3_sb[i][:, 128 * j:128 * (j + 1)],
                                xtb_all[:, i * L + n0: i * L + n0 + 512],
                                start=(i == 0), stop=(i == NCH - 1))
                        nc.scalar.activation(
                            out=att_all[:, j * L + n0: j * L + n0 + 512], in_=ppv,
                            func=AT.Relu)
                st[g]["att"] = att_all

            def ln_apply(g, obs, mv_all, rstd_all, t):
                och = sp.tile([128, HID], BF16, name=f"och{g}{t}", tag="och",
                              bufs=4)
                nc.gpsimd.tensor_scalar(
                    out=och, in0=obs[t], scalar1=mv_all[:, 2 * t:2 * t + 1],
                    scalar2=rstd_all[:, t:t + 1], op0=OP.subtract, op1=OP.mult)
                if apply_affine:
                    nc.vector.tensor_mul(out=och, in0=och, in1=lng_sb)
                    nc.vector.tensor_add(out=och, in0=och, in1=lnb_sb)
                nc.sync.dma_start(
                    out=outd.ap()[g * L + 128 * t: g * L + 128 * (t + 1), :],
                    in_=och)

            def phase_Wo(g, tail=False):
                att_all = st[g]["att"]
                mv_all = sp.tile([128, 2 * NT], F32, name=f"mv{g}", tag="mv")
                rstd_all = sp.tile([128, NT], F32, name=f"rstd{g}", tag="rstd")
                vf = sp.tile([128, NT], F32, name=f"vf{g}", tag="vf")
                lnv = sp.tile([128, NT], F32, name=f"lnv{g}", tag="lnv")
                obs = []
                for t in range(NT):
                    o_ps = ppool.tile([128, HID], F32, name=f"ops{g}{t}", tag="pp",
                                      bufs=6)
                    last = NCH - 1
                    for j in range(NCH):
                        nc.tensor.matmul(
                            o_ps, att_all[:, j * L + 128 * t: j * L + 128 * (t + 1)],
                            wot_sb[j], start=(j == 0),
                            stop=(j == last and not apply_bo))
                    if apply_bo:
                        nc.tensor.matmul(o_ps, ones1_sb, bo_sb, start=False,
                                         stop=True)
                    ob = sp.tile([128, HID], BF16, name=f"ob{g}{t}", tag="ob",
                                 bufs=NT + 2)
                    nc.scalar.copy(out=ob, in_=o_ps)
                    stats = sp.tile([128, 6], F32, name=f"sst{g}{t}", tag="sst")
                    nc.vector.bn_stats(out=stats, in_=ob)
                    nc.vector.bn_aggr(out=mv_all[:, 2 * t:2 * t + 2], in_=stats)
                    obs.append(ob)
                    if tail:
                        nc.vector.tensor_scalar_add(
                            out=vf[:, t:t + 1], in0=mv_all[:, 2 * t + 1:2 * t + 2],
                            scalar1=EPS)
                        nc.scalar.activation(out=lnv[:, t:t + 1], in_=vf[:, t:t + 1],
                                             func=AT.Ln)
                        nc.scalar.activation(out=rstd_all[:, t:t + 1],
                                             in_=lnv[:, t:t + 1], func=AT.Exp,
                                             scale=-0.5)
                        ln_apply(g, obs, mv_all, rstd_all, t)
                if not tail:
                    nc.gpsimd.tensor_scalar_add(out=vf, in0=mv_all[:, 1:2 * NT:2],
                                                scalar1=EPS)
                    nc.scalar.activation(out=lnv, in_=vf, func=AT.Ln)
                    nc.scalar.activation(out=rstd_all, in_=lnv, func=AT.Exp, scale=-0.5)
                    for t in range(NT):
                        ln_apply(g, obs, mv_all, rstd_all, t)
                del st[g]

            # -------- modulo schedule (A shifted one slot early) --------
            # per iter g: A(g+1) fills sttA(g); stream/Wo(g-1) fill sttB(g);
            # rbcA(g+1) at iter end once recipA(g+1) is ready.
            phase_load(0)
            phase_load(1)
            load_bulk_consts()
            phase_A(0)
            phase_rbcA(0)
            for g in range(G):
                if g + 2 < G:
                    phase_load(g + 2)
                if g + 1 < G:
                    phase_A(g + 1)
                elif g > 0:
                    # no A(g+1) filler on the last iteration: pull stream(g-1)
                    # forward to cover the sttA(g) chain instead
                    phase_stream(g - 1)
                phase_K(g)
                phase_rbcB(g)
                if g > 0:
                    if g + 1 < G:
                        phase_stream(g - 1)
                    phase_Wo(g - 1)
                phase_prep(g)
                if g + 1 < G:
                    phase_rbcA(g + 1)
            phase_stream(G - 1)
            phase_Wo(G - 1, tail=True)

    _bacc_mod.get_activation_tables = _gat
    try:
        nc.compile()
    finally:
        _bacc_mod.get_activation_tables = _orig_gat
    return nc


_NC_CACHE = {}


def _get_nc(apply_bo, apply_affine):
    key = (apply_bo, apply_affine, USE_FP8)
    if key not in _NC_CACHE:
        _NC_CACHE[key] = _build(apply_bo, apply_affine, USE_FP8)
    return _NC_CACHE[key]


def _host_consts(Wq, Wk, Wv, Wr, w_alpha, w_beta, Wo, bo, ln_g, ln_b):
    import ml_dtypes
    bf = ml_dtypes.bfloat16
    f8 = ml_dtypes.float8_e4m3fn

    wqt = np.ascontiguousarray(Wq.T)                       # [h, e]
    wvr = np.ascontiguousarray(Wv)                         # [d, h]
    wot = np.ascontiguousarray(Wo.T)
    wrt = Wr.T.astype(np.float32)                          # WrT[d, e] = Wr[e, d]
    wrbd = np.zeros((128, 128), np.float32)
    wrbd[:64, :64] = wrt; wrbd[64:, 64:] = wrt
    ident = np.eye(128, dtype=np.float32)
    wa_vec = np.tile(w_alpha, HEADS) * SCALE               # [512]
    wb_vec = np.tile(w_beta, HEADS) * SCALE
    SWV = SW if USE_FP8 else 1.0
    wacol = (wa_vec / SWV).reshape(NCH, 128).T.copy()      # [128, NCH]
    wbcol = (wb_vec / SWV).reshape(NCH, 128).T.copy()

    segs = np.zeros((128, 8 * NCH), np.float32)
    sels = np.zeros((8, HID), np.float32)
    for j in range(NCH):
        for p in range(128):
            segs[p, 8 * j + 2 * j + p // 64] = 1.0
        for m in range(HID):
            if m // 128 == j:
                sels[2 * j + (m % 128) // 64, m] = 1.0
    segs8 = np.zeros((128, 2 * 32), np.float32)
    for p in range(2):
        for d in range(128):
            hA = 4 * p + d // 64          # head of (chunk 2p, partition d)
            hB = 4 * p + 2 + d // 64      # head of (chunk 2p+1, partition d)
            segs8[d, 32 * p + hA] = 1.0
            segs8[d, 32 * p + 16 + hB] = 1.0

    common = {"wqt": wqt.astype(bf), "wvr": wvr.astype(bf),
              "wot": wot.astype(bf), "wrbd": wrbd.astype(bf),
              "ident": ident.astype(bf), "segs": segs.astype(bf),
              "segs8": segs8.astype(f8),
              "sels": sels, "wacol": wacol.astype(np.float32),
              "wbcol": wbcol.astype(np.float32)}

    if USE_FP8:
        def pack_dr(WT):   # WT [h, e] -> [128, NCH*2*256] DoubleRow stationary
            out = np.zeros((128, NCH * 2 * 256), np.float32)
            for j in range(NCH):
                for p in range(2):
                    blkA = WT[256 * p:256 * p + 128, 128 * j:128 * (j + 1)]
                    blkB = WT[256 * p + 128:256 * (p + 1), 128 * j:128 * (j + 1)]
                    c0 = (2 * j + p) * 256
                    out[:, c0:c0 + 128] = blkA
                    out[:, c0 + 128:c0 + 256] = blkB
            return out
        wq_pk = pack_dr(SW * Wq.T)
        wk_pk = pack_dr(SW * Wk.T)
        wq8 = wq_pk.astype(f8)
        wk8 = wk_pk.astype(f8)
        common["wq8"] = wq8
        common["wk8"] = wk8
        common["dwq8"] = (wq_pk - wq8.astype(np.float32)).astype(f8)
        common["dwk8"] = (wk_pk - wk8.astype(np.float32)).astype(f8)
    else:
        common["wkt"] = np.ascontiguousarray(Wk.T).astype(bf)

    apply_bo = not np.allclose(bo, 0.0)
    apply_affine = not (np.allclose(ln_g, 1.0) and np.allclose(ln_b, 0.0))
    if apply_bo:
        common["bo"] = bo.reshape(1, HID).astype(np.float32)
        common["ones1"] = np.ones((1, 128), np.float32)
    if apply_affine:
        common["ln_g"] = np.tile(ln_g, (128, 1)).astype(bf)
        common["ln_b"] = np.tile(ln_b, (128, 1)).astype(bf)
    return common, apply_bo, apply_affine


def kernel(edge_attr, batch_scopes, Wq, Wk, Wv, Wr, w_alpha, w_beta, Wo, bo,
           ln_g, ln_b):
    from concourse import bass_utils
    import ml_dtypes

    edge_attr = np.asarray(edge_attr, dtype=np.float32)
    scopes = np.asarray(batch_scopes)
    Wq = np.asarray(Wq, np.float32); Wk = np.asarray(Wk, np.float32)
    Wv = np.asarray(Wv, np.float32); Wr = np.asarray(Wr, np.float32)
    Wo = np.asarray(Wo, np.float32)
    w_alpha = np.asarray(w_alpha, np.float32); w_beta = np.asarray(w_beta, np.float32)
    bo = np.asarray(bo, np.float32)
    ln_g = np.asarray(ln_g, np.float32); ln_b = np.asarray(ln_b, np.float32)

    assert np.all(scopes[:, 1] == L), "equal-length contiguous scopes expected"
    starts = scopes[:, 0].astype(np.int64)

    common, apply_bo, apply_affine = _host_consts(
        Wq, Wk, Wv, Wr, w_alpha, w_beta, Wo, bo, ln_g, ln_b)
    nc = _get_nc(apply_bo, apply_affine)

    bf = ml_dtypes.bfloat16
    f8 = ml_dtypes.float8_e4m3fn
    in_maps = []
    for c in range(NCORES):
        rows = np.concatenate([
            np.arange(starts[c * G + g], starts[c * G + g] + L)
            for g in range(G)])
        xslab = edge_attr[rows]                       # [G*L, 512]
        xt = np.ascontiguousarray(xslab.T)
        m = {"xtb": xt.astype(bf), **common}
        if USE_FP8:
            x8 = xt.astype(f8)
            m["xt8"] = x8
            m["dxt8"] = (xt - x8.astype(np.float32)).astype(f8)
        in_maps.append(m)

    res = bass_utils.run_bass_kernel_spmd(nc, in_maps, core_ids=list(range(NCORES)))
    out = np.concatenate([r["out"] for r in res.results], axis=0)
    return out.astype(np.float32)


# revision 37
# speedup vs baseline: 1.1927x; 1.0168x over previous
"""BondFastAttention Trainium2 kernel (self-contained), v2.

Shapes (hardcoded from the problem spec):
  edge_attr [65536, 512] fp32, B=64 graphs x L=1024 bonds, HID=512, 8 heads x D=64.
  8 NeuronCores, data-parallel over graphs: G=8 graphs per core.

Device layout: transposed domain - features on partitions, tokens on free dim
for Q/K/kvout; tokens on partitions for the Wo/LayerNorm stage.

Key structure vs v1:
  - The V projection, Wr matmul, gk scaling and +q add are all folded into a
    single per-graph combined weight W''' = Wv^T (gk . Wr^T) + Wq^T, built on
    the PE (16 small matmuls + identity-add), so one X-stream produces
    relu-input directly.
  - Projection PSUM is consumed in place (ACT exp, Pool multiply) - no
    psum->sbuf copies for q/k/v.
  - Optional fp8 path: Q/K projections and softmax seg-sums run as fp8
    DoubleRow matmuls (K=256 per pass).
  - Output is written bf16 and upcast to f32 on the host.
"""
import numpy as np

HID = 512
HEADS = 8
D = 64
B = 64
L = 1024
SCALE = D ** -0.5
EPS = 1e-5
NCORES = 8
G = B // NCORES          # graphs per core
NCH = HID // 128         # 4 feature chunks (2 heads each)
NT = L // 128            # 8 token chunks
SW = 16.0                # fp8 weight prescale

USE_FP8 = True


def _build(apply_bo: bool, apply_affine: bool, use_fp8: bool):
    import concourse.bass as bass
    from concourse import bacc
    import concourse.mybir as mybir
    from concourse.tile import TileContext

    F32 = mybir.dt.float32
    F32R = mybir.dt.float32r
    BF16 = mybir.dt.bfloat16
    FP8 = mybir.dt.float8e4
    AT = mybir.ActivationFunctionType
    OP = mybir.AluOpType
    PM = mybir.MatmulPerfMode

    nc = bacc.Bacc()

    import concourse.bacc as _bacc_mod
    _orig_gat = _bacc_mod.get_activation_tables

    def _gat(arch):
        # Keep dict order but strip our funcs from every other set, so the
        # table-load pass assigns all of them to natural_log_exp_and_others
        # -> a single physical table load.
        t = _orig_gat(arch)
        ours = {AT.Exp, AT.Ln, AT.Copy, AT.Relu, AT.Identity}
        out = {}
        for k, funcs in t.items():
            if k == "natural_log_exp_and_others":
                out[k] = funcs
            else:
                out[k] = {f for f in funcs if f not in ours}
        return out

    # ---------------- dram tensors ----------------
    xtb = nc.dram_tensor("xtb", [HID, G * L], BF16, kind="ExternalInput")
    if use_fp8:
        xt8 = nc.dram_tensor("xt8", [HID, G * L], FP8, kind="ExternalInput")
        dxt8 = nc.dram_tensor("dxt8", [HID, G * L], FP8, kind="ExternalInput")
        wq8d = nc.dram_tensor("wq8", [128, NCH * 2 * 256], FP8, kind="ExternalInput")
        wk8d = nc.dram_tensor("wk8", [128, NCH * 2 * 256], FP8, kind="ExternalInput")
        dwq8d = nc.dram_tensor("dwq8", [128, NCH * 2 * 256], FP8, kind="ExternalInput")
        dwk8d = nc.dram_tensor("dwk8", [128, NCH * 2 * 256], FP8, kind="ExternalInput")
    else:
        wktd = nc.dram_tensor("wkt", [HID, HID], BF16, kind="ExternalInput")
    wqtd = nc.dram_tensor("wqt", [HID, HID], BF16, kind="ExternalInput")
    wvrd = nc.dram_tensor("wvr", [HID, HID], BF16, kind="ExternalInput")
    wotd = nc.dram_tensor("wot", [HID, HID], BF16, kind="ExternalInput")
    wrbdd = nc.dram_tensor("wrbd", [128, 128], BF16, kind="ExternalInput")
    identd = nc.dram_tensor("ident", [128, 128], BF16, kind="ExternalInput")
    segsd = nc.dram_tensor("segs", [128, 8 * NCH], BF16, kind="ExternalInput")
    selsd = nc.dram_tensor("sels", [8, HID], F32, kind="ExternalInput")
    wacold = nc.dram_tensor("wacol", [128, NCH], F32, kind="ExternalInput")
    wbcold = nc.dram_tensor("wbcol", [128, NCH], F32, kind="ExternalInput")
    if apply_bo:
        bod = nc.dram_tensor("bo", [1, HID], F32, kind="ExternalInput")
        onesd = nc.dram_tensor("ones1", [1, 128], F32, kind="ExternalInput")
    if apply_affine:
        lngd = nc.dram_tensor("ln_g", [128, HID], BF16, kind="ExternalInput")
        lnbd = nc.dram_tensor("ln_b", [128, HID], BF16, kind="ExternalInput")
    outd = nc.dram_tensor("out", [G * L, HID], BF16, kind="ExternalOutput")

    SWV = SW if use_fp8 else 1.0

    with TileContext(nc) as tc:
        with tc.tile_pool(name="consts", bufs=1) as cp, \
             tc.tile_pool(name="big", bufs=1) as bp, \
             tc.tile_pool(name="small", bufs=2) as sp, \
             tc.tile_pool(name="psum", bufs=1, space="PSUM") as ppool:

            # ---- constants to SBUF ----
            # Phase-0-critical consts first; bulk consts go AFTER the first
            # graph's X loads are queued, spread across SP/ACT/Pool DGEs.
            wacol_sb = cp.tile([128, NCH], F32)
            nc.sync.dma_start(out=wacol_sb, in_=wacold.ap())
            if use_fp8:
                wq8_sb = cp.tile([128, NCH * 2 * 256], FP8)
                nc.sync.dma_start(out=wq8_sb, in_=wq8d.ap())
                dwq8_sb = cp.tile([128, NCH * 2 * 256], FP8)
                nc.gpsimd.dma_start(out=dwq8_sb, in_=dwq8d.ap())
                wk8_sb = cp.tile([128, NCH * 2 * 256], FP8)
                dwk8_sb = cp.tile([128, NCH * 2 * 256], FP8)
            else:
                wkt_sb = [cp.tile([128, HID], BF16, name=f"wkt{i}") for i in range(NCH)]
            wqt_sb = [cp.tile([128, HID], BF16, name=f"wqt{i}") for i in range(NCH)]
            segs_sb = cp.tile([128, 8 * NCH], BF16)
            sels_sb = cp.tile([8, HID], F32R)

            def load_bulk_consts():
                nc.scalar.dma_start(out=segs_sb, in_=segsd.ap())
                nc.sync.dma_start(out=sels_sb, in_=selsd.ap().bitcast(F32R))
                if use_fp8:
                    nc.scalar.dma_start(out=wk8_sb, in_=wk8d.ap())
                    nc.scalar.dma_start(out=dwk8_sb, in_=dwk8d.ap())
                for i in range(NCH):
                    nc.sync.dma_start(out=wqt_sb[i],
                                      in_=wqtd.ap()[128 * i:128 * (i + 1), :])
                if not use_fp8:
                    for i in range(NCH):
                        nc.scalar.dma_start(
                            out=wkt_sb[i], in_=wktd.ap()[128 * i:128 * (i + 1), :])
                for j in range(NCH):
                    nc.gpsimd.dma_start(out=wvr_sb[j],
                                        in_=wvrd.ap()[128 * j:128 * (j + 1), :])
                for j in range(NCH):
                    nc.gpsimd.dma_start(out=wot_sb[j],
                                        in_=wotd.ap()[128 * j:128 * (j + 1), :])
                nc.scalar.dma_start(out=wrbd_sb, in_=wrbdd.ap())
                nc.scalar.dma_start(out=ident_sb, in_=identd.ap())
                nc.scalar.dma_start(out=wbcol_sb, in_=wbcold.ap())
                if apply_bo:
                    nc.scalar.dma_start(out=ones1_sb, in_=onesd.ap().bitcast(F32R))
                    nc.scalar.dma_start(out=bo_sb, in_=bod.ap().bitcast(F32R))
                if apply_affine:
                    nc.gpsimd.dma_start(out=lng_sb, in_=lngd.ap())
                    nc.gpsimd.dma_start(out=lnb_sb, in_=lnbd.ap())

            wvr_sb = [cp.tile([128, HID], BF16, name=f"wvr{j}") for j in range(NCH)]
            wot_sb = [cp.tile([128, HID], BF16, name=f"wot{j}") for j in range(NCH)]
            wrbd_sb = cp.tile([128, 128], BF16)
            ident_sb = cp.tile([128, 128], BF16)
            wbcol_sb = cp.tile([128, NCH], F32)
            if apply_bo:
                ones1_sb = cp.tile([1, 128], F32R)
                bo_sb = cp.tile([1, HID], F32R)
            if apply_affine:
                lng_sb = cp.tile([128, HID], BF16)
                lnb_sb = cp.tile([128, HID], BF16)

            EDT = BF16

            # -------- per-graph state (software-pipelined emission) --------
            st = {}

            def phase_load(g):
                s = {}
                s["xtb"] = bp.tile([128, NCH * L], BF16, name=f"xtb{g}", tag="xtb",
                                   bufs=5)
                xtb_src = bass.AP(
                    tensor=xtb.ap().tensor, offset=g * L,
                    ap=[[G * L, 128], [128 * G * L, NCH], [1, L]])
                nc.sync.dma_start(
                    out=s["xtb"].rearrange("p (i l) -> p i l", i=NCH), in_=xtb_src)
                if use_fp8:
                    s["xt8"] = bp.tile([128, NCH * L], FP8, name=f"xt8{g}",
                                       tag="xt8", bufs=3)
                    xt8_src = bass.AP(
                        tensor=xt8.ap().tensor, offset=g * L,
                        ap=[[G * L, 128], [128 * G * L, NCH], [1, L]])
                    nc.scalar.dma_start(
                        out=s["xt8"].rearrange("p (i l) -> p i l", i=NCH),
                        in_=xt8_src)
                    s["dxt8"] = bp.tile([128, NCH * L], FP8, name=f"dxt8{g}",
                                        tag="dxt8", bufs=3)
                    dxt8_src = bass.AP(
                        tensor=dxt8.ap().tensor, offset=g * L,
                        ap=[[G * L, 128], [128 * G * L, NCH], [1, L]])
                    nc.gpsimd.dma_start(
                        out=s["dxt8"].rearrange("p (i l) -> p i l", i=NCH),
                        in_=dxt8_src)
                st[g] = s

            def proj_half(g, w8_sb, w_sb, j, n0, pp):
                if use_fp8:
                    w8, dw8 = w8_sb
                    xt8_3d = st[g]["xt8"].rearrange("p (i l) -> p i l", i=NCH)
                    dxt8_3d = st[g]["dxt8"].rearrange("p (i l) -> p i l", i=NCH)
                    # psum = X8 @ W8 + X8 @ dW8 + dX8 @ W8  (~bf16 accuracy)
                    plan = [(w8, xt8_3d), (dw8, xt8_3d), (w8, dxt8_3d)]
                    nmm = len(plan) * 2
                    k = 0
                    for wsb, xsb in plan:
                        for p in range(2):
                            lhs = wsb[:, (2 * j + p) * 256:(2 * j + p + 1) * 256] \
                                .rearrange("p (two f) -> p two f", two=2)
                            rhs = xsb[:, 2 * p:2 * p + 2, n0:n0 + 512]
                            nc.tensor.matmul(pp, lhs, rhs, start=(k == 0),
                                             stop=(k == nmm - 1),
                                             perf_mode=PM.DoubleRow)
                            k += 1
                else:
                    for i in range(NCH):
                        nc.tensor.matmul(
                            pp, w_sb[i][:, 128 * j:128 * (j + 1)],
                            st[g]["xtb"][:, i * L + n0: i * L + n0 + 512],
                            start=(i == 0), stop=(i == NCH - 1))

            def proj_stage(g, tag, w8_sb, w_sb, scale_ap):
                """projection + exp + m for one of Q/K; then seg-sums + recip."""
                e_all = bp.tile([128, NCH * L], EDT, name=f"e{tag}{g}", tag="e",
                                bufs=3)
                m_tiles = []
                for j in range(NCH):
                    mt = sp.tile([128, L], BF16, name=f"m{tag}{g}{j}", tag="scr",
                                 bufs=8)
                    m_tiles.append(mt)
                for j in range(NCH):
                    for n0 in (0, 512):
                        pp = ppool.tile([128, 512], F32, name=f"pp{tag}{g}{j}{n0}",
                                        tag="pp", bufs=6)
                        proj_half(g, w8_sb, w_sb, j, n0, pp)
                        nc.scalar.activation(
                            out=e_all[:, j * L + n0: j * L + n0 + 512], in_=pp,
                            func=AT.Exp, scale=scale_ap[:, j:j + 1])
                        nc.vector.tensor_mul(
                            out=m_tiles[j][:, n0:n0 + 512],
                            in0=e_all[:, j * L + n0: j * L + n0 + 512], in1=pp)
                sos = []
                for hi, n0 in enumerate((0, 512)):
                    so = ppool.tile([16, 512], F32, name=f"so{tag}{g}{n0}",
                                    tag="rs", bufs=2, padded_shape=[128, 512])
                    for j in range(NCH):
                        nc.tensor.matmul(
                            so[0:8, :], segs_sb[:, 8 * j:8 * (j + 1)],
                            e_all[:, j * L + n0: j * L + n0 + 512],
                            start=(j == 0), stop=(j == NCH - 1))
                    sos.append(so)
                rt = sp.tile([8, 1024], F32, name=f"rt{tag}{g}", tag="rt", bufs=3)
                nc.vector.reciprocal_approx_fast(out=rt[:, 0:512], in_=sos[0][0:8, :])
                nc.vector.reciprocal_approx_fast(out=rt[:, 512:1024], in_=sos[1][0:8, :])
                rtr = sp.tile([8, 1024], F32R, name=f"rtr{tag}{g}", tag="rtr", bufs=3)
                nc.scalar.copy(out=rtr, in_=rt)
                st[g][f"m{tag}"] = m_tiles
                st[g][f"rt{tag}"] = rtr

            def rbc_stage(g, tag):
                """rbc expand + stt accumulate; returns summed [128, NCH] tile."""
                rtr = st[g][f"rt{tag}"]
                m_tiles = st[g][f"m{tag}"]
                parts = sp.tile([128, 2 * NCH], F32, name=f"pts{tag}{g}",
                                tag=f"pts_{tag}")
                for j in range(NCH):
                    for hi, n0 in enumerate((0, 512)):
                        rbc = ppool.tile([128, 512], F32, name=f"rbc{tag}{g}{j}{n0}",
                                         tag="rs", bufs=2)
                        nc.tensor.matmul(rbc, sels_sb[:, 128 * j:128 * (j + 1)],
                                         rtr[:, n0:n0 + 512])
                        nc.vector.scalar_tensor_tensor(
                            out=m_tiles[j][:, n0:n0 + 512],
                            in0=m_tiles[j][:, n0:n0 + 512],
                            scalar=1.0 / SWV, in1=rbc,
                            op0=OP.mult, op1=OP.mult,
                            accum_out=parts[:, hi * NCH + j:hi * NCH + j + 1])
                tot = sp.tile([128, NCH], F32, name=f"tot{tag}{g}", tag=f"tot{tag}")
                nc.gpsimd.tensor_add(out=tot, in0=parts[:, 0:NCH],
                                     in1=parts[:, NCH:2 * NCH])
                return tot

            def phase_A(g):
                proj_stage(g, "a", (wq8_sb, dwq8_sb) if use_fp8 else None,
                           None if use_fp8 else wqt_sb, wacol_sb)

            def phase_rbcA(g):
                gq = rbc_stage(g, "a")
                gqwb = sp.tile([128, NCH], F32, name=f"gqwb{g}", tag="gqwb")
                nc.gpsimd.tensor_mul(out=gqwb, in0=gq, in1=wbcol_sb)
                st[g]["gq"] = gq
                st[g]["gqwb"] = gqwb

            def phase_K(g):
                proj_stage(g, "b", (wk8_sb, dwk8_sb) if use_fp8 else None,
                           None if use_fp8 else wkt_sb, st[g]["gqwb"])

            def phase_rbcB(g):
                acc = rbc_stage(g, "b")
                gk = sp.tile([128, NCH], F32, name=f"gk{g}", tag="gk")
                nc.gpsimd.tensor_mul(out=gk, in0=acc, in1=st[g]["gq"])
                gkwr = sp.tile([128, NCH * 128], BF16, name=f"gkwr{g}", tag="gkwr")
                for j in range(NCH):
                    nc.gpsimd.tensor_scalar_mul(
                        out=gkwr[:, 128 * j:128 * (j + 1)], in0=wrbd_sb,
                        scalar1=gk[:, j:j + 1])
                st[g]["gkwr"] = gkwr

            def phase_prep(g):
                gkwr = st[g]["gkwr"]
                w3_sb = []
                for i in range(NCH):
                    ppw = ppool.tile([128, 512], F32, name=f"ppw{g}{i}", tag="pp",
                                     bufs=6)
                    for j in range(NCH):
                        nc.tensor.matmul(
                            ppw[:, 128 * j:128 * (j + 1)],
                            wvr_sb[j][:, 128 * i:128 * (i + 1)],
                            gkwr[:, 128 * j:128 * (j + 1)],
                            start=True, stop=False)
                        nc.tensor.matmul(
                            ppw[:, 128 * j:128 * (j + 1)], ident_sb,
                            wqt_sb[i][:, 128 * j:128 * (j + 1)],
                            start=False, stop=True)
                    w3 = sp.tile([128, 512], BF16, name=f"w3{g}{i}", tag="w3",
                                 bufs=8)
                    nc.scalar.copy(out=w3, in_=ppw)
                    w3_sb.append(w3)
                st[g]["w3"] = w3_sb

            def phase_stream(g):
                w3_sb = st[g]["w3"]
                xtb_all = st[g]["xtb"]
                att_all = bp.tile([128, NCH * L], BF16, name=f"att{g}", tag="att",
                                  bufs=2)
                for j in range(NCH):
                    for n0 in (0, 512):
                        ppv = ppool.tile([128, 512], F32, name=f"ppv{g}{j}{n0}",
                                         tag="pp", bufs=6)
                        for i in range(NCH):
                            nc.tensor.matmul(
                                ppv, w3_sb[i][:, 128 * j:128 * (j + 1)],
                                xtb_all[:, i * L + n0: i * L + n0 + 512],
                                start=(i == 0), stop=(i == NCH - 1))
                        nc.scalar.activation(
                            out=att_all[:, j * L + n0: j * L + n0 + 512], in_=ppv,
                            func=AT.Relu)
                st[g]["att"] = att_all

            def ln_apply(g, obs, mv_all, rstd_all, t):
                och = sp.tile([128, HID], BF16, name=f"och{g}{t}", tag="och",
                              bufs=4)
                nc.gpsimd.tensor_scalar(
                    out=och, in0=obs[t], scalar1=mv_all[:, 2 * t:2 * t + 1],
                    scalar2=rstd_all[:, t:t + 1], op0=OP.subtract, op1=OP.mult)
                if apply_affine:
                    nc.vector.tensor_mul(out=och, in0=och, in1=lng_sb)
                    nc.vector.tensor_add(out=och, in0=och, in1=lnb_sb)
                nc.sync.dma_start(
                    out=outd.ap()[g * L + 128 * t: g * L + 128 * (t + 1), :],
                    in_=och)

            def phase_Wo(g, tail=False):
                att_all = st[g]["att"]
                mv_all = sp.tile([128, 2 * NT], F32, name=f"mv{g}", tag="mv")
                rstd_all = sp.tile([128, NT], F32, name=f"rstd{g}", tag="rstd")
                vf = sp.tile([128, NT], F32, name=f"vf{g}", tag="vf")
                lnv = sp.tile([128, NT], F32, name=f"lnv{g}", tag="lnv")
                obs = []
                for t in range(NT):
                    o_ps = ppool.tile([128, HID], F32, name=f"ops{g}{t}", tag="pp",
                                      bufs=6)
                    last = NCH - 1
                    for j in range(NCH):
                        nc.tensor.matmul(
                            o_ps, att_all[:, j * L + 128 * t: j * L + 128 * (t + 1)],
                            wot_sb[j], start=(j == 0),
                            stop=(j == last and not apply_bo))
                    if apply_bo:
                        nc.tensor.matmul(o_ps, ones1_sb, bo_sb, start=False,
                                         stop=True)
                    ob = sp.tile([128, HID], BF16, name=f"ob{g}{t}", tag="ob",
                                 bufs=NT + 2)
                    nc.scalar.copy(out=ob, in_=o_ps)
                    stats = sp.tile([128, 6], F32, name=f"sst{g}{t}", tag="sst")
                    nc.vector.bn_stats(out=stats, in_=ob)
                    nc.vector.bn_aggr(out=mv_all[:, 2 * t:2 * t + 2], in_=stats)
                    obs.append(ob)
                    if tail:
                        nc.vector.tensor_scalar_add(
                            out=vf[:, t:t + 1], in0=mv_all[:, 2 * t + 1:2 * t + 2],
                            scalar1=EPS)
                        nc.scalar.activation(out=lnv[:, t:t + 1], in_=vf[:, t:t + 1],
                                             func=AT.Ln)
                        nc.scalar.activation(out=rstd_all[:, t:t + 1],
                                             in_=lnv[:, t:t + 1], func=AT.Exp,
                                             scale=-0.5)
                        ln_apply(g, obs, mv_all, rstd_all, t)
                if not tail:
                    nc.gpsimd.tensor_scalar_add(out=vf, in0=mv_all[:, 1:2 * NT:2],
                                                scalar1=EPS)
                    nc.scalar.activation(out=lnv, in_=vf, func=AT.Ln)
                    nc.scalar.activation(out=rstd_all, in_=lnv, func=AT.Exp, scale=-0.5)
                    for t in range(NT):
                        ln_apply(g, obs, mv_all, rstd_all, t)
                del st[g]

            # -------- modulo schedule (A shifted one slot early) --------
            # per iter g: A(g+1) fills sttA(g); stream/Wo(g-1) fill sttB(g);
            # rbcA(g+1) at iter end once recipA(g+1) is ready.
            phase_load(0)
            phase_load(1)
            load_bulk_consts()
            phase_load(2)
            phase_A(0)
            phase_rbcA(0)
            for g in range(G):
                if g + 3 < G:
                    phase_load(g + 3)
                if g == 0:
                    phase_A(1)
                elif g == 1:
                    # A(2) was pulled into iter 0; stream(0) fills sttA(1)
                    phase_stream(0)
                elif g + 1 < G:
                    phase_A(g + 1)
                else:
                    # last iteration: stream(g-1) covers the sttA(g) chain
                    phase_stream(g - 1)
                phase_K(g)
                phase_rbcB(g)
                if g == 0:
                    phase_A(2)
                else:
                    if g + 1 < G and g != 1:
                        phase_stream(g - 1)
                    phase_Wo(g - 1)
                phase_prep(g)
                if g + 1 < G:
                    phase_rbcA(g + 1)
            phase_stream(G - 1)
            phase_Wo(G - 1, tail=True)

    _bacc_mod.get_activation_tables = _gat
    try:
        nc.compile()
    finally:
        _bacc_mod.get_activation_tables = _orig_gat
    return nc


_NC_CACHE = {}


def _get_nc(apply_bo, apply_affine):
    key = (apply_bo, apply_affine, USE_FP8)
    if key not in _NC_CACHE:
        _NC_CACHE[key] = _build(apply_bo, apply_affine, USE_FP8)
    return _NC_CACHE[key]


def _host_consts(Wq, Wk, Wv, Wr, w_alpha, w_beta, Wo, bo, ln_g, ln_b):
    import ml_dtypes
    bf = ml_dtypes.bfloat16
    f8 = ml_dtypes.float8_e4m3fn

    wqt = np.ascontiguousarray(Wq.T)                       # [h, e]
    wvr = np.ascontiguousarray(Wv)                         # [d, h]
    wot = np.ascontiguousarray(Wo.T)
    wrt = Wr.T.astype(np.float32)                          # WrT[d, e] = Wr[e, d]
    wrbd = np.zeros((128, 128), np.float32)
    wrbd[:64, :64] = wrt; wrbd[64:, 64:] = wrt
    ident = np.eye(128, dtype=np.float32)
    wa_vec = np.tile(w_alpha, HEADS) * SCALE               # [512]
    wb_vec = np.tile(w_beta, HEADS) * SCALE
    SWV = SW if USE_FP8 else 1.0
    wacol = (wa_vec / SWV).reshape(NCH, 128).T.copy()      # [128, NCH]
    wbcol = (wb_vec / SWV).reshape(NCH, 128).T.copy()

    segs = np.zeros((128, 8 * NCH), np.float32)
    sels = np.zeros((8, HID), np.float32)
    for j in range(NCH):
        for p in range(128):
            segs[p, 8 * j + 2 * j + p // 64] = 1.0
        for m in range(HID):
            if m // 128 == j:
                sels[2 * j + (m % 128) // 64, m] = 1.0

    common = {"wqt": wqt.astype(bf), "wvr": wvr.astype(bf),
              "wot": wot.astype(bf), "wrbd": wrbd.astype(bf),
              "ident": ident.astype(bf), "segs": segs.astype(bf),
              "sels": sels, "wacol": wacol.astype(np.float32),
              "wbcol": wbcol.astype(np.float32)}

    if USE_FP8:
        def pack_dr(WT):   # WT [h, e] -> [128, NCH*2*256] DoubleRow stationary
            out = np.zeros((128, NCH * 2 * 256), np.float32)
            for j in range(NCH):
                for p in range(2):
                    blkA = WT[256 * p:256 * p + 128, 128 * j:128 * (j + 1)]
                    blkB = WT[256 * p + 128:256 * (p + 1), 128 * j:128 * (j + 1)]
                    c0 = (2 * j + p) * 256
                    out[:, c0:c0 + 128] = blkA
                    out[:, c0 + 128:c0 + 256] = blkB
            return out
        wq_pk = pack_dr(SW * Wq.T)
        wk_pk = pack_dr(SW * Wk.T)
        wq8 = wq_pk.astype(f8)
        wk8 = wk_pk.astype(f8)
        common["wq8"] = wq8
        common["wk8"] = wk8
        common["dwq8"] = (wq_pk - wq8.astype(np.float32)).astype(f8)
        common["dwk8"] = (wk_pk - wk8.astype(np.float32)).astype(f8)
    else:
        common["wkt"] = np.ascontiguousarray(Wk.T).astype(bf)

    apply_bo = not np.allclose(bo, 0.0)
    apply_affine = not (np.allclose(ln_g, 1.0) and np.allclose(ln_b, 0.0))
    if apply_bo:
        common["bo"] = bo.reshape(1, HID).astype(np.float32)
        common["ones1"] = np.ones((1, 128), np.float32)
    if apply_affine:
        common["ln_g"] = np.tile(ln_g, (128, 1)).astype(bf)
        common["ln_b"] = np.tile(ln_b, (128, 1)).astype(bf)
    return common, apply_bo, apply_affine


def kernel(edge_attr, batch_scopes, Wq, Wk, Wv, Wr, w_alpha, w_beta, Wo, bo,
           ln_g, ln_b):
    from concourse import bass_utils
    import ml_dtypes

    edge_attr = np.asarray(edge_attr, dtype=np.float32)
    scopes = np.asarray(batch_scopes)
    Wq = np.asarray(Wq, np.float32); Wk = np.asarray(Wk, np.float32)
    Wv = np.asarray(Wv, np.float32); Wr = np.asarray(Wr, np.float32)
    Wo = np.asarray(Wo, np.float32)
    w_alpha = np.asarray(w_alpha, np.float32); w_beta = np.asarray(w_beta, np.float32)
    bo = np.asarray(bo, np.float32)
    ln_g = np.asarray(ln_g, np.float32); ln_b = np.asarray(ln_b, np.float32)

    assert np.all(scopes[:, 1] == L), "equal-length contiguous scopes expected"
    starts = scopes[:, 0].astype(np.int64)

    common, apply_bo, apply_affine = _host_consts(
        Wq, Wk, Wv, Wr, w_alpha, w_beta, Wo, bo, ln_g, ln_b)
    nc = _get_nc(apply_bo, apply_affine)

    bf = ml_dtypes.bfloat16
    f8 = ml_dtypes.float8_e4m3fn
    in_maps = []
    for c in range(NCORES):
        rows = np.concatenate([
            np.arange(starts[c * G + g], starts[c * G + g] + L)
            for g in range(G)])
        xslab = edge_attr[rows]                       # [G*L, 512]
        xt = np.ascontiguousarray(xslab.T)
        m = {"xtb": xt.astype(bf), **common}
        if USE_FP8:
            x8 = xt.astype(f8)
            m["xt8"] = x8
            m["dxt8"] = (xt - x8.astype(np.float32)).astype(f8)
        in_maps.append(m)

    res = bass_utils.run_bass_kernel_spmd(nc, in_maps, core_ids=list(range(NCORES)))
    out = np.concatenate([r["out"] for r in res.results], axis=0)
    return out.astype(np.float32)


# revision 43
# speedup vs baseline: 1.2002x; 1.0062x over previous
"""BondFastAttention Trainium2 kernel (self-contained), v2.

Shapes (hardcoded from the problem spec):
  edge_attr [65536, 512] fp32, B=64 graphs x L=1024 bonds, HID=512, 8 heads x D=64.
  8 NeuronCores, data-parallel over graphs: G=8 graphs per core.

Device layout: transposed domain - features on partitions, tokens on free dim
for Q/K/kvout; tokens on partitions for the Wo/LayerNorm stage.

Key structure vs v1:
  - The V projection, Wr matmul, gk scaling and +q add are all folded into a
    single per-graph combined weight W''' = Wv^T (gk . Wr^T) + Wq^T, built on
    the PE (16 small matmuls + identity-add), so one X-stream produces
    relu-input directly.
  - Projection PSUM is consumed in place (ACT exp, Pool multiply) - no
    psum->sbuf copies for q/k/v.
  - Optional fp8 path: Q/K projections and softmax seg-sums run as fp8
    DoubleRow matmuls (K=256 per pass).
  - Output is written bf16 and upcast to f32 on the host.
"""
import numpy as np

HID = 512
HEADS = 8
D = 64
B = 64
L = 1024
SCALE = D ** -0.5
EPS = 1e-5
NCORES = 8
G = B // NCORES          # graphs per core
NCH = HID // 128         # 4 feature chunks (2 heads each)
NT = L // 128            # 8 token chunks
SW = 16.0                # fp8 weight prescale

USE_FP8 = True


def _build(apply_bo: bool, apply_affine: bool, use_fp8: bool):
    import concourse.bass as bass
    from concourse import bacc
    import concourse.mybir as mybir
    from concourse.tile import TileContext

    F32 = mybir.dt.float32
    F32R = mybir.dt.float32r
    BF16 = mybir.dt.bfloat16
    FP8 = mybir.dt.float8e4
    AT = mybir.ActivationFunctionType
    OP = mybir.AluOpType
    PM = mybir.MatmulPerfMode

    nc = bacc.Bacc()

    import concourse.bacc as _bacc_mod
    _orig_gat = _bacc_mod.get_activation_tables

    def _gat(arch):
        # Keep dict order but strip our funcs from every other set, so the
        # table-load pass assigns all of them to natural_log_exp_and_others
        # -> a single physical table load.
        t = _orig_gat(arch)
        ours = {AT.Exp, AT.Ln, AT.Copy, AT.Relu, AT.Identity}
        out = {}
        for k, funcs in t.items():
            if k == "natural_log_exp_and_others":
                out[k] = funcs
            else:
                out[k] = {f for f in funcs if f not in ours}
        return out

    # ---------------- dram tensors ----------------
    xtb = nc.dram_tensor("xtb", [HID, G * L], BF16, kind="ExternalInput")
    if use_fp8:
        xt8 = nc.dram_tensor("xt8", [HID, G * L], FP8, kind="ExternalInput")
        dxt8 = nc.dram_tensor("dxt8", [HID, G * L], FP8, kind="ExternalInput")
        wq8d = nc.dram_tensor("wq8", [128, NCH * 2 * 256], FP8, kind="ExternalInput")
        wk8d = nc.dram_tensor("wk8", [128, NCH * 2 * 256], FP8, kind="ExternalInput")
        dwq8d = nc.dram_tensor("dwq8", [128, NCH * 2 * 256], FP8, kind="ExternalInput")
        dwk8d = nc.dram_tensor("dwk8", [128, NCH * 2 * 256], FP8, kind="ExternalInput")
    else:
        wktd = nc.dram_tensor("wkt", [HID, HID], BF16, kind="ExternalInput")
    wqtd = nc.dram_tensor("wqt", [HID, HID], BF16, kind="ExternalInput")
    wvrd = nc.dram_tensor("wvr", [HID, HID], BF16, kind="ExternalInput")
    wotd = nc.dram_tensor("wot", [HID, HID], BF16, kind="ExternalInput")
    wrbdd = nc.dram_tensor("wrbd", [128, 128], BF16, kind="ExternalInput")
    identd = nc.dram_tensor("ident", [128, 128], BF16, kind="ExternalInput")
    segsd = nc.dram_tensor("segs", [128, 8 * NCH], BF16, kind="ExternalInput")
    selsd = nc.dram_tensor("sels", [8, HID], F32, kind="ExternalInput")
    wacold = nc.dram_tensor("wacol", [128, NCH], F32, kind="ExternalInput")
    wbcold = nc.dram_tensor("wbcol", [128, NCH], F32, kind="ExternalInput")
    if apply_bo:
        bod = nc.dram_tensor("bo", [1, HID], F32, kind="ExternalInput")
        onesd = nc.dram_tensor("ones1", [1, 128], F32, kind="ExternalInput")
    if apply_affine:
        lngd = nc.dram_tensor("ln_g", [128, HID], BF16, kind="ExternalInput")
        lnbd = nc.dram_tensor("ln_b", [128, HID], BF16, kind="ExternalInput")
    outd = nc.dram_tensor("out", [G * L, HID], BF16, kind="ExternalOutput")

    SWV = SW if use_fp8 else 1.0

    with TileContext(nc) as tc:
        with tc.tile_pool(name="consts", bufs=1) as cp, \
             tc.tile_pool(name="big", bufs=1) as bp, \
             tc.tile_pool(name="small", bufs=2) as sp, \
             tc.tile_pool(name="psum", bufs=1, space="PSUM") as ppool:

            # ---- constants to SBUF ----
            # Phase-0-critical consts first; bulk consts go AFTER the first
            # graph's X loads are queued, spread across SP/ACT/Pool DGEs.
            wacol_sb = cp.tile([128, NCH], F32)
            if use_fp8:
                wq8_sb = cp.tile([128, NCH * 2 * 256], FP8)
                nc.sync.dma_start(out=wq8_sb, in_=wq8d.ap())
            nc.sync.dma_start(out=wacol_sb, in_=wacold.ap())
            if use_fp8:
                dwq8_sb = cp.tile([128, NCH * 2 * 256], FP8)
                nc.gpsimd.dma_start(out=dwq8_sb, in_=dwq8d.ap())
                wk8_sb = cp.tile([128, NCH * 2 * 256], FP8)
                dwk8_sb = cp.tile([128, NCH * 2 * 256], FP8)
            else:
                wkt_sb = [cp.tile([128, HID], BF16, name=f"wkt{i}") for i in range(NCH)]
            wqt_sb = [cp.tile([128, HID], BF16, name=f"wqt{i}") for i in range(NCH)]
            segs_sb = cp.tile([128, 8 * NCH], BF16)
            sels_sb = cp.tile([8, HID], F32R)

            def load_bulk_consts():
                nc.scalar.dma_start(out=segs_sb, in_=segsd.ap())
                nc.sync.dma_start(out=sels_sb, in_=selsd.ap().bitcast(F32R))
                if use_fp8:
                    nc.scalar.dma_start(out=wk8_sb, in_=wk8d.ap())
                    nc.scalar.dma_start(out=dwk8_sb, in_=dwk8d.ap())
                for i in range(NCH):
                    nc.sync.dma_start(out=wqt_sb[i],
                                      in_=wqtd.ap()[128 * i:128 * (i + 1), :])
                if not use_fp8:
                    for i in range(NCH):
                        nc.scalar.dma_start(
                            out=wkt_sb[i], in_=wktd.ap()[128 * i:128 * (i + 1), :])
                for j in range(NCH):
                    nc.gpsimd.dma_start(out=wvr_sb[j],
                                        in_=wvrd.ap()[128 * j:128 * (j + 1), :])
                for j in range(NCH):
                    nc.gpsimd.dma_start(out=wot_sb[j],
                                        in_=wotd.ap()[128 * j:128 * (j + 1), :])
                nc.scalar.dma_start(out=wrbd_sb, in_=wrbdd.ap())
                nc.scalar.dma_start(out=ident_sb, in_=identd.ap())
                nc.scalar.dma_start(out=wbcol_sb, in_=wbcold.ap())
                if apply_bo:
                    nc.scalar.dma_start(out=ones1_sb, in_=onesd.ap().bitcast(F32R))
                    nc.scalar.dma_start(out=bo_sb, in_=bod.ap().bitcast(F32R))
                if apply_affine:
                    nc.gpsimd.dma_start(out=lng_sb, in_=lngd.ap())
                    nc.gpsimd.dma_start(out=lnb_sb, in_=lnbd.ap())

            wvr_sb = [cp.tile([128, HID], BF16, name=f"wvr{j}") for j in range(NCH)]
            wot_sb = [cp.tile([128, HID], BF16, name=f"wot{j}") for j in range(NCH)]
            wrbd_sb = cp.tile([128, 128], BF16)
            ident_sb = cp.tile([128, 128], BF16)
            wbcol_sb = cp.tile([128, NCH], F32)
            if apply_bo:
                ones1_sb = cp.tile([1, 128], F32R)
                bo_sb = cp.tile([1, HID], F32R)
            if apply_affine:
                lng_sb = cp.tile([128, HID], BF16)
                lnb_sb = cp.tile([128, HID], BF16)

            EDT = BF16

            # -------- per-graph state (software-pipelined emission) --------
            st = {}

            def phase_load(g):
                s = {}
                s["xtb"] = bp.tile([128, NCH * L], BF16, name=f"xtb{g}", tag="xtb",
                                   bufs=5)
                xtb_src = bass.AP(
                    tensor=xtb.ap().tensor, offset=g * L,
                    ap=[[G * L, 128], [128 * G * L, NCH], [1, L]])
                nc.sync.dma_start(
                    out=s["xtb"].rearrange("p (i l) -> p i l", i=NCH), in_=xtb_src)
                if use_fp8:
                    s["xt8"] = bp.tile([128, NCH * L], FP8, name=f"xt8{g}",
                                       tag="xt8", bufs=3)
                    xt8_src = bass.AP(
                        tensor=xt8.ap().tensor, offset=g * L,
                        ap=[[G * L, 128], [128 * G * L, NCH], [1, L]])
                    nc.scalar.dma_start(
                        out=s["xt8"].rearrange("p (i l) -> p i l", i=NCH),
                        in_=xt8_src)
                    s["dxt8"] = bp.tile([128, NCH * L], FP8, name=f"dxt8{g}",
                                        tag="dxt8", bufs=3)
                    dxt8_src = bass.AP(
                        tensor=dxt8.ap().tensor, offset=g * L,
                        ap=[[G * L, 128], [128 * G * L, NCH], [1, L]])
                    nc.gpsimd.dma_start(
                        out=s["dxt8"].rearrange("p (i l) -> p i l", i=NCH),
                        in_=dxt8_src)
                st[g] = s

            def proj_half(g, w8_sb, w_sb, j, n0, pp):
                if use_fp8:
                    w8, dw8 = w8_sb
                    xt8_3d = st[g]["xt8"].rearrange("p (i l) -> p i l", i=NCH)
                    dxt8_3d = st[g]["dxt8"].rearrange("p (i l) -> p i l", i=NCH)
                    # psum = X8 @ W8 + X8 @ dW8 + dX8 @ W8  (~bf16 accuracy)
                    plan = [(w8, xt8_3d), (dw8, xt8_3d), (w8, dxt8_3d)]
                    nmm = len(plan) * 2
                    k = 0
                    for wsb, xsb in plan:
                        for p in range(2):
                            lhs = wsb[:, (2 * j + p) * 256:(2 * j + p + 1) * 256] \
                                .rearrange("p (two f) -> p two f", two=2)
                            rhs = xsb[:, 2 * p:2 * p + 2, n0:n0 + 512]
                            nc.tensor.matmul(pp, lhs, rhs, start=(k == 0),
                                             stop=(k == nmm - 1),
                                             perf_mode=PM.DoubleRow)
                            k += 1
                else:
                    for i in range(NCH):
                        nc.tensor.matmul(
                            pp, w_sb[i][:, 128 * j:128 * (j + 1)],
                            st[g]["xtb"][:, i * L + n0: i * L + n0 + 512],
                            start=(i == 0), stop=(i == NCH - 1))

            def proj_stage(g, tag, w8_sb, w_sb, scale_ap):
                """projection + exp + m for one of Q/K; then seg-sums + recip."""
                e_all = bp.tile([128, NCH * L], EDT, name=f"e{tag}{g}", tag="e",
                                bufs=3)
                m_tiles = []
                for j in range(NCH):
                    mt = sp.tile([128, L], BF16, name=f"m{tag}{g}{j}", tag="scr",
                                 bufs=8)
                    m_tiles.append(mt)
                for j in range(NCH):
                    for n0 in (0, 512):
                        pp = ppool.tile([128, 512], F32, name=f"pp{tag}{g}{j}{n0}",
                                        tag="pp", bufs=6)
                        proj_half(g, w8_sb, w_sb, j, n0, pp)
                        nc.scalar.activation(
                            out=e_all[:, j * L + n0: j * L + n0 + 512], in_=pp,
                            func=AT.Exp, scale=scale_ap[:, j:j + 1])
                        nc.vector.tensor_mul(
                            out=m_tiles[j][:, n0:n0 + 512],
                            in0=e_all[:, j * L + n0: j * L + n0 + 512], in1=pp)
                sos = []
                for hi, n0 in enumerate((0, 512)):
                    so = ppool.tile([16, 512], F32, name=f"so{tag}{g}{n0}",
                                    tag="rs", bufs=2, padded_shape=[128, 512])
                    for j in range(NCH):
                        nc.tensor.matmul(
                            so[0:8, :], segs_sb[:, 8 * j:8 * (j + 1)],
                            e_all[:, j * L + n0: j * L + n0 + 512],
                            start=(j == 0), stop=(j == NCH - 1))
                    sos.append(so)

                rt = sp.tile([8, 1024], F32, name=f"rt{tag}{g}", tag="rt", bufs=3)
                rtr = sp.tile([8, 1024], F32R, name=f"rtr{tag}{g}", tag="rtr", bufs=3)
                # per-half recip+round so rbc-lo can start before recip-hi lands
                nc.vector.reciprocal_approx_fast(out=rt[:, 0:512], in_=sos[0][0:8, :])
                nc.scalar.copy(out=rtr[:, 0:512], in_=rt[:, 0:512])
                nc.vector.reciprocal_approx_fast(out=rt[:, 512:1024], in_=sos[1][0:8, :])
                nc.scalar.copy(out=rtr[:, 512:1024], in_=rt[:, 512:1024])
                st[g][f"m{tag}"] = m_tiles
                st[g][f"rt{tag}"] = rtr

            def rbc_stage(g, tag):
                """rbc expand + stt accumulate; returns summed [128, NCH] tile."""
                rtr = st[g][f"rt{tag}"]
                m_tiles = st[g][f"m{tag}"]
                parts = sp.tile([128, 2 * NCH], F32, name=f"pts{tag}{g}",
                                tag=f"pts_{tag}")
                for j in range(NCH):
                    for hi, n0 in enumerate((0, 512)):
                        rbc = ppool.tile([128, 512], F32, name=f"rbc{tag}{g}{j}{n0}",
                                         tag="rs", bufs=2)
                        nc.tensor.matmul(rbc, sels_sb[:, 128 * j:128 * (j + 1)],
                                         rtr[:, n0:n0 + 512])
                        nc.vector.scalar_tensor_tensor(
                            out=m_tiles[j][:, n0:n0 + 512],
                            in0=m_tiles[j][:, n0:n0 + 512],
                            scalar=1.0 / SWV, in1=rbc,
                            op0=OP.mult, op1=OP.mult,
                            accum_out=parts[:, hi * NCH + j:hi * NCH + j + 1])
                tot = sp.tile([128, NCH], F32, name=f"tot{tag}{g}", tag=f"tot{tag}")
                nc.gpsimd.tensor_add(out=tot, in0=parts[:, 0:NCH],
                                     in1=parts[:, NCH:2 * NCH])
                return tot

            def phase_A(g):
                proj_stage(g, "a", (wq8_sb, dwq8_sb) if use_fp8 else None,
                           None if use_fp8 else wqt_sb, wacol_sb)

            def phase_rbcA(g):
                gq = rbc_stage(g, "a")
                gqwb = sp.tile([128, NCH], F32, name=f"gqwb{g}", tag="gqwb")
                nc.gpsimd.tensor_mul(out=gqwb, in0=gq, in1=wbcol_sb)
                st[g]["gq"] = gq
                st[g]["gqwb"] = gqwb

            def phase_K(g):
                proj_stage(g, "b", (wk8_sb, dwk8_sb) if use_fp8 else None,
                           None if use_fp8 else wkt_sb, st[g]["gqwb"])

            def phase_rbcB(g):
                acc = rbc_stage(g, "b")
                gk = sp.tile([128, NCH], F32, name=f"gk{g}", tag="gk")
                nc.gpsimd.tensor_mul(out=gk, in0=acc, in1=st[g]["gq"])
                gkwr = sp.tile([128, NCH * 128], BF16, name=f"gkwr{g}", tag="gkwr")
                for j in range(NCH):
                    nc.gpsimd.tensor_scalar_mul(
                        out=gkwr[:, 128 * j:128 * (j + 1)], in0=wrbd_sb,
                        scalar1=gk[:, j:j + 1])
                st[g]["gkwr"] = gkwr

            def prep_chunk(g, i):
                gkwr = st[g]["gkwr"]
                ppw = ppool.tile([128, 512], F32, name=f"ppw{g}{i}", tag="pp",
                                 bufs=6)
                for j in range(NCH):
                    nc.tensor.matmul(
                        ppw[:, 128 * j:128 * (j + 1)],
                        wvr_sb[j][:, 128 * i:128 * (i + 1)],
                        gkwr[:, 128 * j:128 * (j + 1)],
                        start=True, stop=False)
                    nc.tensor.matmul(
                        ppw[:, 128 * j:128 * (j + 1)], ident_sb,
                        wqt_sb[i][:, 128 * j:128 * (j + 1)],
                        start=False, stop=True)
                w3 = sp.tile([128, 512], BF16, name=f"w3{g}{i}", tag="w3",
                             bufs=8)
                nc.scalar.copy(out=w3, in_=ppw)
                st[g].setdefault("w3", []).append(w3)

            def phase_prep(g):
                for i in range(NCH):
                    prep_chunk(g, i)

            def phase_stream(g):
                w3_sb = st[g]["w3"]
                xtb_all = st[g]["xtb"]
                att_all = bp.tile([128, NCH * L], BF16, name=f"att{g}", tag="att",
                                  bufs=2)
                for j in range(NCH):
                    for n0 in (0, 512):
                        ppv = ppool.tile([128, 512], F32, name=f"ppv{g}{j}{n0}",
                                         tag="pp", bufs=6)
                        for i in range(NCH):
                            nc.tensor.matmul(
                                ppv, w3_sb[i][:, 128 * j:128 * (j + 1)],
                                xtb_all[:, i * L + n0: i * L + n0 + 512],
                                start=(i == 0), stop=(i == NCH - 1))
                        nc.scalar.activation(
                            out=att_all[:, j * L + n0: j * L + n0 + 512], in_=ppv,
                            func=AT.Relu)
                st[g]["att"] = att_all

            def ln_apply(g, obs, mv_all, rstd_all, t):
                och = sp.tile([128, HID], BF16, name=f"och{g}{t}", tag="och",
                              bufs=4)
                nc.gpsimd.tensor_scalar(
                    out=och, in0=obs[t], scalar1=mv_all[:, 2 * t:2 * t + 1],
                    scalar2=rstd_all[:, t:t + 1], op0=OP.subtract, op1=OP.mult)
                if apply_affine:
                    nc.vector.tensor_mul(out=och, in0=och, in1=lng_sb)
                    nc.vector.tensor_add(out=och, in0=och, in1=lnb_sb)
                nc.sync.dma_start(
                    out=outd.ap()[g * L + 128 * t: g * L + 128 * (t + 1), :],
                    in_=och)

            def phase_Wo(g, tail=False):
                att_all = st[g]["att"]
                mv_all = sp.tile([128, 2 * NT], F32, name=f"mv{g}", tag="mv")
                rstd_all = sp.tile([128, NT], F32, name=f"rstd{g}", tag="rstd")
                vf = sp.tile([128, NT], F32, name=f"vf{g}", tag="vf")
                lnv = sp.tile([128, NT], F32, name=f"lnv{g}", tag="lnv")
                obs = []
                for t in range(NT):
                    o_ps = ppool.tile([128, HID], F32, name=f"ops{g}{t}", tag="pp",
                                      bufs=6)
                    last = NCH - 1
                    for j in range(NCH):
                        nc.tensor.matmul(
                            o_ps, att_all[:, j * L + 128 * t: j * L + 128 * (t + 1)],
                            wot_sb[j], start=(j == 0),
                            stop=(j == last and not apply_bo))
                    if apply_bo:
                        nc.tensor.matmul(o_ps, ones1_sb, bo_sb, start=False,
                                         stop=True)
                    ob = sp.tile([128, HID], BF16, name=f"ob{g}{t}", tag="ob",
                                 bufs=NT + 2)
                    nc.scalar.copy(out=ob, in_=o_ps)
                    stats = sp.tile([128, 6], F32, name=f"sst{g}{t}", tag="sst")
                    nc.vector.bn_stats(out=stats, in_=ob)
                    nc.vector.bn_aggr(out=mv_all[:, 2 * t:2 * t + 2], in_=stats)
                    obs.append(ob)
                    if tail:
                        nc.vector.tensor_scalar_add(
                            out=vf[:, t:t + 1], in0=mv_all[:, 2 * t + 1:2 * t + 2],
                            scalar1=EPS)
                        nc.scalar.activation(out=lnv[:, t:t + 1], in_=vf[:, t:t + 1],
                                             func=AT.Ln)
                        nc.scalar.activation(out=rstd_all[:, t:t + 1],
                                             in_=lnv[:, t:t + 1], func=AT.Exp,
                                             scale=-0.5)
                        ln_apply(g, obs, mv_all, rstd_all, t)
                if not tail:
                    nc.gpsimd.tensor_scalar_add(out=vf, in0=mv_all[:, 1:2 * NT:2],
                                                scalar1=EPS)
                    nc.scalar.activation(out=lnv, in_=vf, func=AT.Ln)
                    nc.scalar.activation(out=rstd_all, in_=lnv, func=AT.Exp, scale=-0.5)
                    for t in range(NT):
                        ln_apply(g, obs, mv_all, rstd_all, t)
                del st[g]

            # -------- modulo schedule (A shifted one slot early) --------
            # per iter g: A(g+1) fills sttA(g); stream/Wo(g-1) fill sttB(g);
            # rbcA(g+1) at iter end once recipA(g+1) is ready.
            phase_load(0)
            phase_load(1)
            load_bulk_consts()
            phase_load(2)
            phase_A(0)
            phase_rbcA(0)
            for g in range(G):
                if g + 3 < G:
                    phase_load(g + 3)
                if g == 0:
                    phase_A(1)
                elif g == 1:
                    # A(2) was pulled into iter 0; stream(0) fills sttA(1)
                    phase_stream(0)
                elif g + 1 < G:
                    phase_A(g + 1)
                else:
                    # last iteration: stream(g-1) covers the sttA(g) chain
                    phase_stream(g - 1)
                phase_K(g)
                phase_rbcB(g)
                if g == 0:
                    phase_A(2)
                else:
                    if g + 1 < G and g != 1:
                        phase_stream(g - 1)
                    phase_Wo(g - 1)
                phase_prep(g)
                if g + 1 < G:
                    phase_rbcA(g + 1)
            phase_stream(G - 1)
            phase_Wo(G - 1, tail=True)

    _bacc_mod.get_activation_tables = _gat
    try:
        nc.compile()
    finally:
        _bacc_mod.get_activation_tables = _orig_gat
    return nc


_NC_CACHE = {}


def _get_nc(apply_bo, apply_affine):
    key = (apply_bo, apply_affine, USE_FP8)
    if key not in _NC_CACHE:
        _NC_CACHE[key] = _build(apply_bo, apply_affine, USE_FP8)
    return _NC_CACHE[key]


def _host_consts(Wq, Wk, Wv, Wr, w_alpha, w_beta, Wo, bo, ln_g, ln_b):
    import ml_dtypes
    bf = ml_dtypes.bfloat16
    f8 = ml_dtypes.float8_e4m3fn

    wqt = np.ascontiguousarray(Wq.T)                       # [h, e]
    wvr = np.ascontiguousarray(Wv)                         # [d, h]
    wot = np.ascontiguousarray(Wo.T)
    wrt = Wr.T.astype(np.float32)                          # WrT[d, e] = Wr[e, d]
    wrbd = np.zeros((128, 128), np.float32)
    wrbd[:64, :64] = wrt; wrbd[64:, 64:] = wrt
    ident = np.eye(128, dtype=np.float32)
    wa_vec = np.tile(w_alpha, HEADS) * SCALE               # [512]
    wb_vec = np.tile(w_beta, HEADS) * SCALE
    SWV = SW if USE_FP8 else 1.0
    wacol = (wa_vec / SWV).reshape(NCH, 128).T.copy()      # [128, NCH]
    wbcol = (wb_vec / SWV).reshape(NCH, 128).T.copy()

    segs = np.zeros((128, 8 * NCH), np.float32)
    sels = np.zeros((8, HID), np.float32)
    for j in range(NCH):
        for p in range(128):
            segs[p, 8 * j + 2 * j + p // 64] = 1.0
        for m in range(HID):
            if m // 128 == j:
                sels[2 * j + (m % 128) // 64, m] = 1.0

    common = {"wqt": wqt.astype(bf), "wvr": wvr.astype(bf),
              "wot": wot.astype(bf), "wrbd": wrbd.astype(bf),
              "ident": ident.astype(bf), "segs": segs.astype(bf),
              "sels": sels, "wacol": wacol.astype(np.float32),
              "wbcol": wbcol.astype(np.float32)}

    if USE_FP8:
        def pack_dr(WT):   # WT [h, e] -> [128, NCH*2*256] DoubleRow stationary
            out = np.zeros((128, NCH * 2 * 256), np.float32)
            for j in range(NCH):
                for p in range(2):
                    blkA = WT[256 * p:256 * p + 128, 128 * j:128 * (j + 1)]
                    blkB = WT[256 * p + 128:256 * (p + 1), 128 * j:128 * (j + 1)]
                    c0 = (2 * j + p) * 256
                    out[:, c0:c0 + 128] = blkA
                    out[:, c0 + 128:c0 + 256] = blkB
            return out
        wq_pk = pack_dr(SW * Wq.T)
        wk_pk = pack_dr(SW * Wk.T)
        wq8 = wq_pk.astype(f8)
        wk8 = wk_pk.astype(f8)
        common["wq8"] = wq8
        common["wk8"] = wk8
        common["dwq8"] = (wq_pk - wq8.astype(np.float32)).astype(f8)
        common["dwk8"] = (wk_pk - wk8.astype(np.float32)).astype(f8)
    else:
        common["wkt"] = np.ascontiguousarray(Wk.T).astype(bf)

    apply_bo = not np.allclose(bo, 0.0)
    apply_affine = not (np.allclose(ln_g, 1.0) and np.allclose(ln_b, 0.0))
    if apply_bo:
        common["bo"] = bo.reshape(1, HID).astype(np.float32)
        common["ones1"] = np.ones((1, 128), np.float32)
    if apply_affine:
        common["ln_g"] = np.tile(ln_g, (128, 1)).astype(bf)
        common["ln_b"] = np.tile(ln_b, (128, 1)).astype(bf)
    return common, apply_bo, apply_affine


def kernel(edge_attr, batch_scopes, Wq, Wk, Wv, Wr, w_alpha, w_beta, Wo, bo,
           ln_g, ln_b):
    from concourse import bass_utils
    import ml_dtypes

    edge_attr = np.asarray(edge_attr, dtype=np.float32)
    scopes = np.asarray(batch_scopes)
    Wq = np.asarray(Wq, np.float32); Wk = np.asarray(Wk, np.float32)
    Wv = np.asarray(Wv, np.float32); Wr = np.asarray(Wr, np.float32)
    Wo = np.asarray(Wo, np.float32)
    w_alpha = np.asarray(w_alpha, np.float32); w_beta = np.asarray(w_beta, np.float32)
    bo = np.asarray(bo, np.float32)
    ln_g = np.asarray(ln_g, np.float32); ln_b = np.asarray(ln_b, np.float32)

    assert np.all(scopes[:, 1] == L), "equal-length contiguous scopes expected"
    starts = scopes[:, 0].astype(np.int64)

    common, apply_bo, apply_affine = _host_consts(
        Wq, Wk, Wv, Wr, w_alpha, w_beta, Wo, bo, ln_g, ln_b)
    nc = _get_nc(apply_bo, apply_affine)

    bf = ml_dtypes.bfloat16
    f8 = ml_dtypes.float8_e4m3fn
    in_maps = []
    for c in range(NCORES):
        rows = np.concatenate([
            np.arange(starts[c * G + g], starts[c * G + g] + L)
            for g in range(G)])
        xslab = edge_attr[rows]                       # [G*L, 512]
        xt = np.ascontiguousarray(xslab.T)
        m = {"xtb": xt.astype(bf), **common}
        if USE_FP8:
            x8 = xt.astype(f8)
            m["xt8"] = x8
            m["dxt8"] = (xt - x8.astype(np.float32)).astype(f8)
        in_maps.append(m)

    res = bass_utils.run_bass_kernel_spmd(nc, in_maps, core_ids=list(range(NCORES)))
    out = np.concatenate([r["out"] for r in res.results], axis=0)
    return out.astype(np.float32)


# revision 44
# speedup vs baseline: 1.2028x; 1.0022x over previous
"""BondFastAttention Trainium2 kernel (self-contained), v2.

Shapes (hardcoded from the problem spec):
  edge_attr [65536, 512] fp32, B=64 graphs x L=1024 bonds, HID=512, 8 heads x D=64.
  8 NeuronCores, data-parallel over graphs: G=8 graphs per core.

Device layout: transposed domain - features on partitions, tokens on free dim
for Q/K/kvout; tokens on partitions for the Wo/LayerNorm stage.

Key structure vs v1:
  - The V projection, Wr matmul, gk scaling and +q add are all folded into a
    single per-graph combined weight W''' = Wv^T (gk . Wr^T) + Wq^T, built on
    the PE (16 small matmuls + identity-add), so one X-stream produces
    relu-input directly.
  - Projection PSUM is consumed in place (ACT exp, Pool multiply) - no
    psum->sbuf copies for q/k/v.
  - Optional fp8 path: Q/K projections and softmax seg-sums run as fp8
    DoubleRow matmuls (K=256 per pass).
  - Output is written bf16 and upcast to f32 on the host.
"""
import numpy as np

HID = 512
HEADS = 8
D = 64
B = 64
L = 1024
SCALE = D ** -0.5
EPS = 1e-5
NCORES = 8
G = B // NCORES          # graphs per core
NCH = HID // 128         # 4 feature chunks (2 heads each)
NT = L // 128            # 8 token chunks
SW = 16.0                # fp8 weight prescale

USE_FP8 = True


def _build(apply_bo: bool, apply_affine: bool, use_fp8: bool):
    import concourse.bass as bass
    from concourse import bacc
    import concourse.mybir as mybir
    from concourse.tile import TileContext

    F32 = mybir.dt.float32
    F32R = mybir.dt.float32r
    BF16 = mybir.dt.bfloat16
    FP8 = mybir.dt.float8e4
    AT = mybir.ActivationFunctionType
    OP = mybir.AluOpType
    PM = mybir.MatmulPerfMode

    nc = bacc.Bacc()

    import concourse.bacc as _bacc_mod
    _orig_gat = _bacc_mod.get_activation_tables

    def _gat(arch):
        # Keep dict order but strip our funcs from every other set, so the
        # table-load pass assigns all of them to natural_log_exp_and_others
        # -> a single physical table load.
        t = _orig_gat(arch)
        ours = {AT.Exp, AT.Ln, AT.Copy, AT.Relu, AT.Identity}
        out = {}
        for k, funcs in t.items():
            if k == "natural_log_exp_and_others":
                out[k] = funcs
            else:
                out[k] = {f for f in funcs if f not in ours}
        return out

    # ---------------- dram tensors ----------------
    xtb = nc.dram_tensor("xtb", [HID, G * L], BF16, kind="ExternalInput")
    if use_fp8:
        xt8 = nc.dram_tensor("xt8", [HID, G * L], FP8, kind="ExternalInput")
        dxt8 = nc.dram_tensor("dxt8", [HID, G * L], FP8, kind="ExternalInput")
        wq8d = nc.dram_tensor("wq8", [128, NCH * 2 * 256], FP8, kind="ExternalInput")
        wk8d = nc.dram_tensor("wk8", [128, NCH * 2 * 256], FP8, kind="ExternalInput")
        dwq8d = nc.dram_tensor("dwq8", [128, NCH * 2 * 256], FP8, kind="ExternalInput")
        dwk8d = nc.dram_tensor("dwk8", [128, NCH * 2 * 256], FP8, kind="ExternalInput")
    else:
        wktd = nc.dram_tensor("wkt", [HID, HID], BF16, kind="ExternalInput")
    wqtd = nc.dram_tensor("wqt", [HID, HID], BF16, kind="ExternalInput")
    wvrd = nc.dram_tensor("wvr", [HID, HID], BF16, kind="ExternalInput")
    wotd = nc.dram_tensor("wot", [HID, HID], BF16, kind="ExternalInput")
    wrbdd = nc.dram_tensor("wrbd", [128, 128], BF16, kind="ExternalInput")
    identd = nc.dram_tensor("ident", [128, 128], BF16, kind="ExternalInput")
    segsd = nc.dram_tensor("segs", [128, 8 * NCH], BF16, kind="ExternalInput")
    selsd = nc.dram_tensor("sels", [8, HID], F32, kind="ExternalInput")
    wacold = nc.dram_tensor("wacol", [128, NCH], F32, kind="ExternalInput")
    wbcold = nc.dram_tensor("wbcol", [128, NCH], F32, kind="ExternalInput")
    if apply_bo:
        bod = nc.dram_tensor("bo", [1, HID], F32, kind="ExternalInput")
        onesd = nc.dram_tensor("ones1", [1, 128], F32, kind="ExternalInput")
    if apply_affine:
        lngd = nc.dram_tensor("ln_g", [128, HID], BF16, kind="ExternalInput")
        lnbd = nc.dram_tensor("ln_b", [128, HID], BF16, kind="ExternalInput")
    outd = nc.dram_tensor("out", [G * L, HID], BF16, kind="ExternalOutput")

    SWV = SW if use_fp8 else 1.0

    with TileContext(nc) as tc:
        with tc.tile_pool(name="consts", bufs=1) as cp, \
             tc.tile_pool(name="big", bufs=1) as bp, \
             tc.tile_pool(name="small", bufs=2) as sp, \
             tc.tile_pool(name="psum", bufs=1, space="PSUM") as ppool:

            # ---- constants to SBUF ----
            # Phase-0-critical consts first; bulk consts go AFTER the first
            # graph's X loads are queued, spread across SP/ACT/Pool DGEs.
            wacol_sb = cp.tile([128, NCH], F32)
            if use_fp8:
                wq8_sb = cp.tile([128, NCH * 2 * 256], FP8)
                nc.sync.dma_start(out=wq8_sb, in_=wq8d.ap())
            nc.sync.dma_start(out=wacol_sb, in_=wacold.ap())
            if use_fp8:
                dwq8_sb = cp.tile([128, NCH * 2 * 256], FP8)
                nc.gpsimd.dma_start(out=dwq8_sb, in_=dwq8d.ap())
                wk8_sb = cp.tile([128, NCH * 2 * 256], FP8)
                dwk8_sb = cp.tile([128, NCH * 2 * 256], FP8)
            else:
                wkt_sb = [cp.tile([128, HID], BF16, name=f"wkt{i}") for i in range(NCH)]
            wqt_sb = [cp.tile([128, HID], BF16, name=f"wqt{i}") for i in range(NCH)]
            segs_sb = cp.tile([128, 8 * NCH], BF16)
            sels_sb = cp.tile([8, HID], F32R)

            def load_bulk_consts():
                nc.scalar.dma_start(out=segs_sb, in_=segsd.ap())
                nc.sync.dma_start(out=sels_sb, in_=selsd.ap().bitcast(F32R))
                if use_fp8:
                    nc.scalar.dma_start(out=wk8_sb, in_=wk8d.ap())
                    nc.scalar.dma_start(out=dwk8_sb, in_=dwk8d.ap())
                for i in range(NCH):
                    nc.sync.dma_start(out=wqt_sb[i],
                                      in_=wqtd.ap()[128 * i:128 * (i + 1), :])
                if not use_fp8:
                    for i in range(NCH):
                        nc.scalar.dma_start(
                            out=wkt_sb[i], in_=wktd.ap()[128 * i:128 * (i + 1), :])
                for j in range(NCH):
                    nc.gpsimd.dma_start(out=wvr_sb[j],
                                        in_=wvrd.ap()[128 * j:128 * (j + 1), :])
                for j in range(NCH):
                    nc.gpsimd.dma_start(out=wot_sb[j],
                                        in_=wotd.ap()[128 * j:128 * (j + 1), :])
                nc.scalar.dma_start(out=wrbd_sb, in_=wrbdd.ap())
                nc.scalar.dma_start(out=ident_sb, in_=identd.ap())
                nc.scalar.dma_start(out=wbcol_sb, in_=wbcold.ap())
                if apply_bo:
                    nc.scalar.dma_start(out=ones1_sb, in_=onesd.ap().bitcast(F32R))
                    nc.scalar.dma_start(out=bo_sb, in_=bod.ap().bitcast(F32R))
                if apply_affine:
                    nc.gpsimd.dma_start(out=lng_sb, in_=lngd.ap())
                    nc.gpsimd.dma_start(out=lnb_sb, in_=lnbd.ap())

            wvr_sb = [cp.tile([128, HID], BF16, name=f"wvr{j}") for j in range(NCH)]
            wot_sb = [cp.tile([128, HID], BF16, name=f"wot{j}") for j in range(NCH)]
            wrbd_sb = cp.tile([128, 128], BF16)
            ident_sb = cp.tile([128, 128], BF16)
            wbcol_sb = cp.tile([128, NCH], F32)
            if apply_bo:
                ones1_sb = cp.tile([1, 128], F32R)
                bo_sb = cp.tile([1, HID], F32R)
            if apply_affine:
                lng_sb = cp.tile([128, HID], BF16)
                lnb_sb = cp.tile([128, HID], BF16)

            EDT = BF16

            # -------- per-graph state (software-pipelined emission) --------
            st = {}

            def phase_load(g):
                s = {}
                s["xtb"] = bp.tile([128, NCH * L], BF16, name=f"xtb{g}", tag="xtb",
                                   bufs=5)
                xtb_src = bass.AP(
                    tensor=xtb.ap().tensor, offset=g * L,
                    ap=[[G * L, 128], [128 * G * L, NCH], [1, L]])
                nc.sync.dma_start(
                    out=s["xtb"].rearrange("p (i l) -> p i l", i=NCH), in_=xtb_src)
                if use_fp8:
                    s["xt8"] = bp.tile([128, NCH * L], FP8, name=f"xt8{g}",
                                       tag="xt8", bufs=3)
                    xt8_src = bass.AP(
                        tensor=xt8.ap().tensor, offset=g * L,
                        ap=[[G * L, 128], [128 * G * L, NCH], [1, L]])
                    nc.scalar.dma_start(
                        out=s["xt8"].rearrange("p (i l) -> p i l", i=NCH),
                        in_=xt8_src)
                    s["dxt8"] = bp.tile([128, NCH * L], FP8, name=f"dxt8{g}",
                                        tag="dxt8", bufs=3)
                    dxt8_src = bass.AP(
                        tensor=dxt8.ap().tensor, offset=g * L,
                        ap=[[G * L, 128], [128 * G * L, NCH], [1, L]])
                    nc.gpsimd.dma_start(
                        out=s["dxt8"].rearrange("p (i l) -> p i l", i=NCH),
                        in_=dxt8_src)
                st[g] = s

            def proj_half(g, w8_sb, w_sb, j, n0, pp):
                if use_fp8:
                    w8, dw8 = w8_sb
                    xt8_3d = st[g]["xt8"].rearrange("p (i l) -> p i l", i=NCH)
                    dxt8_3d = st[g]["dxt8"].rearrange("p (i l) -> p i l", i=NCH)
                    # psum = X8 @ W8 + X8 @ dW8 + dX8 @ W8  (~bf16 accuracy)
                    plan = [(w8, xt8_3d), (dw8, xt8_3d), (w8, dxt8_3d)]
                    nmm = len(plan) * 2
                    k = 0
                    for wsb, xsb in plan:
                        for p in range(2):
                            lhs = wsb[:, (2 * j + p) * 256:(2 * j + p + 1) * 256] \
                                .rearrange("p (two f) -> p two f", two=2)
                            rhs = xsb[:, 2 * p:2 * p + 2, n0:n0 + 512]
                            nc.tensor.matmul(pp, lhs, rhs, start=(k == 0),
                                             stop=(k == nmm - 1),
                                             perf_mode=PM.DoubleRow)
                            k += 1
                else:
                    for i in range(NCH):
                        nc.tensor.matmul(
                            pp, w_sb[i][:, 128 * j:128 * (j + 1)],
                            st[g]["xtb"][:, i * L + n0: i * L + n0 + 512],
                            start=(i == 0), stop=(i == NCH - 1))

            def proj_stage(g, tag, w8_sb, w_sb, scale_ap):
                """projection + exp + m for one of Q/K; then seg-sums + recip."""
                e_all = bp.tile([128, NCH * L], EDT, name=f"e{tag}{g}", tag="e",
                                bufs=3)
                m_tiles = []
                for j in range(NCH):
                    mt = sp.tile([128, L], BF16, name=f"m{tag}{g}{j}", tag="scr",
                                 bufs=10)
                    m_tiles.append(mt)
                for j in range(NCH):
                    for n0 in (0, 512):
                        pp = ppool.tile([128, 512], F32, name=f"pp{tag}{g}{j}{n0}",
                                        tag="pp", bufs=6)
                        proj_half(g, w8_sb, w_sb, j, n0, pp)
                        nc.scalar.activation(
                            out=e_all[:, j * L + n0: j * L + n0 + 512], in_=pp,
                            func=AT.Exp, scale=scale_ap[:, j:j + 1])
                        nc.vector.tensor_mul(
                            out=m_tiles[j][:, n0:n0 + 512],
                            in0=e_all[:, j * L + n0: j * L + n0 + 512], in1=pp)
                sos = []
                for hi, n0 in enumerate((0, 512)):
                    so = ppool.tile([16, 512], F32, name=f"so{tag}{g}{n0}",
                                    tag="rs", bufs=2, padded_shape=[128, 512])
                    for j in range(NCH):
                        nc.tensor.matmul(
                            so[0:8, :], segs_sb[:, 8 * j:8 * (j + 1)],
                            e_all[:, j * L + n0: j * L + n0 + 512],
                            start=(j == 0), stop=(j == NCH - 1))
                    sos.append(so)

                rt = sp.tile([8, 1024], F32, name=f"rt{tag}{g}", tag="rt", bufs=3)
                rtr = sp.tile([8, 1024], F32R, name=f"rtr{tag}{g}", tag="rtr", bufs=3)
                # per-half recip+round so rbc-lo can start before recip-hi lands
                nc.vector.reciprocal_approx_fast(out=rt[:, 0:512], in_=sos[0][0:8, :])
                nc.scalar.copy(out=rtr[:, 0:512], in_=rt[:, 0:512])
                nc.vector.reciprocal_approx_fast(out=rt[:, 512:1024], in_=sos[1][0:8, :])
                nc.scalar.copy(out=rtr[:, 512:1024], in_=rt[:, 512:1024])
                st[g][f"m{tag}"] = m_tiles
                st[g][f"rt{tag}"] = rtr

            def rbc_stage(g, tag):
                """rbc expand + stt accumulate; returns summed [128, NCH] tile."""
                rtr = st[g][f"rt{tag}"]
                m_tiles = st[g][f"m{tag}"]
                parts = sp.tile([128, 2 * NCH], F32, name=f"pts{tag}{g}",
                                tag=f"pts_{tag}")
                for j in range(NCH):
                    for hi, n0 in enumerate((0, 512)):
                        rbc = ppool.tile([128, 512], F32, name=f"rbc{tag}{g}{j}{n0}",
                                         tag="rs", bufs=2)
                        nc.tensor.matmul(rbc, sels_sb[:, 128 * j:128 * (j + 1)],
                                         rtr[:, n0:n0 + 512])
                        nc.vector.scalar_tensor_tensor(
                            out=m_tiles[j][:, n0:n0 + 512],
                            in0=m_tiles[j][:, n0:n0 + 512],
                            scalar=1.0 / SWV, in1=rbc,
                            op0=OP.mult, op1=OP.mult,
                            accum_out=parts[:, hi * NCH + j:hi * NCH + j + 1])
                tot = sp.tile([128, NCH], F32, name=f"tot{tag}{g}", tag=f"tot{tag}")
                nc.gpsimd.tensor_add(out=tot, in0=parts[:, 0:NCH],
                                     in1=parts[:, NCH:2 * NCH])
                return tot

            def phase_A(g):
                proj_stage(g, "a", (wq8_sb, dwq8_sb) if use_fp8 else None,
                           None if use_fp8 else wqt_sb, wacol_sb)

            def phase_rbcA(g):
                gq = rbc_stage(g, "a")
                gqwb = sp.tile([128, NCH], F32, name=f"gqwb{g}", tag="gqwb")
                nc.gpsimd.tensor_mul(out=gqwb, in0=gq, in1=wbcol_sb)
                st[g]["gq"] = gq
                st[g]["gqwb"] = gqwb

            def phase_K(g):
                proj_stage(g, "b", (wk8_sb, dwk8_sb) if use_fp8 else None,
                           None if use_fp8 else wkt_sb, st[g]["gqwb"])

            def phase_rbcB(g):
                acc = rbc_stage(g, "b")
                gk = sp.tile([128, NCH], F32, name=f"gk{g}", tag="gk")
                nc.gpsimd.tensor_mul(out=gk, in0=acc, in1=st[g]["gq"])
                gkwr = sp.tile([128, NCH * 128], BF16, name=f"gkwr{g}", tag="gkwr")
                for j in range(NCH):
                    nc.gpsimd.tensor_scalar_mul(
                        out=gkwr[:, 128 * j:128 * (j + 1)], in0=wrbd_sb,
                        scalar1=gk[:, j:j + 1])
                st[g]["gkwr"] = gkwr

            def prep_chunk(g, i):
                gkwr = st[g]["gkwr"]
                ppw = ppool.tile([128, 512], F32, name=f"ppw{g}{i}", tag="pp",
                                 bufs=6)
                for j in range(NCH):
                    nc.tensor.matmul(
                        ppw[:, 128 * j:128 * (j + 1)],
                        wvr_sb[j][:, 128 * i:128 * (i + 1)],
                        gkwr[:, 128 * j:128 * (j + 1)],
                        start=True, stop=False)
                    nc.tensor.matmul(
                        ppw[:, 128 * j:128 * (j + 1)], ident_sb,
                        wqt_sb[i][:, 128 * j:128 * (j + 1)],
                        start=False, stop=True)
                w3 = sp.tile([128, 512], BF16, name=f"w3{g}{i}", tag="w3",
                             bufs=8)
                nc.scalar.copy(out=w3, in_=ppw)
                st[g].setdefault("w3", []).append(w3)

            def phase_prep(g):
                for i in range(NCH):
                    prep_chunk(g, i)

            def phase_stream(g):
                w3_sb = st[g]["w3"]
                xtb_all = st[g]["xtb"]
                att_all = bp.tile([128, NCH * L], BF16, name=f"att{g}", tag="att",
                                  bufs=2)
                for j in range(NCH):
                    for n0 in (0, 512):
                        ppv = ppool.tile([128, 512], F32, name=f"ppv{g}{j}{n0}",
                                         tag="pp", bufs=6)
                        for i in range(NCH):
                            nc.tensor.matmul(
                                ppv, w3_sb[i][:, 128 * j:128 * (j + 1)],
                                xtb_all[:, i * L + n0: i * L + n0 + 512],
                                start=(i == 0), stop=(i == NCH - 1))
                        nc.scalar.activation(
                            out=att_all[:, j * L + n0: j * L + n0 + 512], in_=ppv,
                            func=AT.Relu)
                st[g]["att"] = att_all

            def ln_apply(g, obs, mv_all, rstd_all, t):
                och = sp.tile([128, HID], BF16, name=f"och{g}{t}", tag="och",
                              bufs=4)
                nc.gpsimd.tensor_scalar(
                    out=och, in0=obs[t], scalar1=mv_all[:, 2 * t:2 * t + 1],
                    scalar2=rstd_all[:, t:t + 1], op0=OP.subtract, op1=OP.mult)
                if apply_affine:
                    nc.vector.tensor_mul(out=och, in0=och, in1=lng_sb)
                    nc.vector.tensor_add(out=och, in0=och, in1=lnb_sb)
                nc.sync.dma_start(
                    out=outd.ap()[g * L + 128 * t: g * L + 128 * (t + 1), :],
                    in_=och)

            def phase_Wo(g, tail=False):
                att_all = st[g]["att"]
                mv_all = sp.tile([128, 2 * NT], F32, name=f"mv{g}", tag="mv")
                rstd_all = sp.tile([128, NT], F32, name=f"rstd{g}", tag="rstd")
                vf = sp.tile([128, NT], F32, name=f"vf{g}", tag="vf")
                lnv = sp.tile([128, NT], F32, name=f"lnv{g}", tag="lnv")
                obs = []
                for t in range(NT):
                    o_ps = ppool.tile([128, HID], F32, name=f"ops{g}{t}", tag="pp",
                                      bufs=6)
                    last = NCH - 1
                    for j in range(NCH):
                        nc.tensor.matmul(
                            o_ps, att_all[:, j * L + 128 * t: j * L + 128 * (t + 1)],
                            wot_sb[j], start=(j == 0),
                            stop=(j == last and not apply_bo))
                    if apply_bo:
                        nc.tensor.matmul(o_ps, ones1_sb, bo_sb, start=False,
                                         stop=True)
                    ob = sp.tile([128, HID], BF16, name=f"ob{g}{t}", tag="ob",
                                 bufs=NT + 2)
                    nc.scalar.copy(out=ob, in_=o_ps)
                    stats = sp.tile([128, 6], F32, name=f"sst{g}{t}", tag="sst")
                    nc.vector.bn_stats(out=stats, in_=ob)
                    nc.vector.bn_aggr(out=mv_all[:, 2 * t:2 * t + 2], in_=stats)
                    obs.append(ob)
                    if tail:
                        nc.vector.tensor_scalar_add(
                            out=vf[:, t:t + 1], in0=mv_all[:, 2 * t + 1:2 * t + 2],
                            scalar1=EPS)
                        nc.scalar.activation(out=lnv[:, t:t + 1], in_=vf[:, t:t + 1],
                                             func=AT.Ln)
                        nc.scalar.activation(out=rstd_all[:, t:t + 1],
                                             in_=lnv[:, t:t + 1], func=AT.Exp,
                                             scale=-0.5)
                        ln_apply(g, obs, mv_all, rstd_all, t)
                if not tail:
                    nc.gpsimd.tensor_scalar_add(out=vf, in0=mv_all[:, 1:2 * NT:2],
                                                scalar1=EPS)
                    nc.scalar.activation(out=lnv, in_=vf, func=AT.Ln)
                    nc.scalar.activation(out=rstd_all, in_=lnv, func=AT.Exp, scale=-0.5)
                    for t in range(NT):
                        ln_apply(g, obs, mv_all, rstd_all, t)
                del st[g]

            # -------- modulo schedule (A shifted one slot early) --------
            # per iter g: A(g+1) fills sttA(g); stream/Wo(g-1) fill sttB(g);
            # rbcA(g+1) at iter end once recipA(g+1) is ready.
            phase_load(0)
            phase_load(1)
            load_bulk_consts()
            phase_load(2)
            phase_A(0)
            phase_rbcA(0)
            for g in range(G):
                if g + 3 < G:
                    phase_load(g + 3)
                if g == 0:
                    phase_A(1)
                elif g == 1:
                    # A(2) was pulled into iter 0; stream(0) fills sttA(1)
                    phase_stream(0)
                elif g + 1 < G:
                    phase_A(g + 1)
                else:
                    # last iteration: stream(g-1) covers the sttA(g) chain
                    phase_stream(g - 1)
                phase_K(g)
                phase_rbcB(g)
                if g == 0:
                    phase_A(2)
                else:
                    if g + 1 < G and g != 1:
                        phase_stream(g - 1)
                    phase_Wo(g - 1)
                phase_prep(g)
                if g + 1 < G:
                    phase_rbcA(g + 1)
            phase_stream(G - 1)
            phase_Wo(G - 1, tail=True)

    _bacc_mod.get_activation_tables = _gat
    try:
        nc.compile()
    finally:
        _bacc_mod.get_activation_tables = _orig_gat
    return nc


_NC_CACHE = {}


def _get_nc(apply_bo, apply_affine):
    key = (apply_bo, apply_affine, USE_FP8)
    if key not in _NC_CACHE:
        _NC_CACHE[key] = _build(apply_bo, apply_affine, USE_FP8)
    return _NC_CACHE[key]


def _host_consts(Wq, Wk, Wv, Wr, w_alpha, w_beta, Wo, bo, ln_g, ln_b):
    import ml_dtypes
    bf = ml_dtypes.bfloat16
    f8 = ml_dtypes.float8_e4m3fn

    wqt = np.ascontiguousarray(Wq.T)                       # [h, e]
    wvr = np.ascontiguousarray(Wv)                         # [d, h]
    wot = np.ascontiguousarray(Wo.T)
    wrt = Wr.T.astype(np.float32)                          # WrT[d, e] = Wr[e, d]
    wrbd = np.zeros((128, 128), np.float32)
    wrbd[:64, :64] = wrt; wrbd[64:, 64:] = wrt
    ident = np.eye(128, dtype=np.float32)
    wa_vec = np.tile(w_alpha, HEADS) * SCALE               # [512]
    wb_vec = np.tile(w_beta, HEADS) * SCALE
    SWV = SW if USE_FP8 else 1.0
    wacol = (wa_vec / SWV).reshape(NCH, 128).T.copy()      # [128, NCH]
    wbcol = (wb_vec / SWV).reshape(NCH, 128).T.copy()

    segs = np.zeros((128, 8 * NCH), np.float32)
    sels = np.zeros((8, HID), np.float32)
    for j in range(NCH):
        for p in range(128):
            segs[p, 8 * j + 2 * j + p // 64] = 1.0
        for m in range(HID):
            if m // 128 == j:
                sels[2 * j + (m % 128) // 64, m] = 1.0

    common = {"wqt": wqt.astype(bf), "wvr": wvr.astype(bf),
              "wot": wot.astype(bf), "wrbd": wrbd.astype(bf),
              "ident": ident.astype(bf), "segs": segs.astype(bf),
              "sels": sels, "wacol": wacol.astype(np.float32),
              "wbcol": wbcol.astype(np.float32)}

    if USE_FP8:
        def pack_dr(WT):   # WT [h, e] -> [128, NCH*2*256] DoubleRow stationary
            out = np.zeros((128, NCH * 2 * 256), np.float32)
            for j in range(NCH):
                for p in range(2):
                    blkA = WT[256 * p:256 * p + 128, 128 * j:128 * (j + 1)]
                    blkB = WT[256 * p + 128:256 * (p + 1), 128 * j:128 * (j + 1)]
                    c0 = (2 * j + p) * 256
                    out[:, c0:c0 + 128] = blkA
                    out[:, c0 + 128:c0 + 256] = blkB
            return out
        wq_pk = pack_dr(SW * Wq.T)
        wk_pk = pack_dr(SW * Wk.T)
        wq8 = wq_pk.astype(f8)
        wk8 = wk_pk.astype(f8)
        common["wq8"] = wq8
        common["wk8"] = wk8
        common["dwq8"] = (wq_pk - wq8.astype(np.float32)).astype(f8)
        common["dwk8"] = (wk_pk - wk8.astype(np.float32)).astype(f8)
    else:
        common["wkt"] = np.ascontiguousarray(Wk.T).astype(bf)

    apply_bo = not np.allclose(bo, 0.0)
    apply_affine = not (np.allclose(ln_g, 1.0) and np.allclose(ln_b, 0.0))
    if apply_bo:
        common["bo"] = bo.reshape(1, HID).astype(np.float32)
        common["ones1"] = np.ones((1, 128), np.float32)
    if apply_affine:
        common["ln_g"] = np.tile(ln_g, (128, 1)).astype(bf)
        common["ln_b"] = np.tile(ln_b, (128, 1)).astype(bf)
    return common, apply_bo, apply_affine


def kernel(edge_attr, batch_scopes, Wq, Wk, Wv, Wr, w_alpha, w_beta, Wo, bo,
           ln_g, ln_b):
    from concourse import bass_utils
    import ml_dtypes

    edge_attr = np.asarray(edge_attr, dtype=np.float32)
    scopes = np.asarray(batch_scopes)
    Wq = np.asarray(Wq, np.float32); Wk = np.asarray(Wk, np.float32)
    Wv = np.asarray(Wv, np.float32); Wr = np.asarray(Wr, np.float32)
    Wo = np.asarray(Wo, np.float32)
    w_alpha = np.asarray(w_alpha, np.float32); w_beta = np.asarray(w_beta, np.float32)
    bo = np.asarray(bo, np.float32)
    ln_g = np.asarray(ln_g, np.float32); ln_b = np.asarray(ln_b, np.float32)

    assert np.all(scopes[:, 1] == L), "equal-length contiguous scopes expected"
    starts = scopes[:, 0].astype(np.int64)

    common, apply_bo, apply_affine = _host_consts(
        Wq, Wk, Wv, Wr, w_alpha, w_beta, Wo, bo, ln_g, ln_b)
    nc = _get_nc(apply_bo, apply_affine)

    bf = ml_dtypes.bfloat16
    f8 = ml_dtypes.float8_e4m3fn
    in_maps = []
    for c in range(NCORES):
        rows = np.concatenate([
            np.arange(starts[c * G + g], starts[c * G + g] + L)
            for g in range(G)])
        xslab = edge_attr[rows]                       # [G*L, 512]
        xt = np.ascontiguousarray(xslab.T)
        m = {"xtb": xt.astype(bf), **common}
        if USE_FP8:
            x8 = xt.astype(f8)
            m["xt8"] = x8
            m["dxt8"] = (xt - x8.astype(np.float32)).astype(f8)
        in_maps.append(m)

    res = bass_utils.run_bass_kernel_spmd(nc, in_maps, core_ids=list(range(NCORES)))
    out = np.concatenate([r["out"] for r in res.results], axis=0)
    return out.astype(np.float32)
